# revision 109
# baseline (speedup 1.0000x reference)
"""Trainium2 Bass kernel for the EngramNew module (dense_cnn), v3.

Sharding: B*T = 8192 tokens split across 8 cores (1024 tokens each); the conv
halo of (K-1)*DIL = 9 tokens is precomputed host-side.  On-device layout is
channels-on-partitions / tokens-on-free: [G*C, T_core].

v10 design vs the v2 baseline (291.8us -> 225.1us):
 - shared rms_v normalizer: rms_v = sqrt(gate^2*mean(vproj^2)+eps)
   ~= gate*sqrt(mean(vproj^2)+eps) since gate = sigmoid(..) > 0, so the
   conv input (normed) = vproj*alpha with ONE shared alpha row; only the
   residual needs the per-group gate.  This decouples the whole conv
   pipeline from the gate chains (error <= ~1e-3, verified vs reference).
 - ONE [16,NTOK] PSUM accumulator shared by all four gate-sum stages via
   disjoint one-hot rows (ak_g=g, aq_g=4+g, dot_g=8+g, sv=12), reset once:
   no PSUM bank-rotation WAR stalls.  Rows are bounced to partition 0 by
   an Act copy + f32r one-hot extraction matmuls (engines can only address
   32-aligned partition bases).
 - per-ct conv input tiles (zero prefix + vproj*alpha) shared by all 4
   groups; the halo contribution to the first 9 outputs is a host-computed
   correction added via an identity matmul.  15 of 32 conv units run as
   DVE MAC chains, the rest as PE diag matmuls.
 - C(3) is split into two per-chunk passes so chunk 0 of the stage-3 gate
   chain + epilogue hides under the chunk-1 pass; 3 conv units are held
   back as PE cover for the chunk-1 chain.
 - kq / ksq+qsq(fp8 DR) / vsq(fp8 DR) reductions all deferred by one
   gg/vv so their producers never stall the PE sum matmuls.
 - startup: split vw/emb DMAs ordered first + 4-psum et-outer first vproj
   pass so PE starts at ~4us instead of 13us.
"""

import os
import sys

for _p in ("/opt/trn_rl_repo",):
    if _p not in sys.path:
        sys.path.insert(0, _p)

import numpy as np
import ml_dtypes

import concourse.bass as bass
from concourse import mybir
from concourse.tile import TileContext
from concourse.bass_utils import run_bass_kernel_spmd
import bass_rust

F32 = mybir.dt.float32
F32R = mybir.dt.float32r
F16 = mybir.dt.float16
FP8 = mybir.dt.float8e4
AF = mybir.ActivationFunctionType
ALU = mybir.AluOpType
DR = mybir.MatmulPerfMode.DoubleRow

# Problem constants (hardcoded per spec nn_EngramNew_2070174237244)
B, T, G, C, E = 2, 4096, 4, 1024, 1024
GC = G * C
KT, DIL = 4, 3          # conv taps / dilation
EPS = 1e-5
NORM_EPS = 1e-5
NCORES = 8
NTOK = (B * T) // NCORES    # 1024 tokens per core
HALO = (KT - 1) * DIL       # 9
NET = E // 128              # 8 e-tiles
NGCT = GC // 128            # 32 gc-tiles
NCT = C // 128              # 8 c-tiles
CHW = 512                   # token chunk width (1 PSUM bank of fp32)
NCH = NTOK // CHW           # 2 chunks



class PatchedTileContext(TileContext):
    """This walrus build allows only one sem wait per instruction (two on
    EventSemaphore). Tile attaches as many waits as an instruction needs,
    so after scheduling we hoist excess waits onto no-op instructions
    inserted just before the owner on the same engine (engines are strict
    FIFO, so observing the sems earlier is equivalent)."""

    def _split_excess_waits(self):
        nc = self.nc

        def make_nop(engine):
            bi = nc.engines[engine].nop()
            bb = nc.cur_bb.bb
            lst = list(bb.instructions)
            assert lst[-1] is bi.ins
            bb.instructions = lst[:-1]
            return bi.ins

        # Phase 1: snapshot every block BEFORE creating any nop, so nops
        # appended to cur_bb can never leak into the iteration or the rebuilt
        # lists (cur_bb may be one of the blocks being processed).
        snapshots = []
        for f in nc.m.functions:
            for blk in f.blocks:
                snapshots.append((blk, list(blk.instructions)))

        for blk, insts in snapshots:
            out = []
            changed = False
            for ins in insts:
                si = ins.sync_info
                waits = list(si.on_wait) if (si and si.on_wait) else []
                cap = 2 if isinstance(ins, mybir.InstEventSemaphore) else 1
                if len(waits) > cap:
                    changed = True
                    for w in waits[cap:]:
                        nop = make_nop(ins.engine)
                        nop.sync_info = bass_rust.SyncInfo(
                            on_wait=[w], on_update=[]
                        )
                        out.append(nop)
                    upd = list(si.on_update) if si.on_update else []
                    ins.sync_info = bass_rust.SyncInfo(
                        on_wait=waits[:cap], on_update=upd
                    )
                out.append(ins)
            if changed:
                blk.instructions = out

    def _drain_and_barrier(self, tick_clock, wait_clock):
        super()._drain_and_barrier(tick_clock, wait_clock)
        self._split_excess_waits()


def _r(ap):
    return ap.bitcast(F32R)


def build_program():
    nc = bass.Bass()

    # ---- DRAM parameters ----
    emb16 = nc.declare_dram_parameter("emb16", [E, NTOK], F16, isOutput=False)
    hidT = nc.declare_dram_parameter("hidT", [GC, NTOK], F16, isOutput=False)
    kwT = nc.declare_dram_parameter("kwT", [E, GC], F16, isOutput=False)
    vwT = nc.declare_dram_parameter("vwT", [E, C], F16, isOutput=False)
    keyb = nc.declare_dram_parameter("keyb", [128, NGCT], F32, isOutput=False)
    valb = nc.declare_dram_parameter("valb", [128, NCT], F32, isOutput=False)
    lk8 = nc.declare_dram_parameter("lk8", [128, 9 * 2 * 16], FP8,
                                    isOutput=False)
    lkq = nc.declare_dram_parameter("lkq", [NGCT, 128, 16], F32, isOutput=False)
    aux16 = nc.declare_dram_parameter("aux16", [128, 16 + 256], F16,
                                      isOutput=False)
    ceps = nc.declare_dram_parameter("ceps", [128, 24], F32, isOutput=False)
    dg16 = nc.declare_dram_parameter("dg16", [NGCT, 128, KT * 128], F16,
                                     isOutput=False)
    cwf = nc.declare_dram_parameter("cwf", [128, NGCT * KT], F32,
                                    isOutput=False)
    hc = nc.declare_dram_parameter("hc", [128, NGCT * HALO], F16,
                                   isOutput=False)
    id16 = nc.declare_dram_parameter("id16", [128, 128], F16, isOutput=False)
    out_d = nc.declare_dram_parameter("out", [GC, NTOK], F16, isOutput=True)

    with PatchedTileContext(nc) as tc:
        consts = tc.alloc_tile_pool(name="consts", bufs=1)
        kwpool = tc.alloc_tile_pool(name="kwpool", bufs=2)
        qpool = tc.alloc_tile_pool(name="qpool", bufs=3)
        mmp = tc.alloc_tile_pool(name="mmp", bufs=3, space=bass.MemorySpace.PSUM)
        sump = tc.alloc_tile_pool(name="sump", bufs=1, space=bass.MemorySpace.PSUM)
        epsum = tc.alloc_tile_pool(name="epsum", bufs=3,
                                   space=bass.MemorySpace.PSUM)
        scr = tc.alloc_tile_pool(name="scr", bufs=4)
        kqpool = tc.alloc_tile_pool(name="kqpool", bufs=4)
        rowm = tc.alloc_tile_pool(name="rowm", bufs=1)
        rowsc = tc.alloc_tile_pool(name="rowsc", bufs=9)
        npool = tc.alloc_tile_pool(name="npool", bufs=3)
        vpool = tc.alloc_tile_pool(name="vpool", bufs=3)
        opool = tc.alloc_tile_pool(name="opool", bufs=4)
        dgpool = tc.alloc_tile_pool(name="dgpool", bufs=2)
        cacc = tc.alloc_tile_pool(name="cacc", bufs=2)

        # ---- load order: vw(vv0) first, then emb per-et, then small consts
        vw_t0 = kwpool.tile([128, NET, 256], F16, name="vw_t0", tag="w")
        for eh in range(2):
            nc.sync.dma_start(
                out=vw_t0[:, eh * 4:(eh + 1) * 4, :],
                in_=vwT.rearrange("(et p) c -> p et c", p=128)[
                    :, eh * 4:(eh + 1) * 4, 0:256],
            )
        emb_all = consts.tile([128, NET, NTOK], F16)
        for et in range(NET):
            nc.sync.dma_start(out=emb_all[:, et, :],
                              in_=emb16[et * 128:(et + 1) * 128, :])
        valb_sb = consts.tile([128, NCT], F32)
        nc.sync.dma_start(out=valb_sb, in_=valb[:, :])
        aux_sb = consts.tile([128, 16 + 256], F16)
        nc.sync.dma_start(out=aux_sb, in_=aux16[:, :])
        ceps_sb = consts.tile([128, 24], F32)
        nc.sync.dma_start(out=ceps_sb, in_=ceps[:, :])
        cepr_sb = consts.tile([128, 24], F32R)
        nc.sync.dma_start(out=cepr_sb, in_=_r(ceps[:, :]))
        keyb_sb = consts.tile([128, NGCT], F32)
        nc.sync.dma_start(out=keyb_sb, in_=keyb[:, :])
        lk8_sb = consts.tile([128, 9, 2, 16], FP8)
        nc.sync.dma_start(out=lk8_sb,
                          in_=lk8.rearrange("p (q i c) -> p q i c", i=2, c=16))
        lkq_sb = consts.tile([128, NGCT, 16], F32R)
        nc.sync.dma_start(out=lkq_sb, in_=_r(lkq.rearrange("n p m -> p n m")))
        cwf_sb = consts.tile([128, NGCT * KT], F32)
        nc.sync.dma_start(out=cwf_sb, in_=cwf[:, :])
        hc_sb = consts.tile([128, NGCT, HALO], F16)
        nc.sync.dma_start(out=hc_sb,
                          in_=hc.rearrange("p (n h) -> p n h", h=HALO))
        id16_sb = consts.tile([128, 128], F16)
        nc.sync.dma_start(out=id16_sb, in_=id16[:, :])
        vproj16 = consts.tile([128, NCT, NTOK], F16)
        bc2_sb = aux_sb[0:1, 16:16 + 128]

        # ---- gate sums: ONE [16, NTOK] psum shared by all stages via
        # disjoint one-hot rows: ak_g = row g, aq_g = 4+g, dot_g = 8+g,
        # sv = 12. Reset once (B's first vsq sum); everything accumulates.
        sums_all = sump.tile([16, NTOK], F32, name="sums_all", tag="sums")
        first_sum = [True] * NCH

        def sum_mm(stage, lhsT, rhs, ch, last=False, perf_mode=None):
            st = first_sum[ch]
            first_sum[ch] = False
            nc.tensor.matmul(
                sums_all[:, ch * CHW:(ch + 1) * CHW],
                lhsT, rhs, start=st, stop=last,
                perf_mode=perf_mode, skip_group_check=True,
            )

        # ---------- stage B: vproj = value_w @ emb + value_b ----------
        # vsq in fp8 (feeds only alpha), DoubleRow-reduced, deferred one vv
        pend_vsq = None

        def flush_vsq(v8):
            for ch in range(NCH):
                cols = slice(ch * CHW, (ch + 1) * CHW)
                sum_mm(3, lk8_sb[:, 8, :, :], v8[:, :, cols], ch,
                       perf_mode=DR)

        for vv in range(NCT // 2):
            if vv == 0:
                vw_t = vw_t0
            else:
                vw_t = kwpool.tile([128, NET, 256], F16, name="vw_t", tag="w")
                nc.sync.dma_start(
                    out=vw_t,
                    in_=vwT.rearrange("(et p) c -> p et c", p=128)[
                        :, :, vv * 256:(vv + 1) * 256],
                )
            vsq = scr.tile([128, 2, NTOK], FP8, name="vsq8", tag="p8")
            if vv == 0:
                # et-outer across 4 psums so PE rate-matches the emb DMAs
                ps4 = [mmp.tile([128, CHW], F32, name=f"psB0_{i}", tag="mm")
                       for i in range(3)]
                ps4.append(epsum.tile([128, CHW], F32, name="psB0_3",
                                      tag="mm"))
                for et in range(NET):
                    for i in range(4):
                        s2, ch = i // 2, i % 2
                        nc.tensor.matmul(
                            ps4[i],
                            vw_t[:, et, s2 * 128:(s2 + 1) * 128],
                            emb_all[:, et, ch * CHW:(ch + 1) * CHW],
                            start=(et == 0), stop=(et == NET - 1),
                        )
                for i in range(4):
                    s2, ch = i // 2, i % 2
                    ct = vv * 2 + s2
                    cols = slice(ch * CHW, (ch + 1) * CHW)
                    nc.scalar.activation(
                        vproj16[:, ct, cols], ps4[i],
                        AF.Identity, bias=valb_sb[:, ct:ct + 1], scale=1.0,
                    )
                    nc.scalar.activation(
                        vsq[:, s2, cols], ps4[i], AF.Square,
                        bias=valb_sb[:, ct:ct + 1], scale=1.0,
                    )
            else:
                for s2 in range(2):
                    ct = vv * 2 + s2
                    for ch in range(NCH):
                        cols = slice(ch * CHW, (ch + 1) * CHW)
                        ps = mmp.tile([128, CHW], F32, name="psB", tag="mm")
                        for et in range(NET):
                            nc.tensor.matmul(
                                ps,
                                vw_t[:, et, s2 * 128:(s2 + 1) * 128],
                                emb_all[:, et, ch * CHW:(ch + 1) * CHW],
                                start=(et == 0), stop=(et == NET - 1),
                            )
                        nc.scalar.activation(
                            vproj16[:, ct, cols], ps,
                            AF.Identity, bias=valb_sb[:, ct:ct + 1], scale=1.0,
                        )
                        nc.scalar.activation(
                            vsq[:, s2, cols], ps, AF.Square,
                            bias=valb_sb[:, ct:ct + 1], scale=1.0,
                        )
                if pend_vsq is not None:
                    flush_vsq(pend_vsq)
                    pend_vsq = None
            pend_vsq = vsq
        flush_vsq(pend_vsq)

        # ---------- stage C for one group-pair ----------
        def emit_c_kq(stage, gg):
            """k path for double-gct gg (two gc tiles); DR sums deferred."""
            kw_t = kwpool.tile([128, NET, 256], F16, name="kw_t", tag="w")
            nc.sync.dma_start(
                out=kw_t,
                in_=kwT.rearrange("(et p) c -> p et c", p=128)[
                    :, :, gg * 256:(gg + 1) * 256],
            )
            ksqp = scr.tile([128, 2, NTOK], FP8, name="ksqp", tag="p8")
            qsqp = scr.tile([128, 2, NTOK], FP8, name="qsqp", tag="p8")
            kqs = []
            for s2 in range(2):
                gct = gg * 2 + s2
                q_sb = qpool.tile([128, NTOK], F16, name="q_sb", tag="q")
                nc.sync.dma_start(
                    out=q_sb, in_=hidT[gct * 128:(gct + 1) * 128, :]
                )
                kq = kqpool.tile([128, NTOK], F32R, name="kq", tag="kq")
                for ch in range(NCH):
                    ps = mmp.tile([128, CHW], F32, name="psC", tag="mm")
                    for et in range(NET):
                        nc.tensor.matmul(
                            ps,
                            kw_t[:, et, s2 * 128:(s2 + 1) * 128],
                            emb_all[:, et, ch * CHW:(ch + 1) * CHW],
                            start=(et == 0), stop=(et == NET - 1),
                        )
                    cols = slice(ch * CHW, (ch + 1) * CHW)
                    nc.scalar.activation(
                        ksqp[:, s2, cols], ps, AF.Square,
                        bias=keyb_sb[:, gct:gct + 1], scale=1.0,
                    )
                    nc.gpsimd.tensor_mul(qsqp[:, s2, cols], q_sb[:, cols],
                                         q_sb[:, cols])
                    nc.vector.scalar_tensor_tensor(
                        kq[:, cols], ps, keyb_sb[:, gct:gct + 1],
                        q_sb[:, cols], op0=ALU.add, op1=ALU.mult,
                    )
                kqs.append((gct, kq))
            return ksqp, qsqp, kqs

        def emit_dr(stage, ksqp, qsqp, kqs, last_gg):
            for gct, kq in kqs:
                for ch in range(NCH):
                    sum_mm(stage, lkq_sb[:, gct, :],
                           kq[:, ch * CHW:(ch + 1) * CHW], ch)
            for ch in range(NCH):
                cols = slice(ch * CHW, (ch + 1) * CHW)
                sum_mm(stage, lk8_sb[:, stage, :, :], ksqp[:, :, cols], ch,
                       perf_mode=DR)
                sum_mm(stage, lk8_sb[:, 4 + stage, :, :], qsqp[:, :, cols],
                       ch, last=last_gg, perf_mode=DR)

        def emit_c_kq1(stage, gg, ch):
            """Single-chunk variant (window-3 ch-split passes)."""
            cols = slice(ch * CHW, (ch + 1) * CHW)
            kw_t = kwpool.tile([128, NET, 256], F16, name="kw_t", tag="w")
            nc.sync.dma_start(
                out=kw_t,
                in_=kwT.rearrange("(et p) c -> p et c", p=128)[
                    :, :, gg * 256:(gg + 1) * 256],
            )
            ksqp = scr.tile([128, 2, CHW], FP8, name="ksqp1", tag="p8")
            qsqp = scr.tile([128, 2, CHW], FP8, name="qsqp1", tag="p8")
            kqs = []
            for s2 in range(2):
                gct = gg * 2 + s2
                q_sb = qpool.tile([128, CHW], F16, name="q_sb1", tag="q")
                nc.sync.dma_start(
                    out=q_sb, in_=hidT[gct * 128:(gct + 1) * 128, cols]
                )
                kq = kqpool.tile([128, CHW], F32R, name="kq1", tag="kq")
                ps = mmp.tile([128, CHW], F32, name="psC", tag="mm")
                for et in range(NET):
                    nc.tensor.matmul(
                        ps,
                        kw_t[:, et, s2 * 128:(s2 + 1) * 128],
                        emb_all[:, et, cols],
                        start=(et == 0), stop=(et == NET - 1),
                    )
                nc.scalar.activation(
                    ksqp[:, s2, :], ps, AF.Square,
                    bias=keyb_sb[:, gct:gct + 1], scale=1.0,
                )
                nc.gpsimd.tensor_mul(qsqp[:, s2, :], q_sb, q_sb)
                nc.vector.scalar_tensor_tensor(
                    kq, ps, keyb_sb[:, gct:gct + 1],
                    q_sb, op0=ALU.add, op1=ALU.mult,
                )
                kqs.append((gct, kq))
            return ksqp, qsqp, kqs

        def emit_dr1(stage, ksqp, qsqp, kqs, ch, last_gg):
            for gct, kq in kqs:
                sum_mm(stage, lkq_sb[:, gct, :], kq, ch)
            sum_mm(stage, lk8_sb[:, stage, :, :], ksqp, ch, perf_mode=DR)
            sum_mm(stage, lk8_sb[:, 4 + stage, :, :], qsqp, ch, last=last_gg,
                   perf_mode=DR)

        # ---------- stage D ----------
        # Shared rms_v normalizer: rms_v = sqrt(gate^2*mean(vproj^2)+eps)
        # ~= gate*sqrt(mean(vproj^2)+eps) since gate=sigmoid(..)>0, so the
        # conv input normed = vproj*alpha with ONE shared alpha row; only the
        # residual (value = vproj*gate) needs the per-group gate.
        def emit_alpha():
            # sv (= sum vproj^2) sits at psum row 12: bounce the block to
            # SBUF and matmul-extract the row to partition 0.
            s3a = rowm.tile([16, NTOK], F32R, name="s3a", tag="svz")
            aln = rowsc.tile([1, NTOK], F32, name="aln", tag="rs")
            alpha16 = rowm.tile([1, NTOK], F16, name="alpha16", tag="alpha16")
            nc.scalar.activation(s3a, sums_all[:, :], AF.Copy)
            for ch in range(NCH):
                cols = slice(ch * CHW, (ch + 1) * CHW)
                p = epsum.tile([1, CHW], F32, name="svx", tag="mm")
                nc.tensor.matmul(p, cepr_sb[0:16, 20:21], s3a[:, cols],
                                 start=True, stop=True)
                nc.scalar.activation(aln[:, cols], p, AF.Ln,
                                     bias=ceps_sb[0:1, 6:7],
                                     scale=1.0 / float(C))
            nc.scalar.activation(alpha16, aln, AF.Exp, scale=-0.5)
            return alpha16

        def make_d_tiles(stage):
            T = {}
            for nm in ("p4", "lnp", "lnd", "lng", "sqg", "sgn", "ss4", "ab4",
                       "akr"):
                T[nm] = rowsc.tile([1, NTOK], F32, name=f"{nm}{stage}",
                                   tag="rs")
            T["gate16"] = rowm.tile([1, NTOK], F16, name=f"gate16{stage}",
                                    tag="gate16")
            T["s3"] = rowm.tile([16, NTOK], F32R, name=f"s3_{stage}",
                                tag="ext")
            return T

        def emit_d_s3(stage, T, chs=(0, 1)):
            """Psum sums -> partition-0-based SBUF bounce (+ stage biases)."""
            for ch in chs:
                sl = slice(ch * CHW, (ch + 1) * CHW)
                nc.scalar.activation(T["s3"][:, sl], sums_all[:, sl],
                                     AF.Identity,
                                     bias=ceps_sb[0:16, stage:stage + 1],
                                     scale=1.0)

        def emit_d(stage, T, mul_eng=None, chs=(0, 1)):
            """Per-group gate chain: gate = sigmoid(sign(dot)*sqrt(|graw|)).

            Engines only address partitions at 32-boundaries, so the psum
            region is Act-copied (aligned base -> partition 0) to s3, and
            rows 1+ are pulled to partition-0 psum via one-hot matmuls.
            Row layout: stages 0-2: [ak, aq, dot]; stage 3: [sv, aq, dot, ak].
            """
            me = mul_eng if mul_eng is not None else nc.vector
            s3 = T["s3"]
            p4, lnp, lnd, lng, sqg, sgn, ss4, ab4, gate16 = (
                T["p4"], T["lnp"], T["lnd"], T["lng"], T["sqg"], T["sgn"],
                T["ss4"], T["ab4"], T["gate16"])
            akr = T["akr"]
            if chs == (0, 1):
                sls = [slice(0, NTOK)]
            else:
                sls = [slice(ch * CHW, (ch + 1) * CHW) for ch in chs]

            def extract(row, ch):
                sel = cepr_sb[0:16, 8 + row:9 + row]
                p = epsum.tile([1, CHW], F32, name=f"x{row}_{stage}",
                               tag="mm")
                nc.tensor.matmul(p, sel,
                                 s3[:, ch * CHW:(ch + 1) * CHW],
                                 start=True, stop=True)
                return p

            # first layer reads the [1, CHW] psums (partition 0), per chunk
            for ch in chs:
                cols = slice(ch * CHW, (ch + 1) * CHW)
                ak_ps = extract(stage, ch)
                aq_ps = extract(4 + stage, ch)
                dot_ps = extract(8 + stage, ch)
                nc.scalar.activation(akr[:, cols], ak_ps, AF.Copy)
                nc.scalar.activation(ab4[:, cols], dot_ps, AF.Square)
                nc.scalar.activation(sgn[:, cols], dot_ps, AF.Sign)
                nc.vector.tensor_mul(p4[:, cols], akr[:, cols], aq_ps)
            # 2ln|dot| and ln(p4/C); 2ln|graw| = 2ln|dot| - ln(p4/C)
            # (plain subtract so the mul engine can be Pool)
            for sl in sls:
                nc.scalar.activation(lnd[:, sl], ab4[:, sl], AF.Ln,
                                     bias=ceps_sb[0:1, 7:8])
            for sl in sls:
                nc.scalar.activation(lnp[:, sl], p4[:, sl], AF.Ln,
                                     scale=1.0 / float(C))
            for sl in sls:
                me.tensor_sub(lng[:, sl], lnd[:, sl], lnp[:, sl])
            for sl in sls:
                nc.scalar.activation(sqg[:, sl], lng[:, sl], AF.Exp,
                                     scale=0.25)
            for sl in sls:
                me.tensor_mul(ss4[:, sl], sqg[:, sl], sgn[:, sl])
            for sl in sls:
                nc.scalar.activation(gate16[:, sl], ss4[:, sl], AF.Sigmoid)
            return gate16

        # ---------- stage E ----------
        def bcast_ch(src, dst, ch):
            bp = epsum.tile([128, CHW], F32, name="bp", tag="mm")
            nc.tensor.matmul(
                bp, bc2_sb[0:1, 0:128],
                src[:, ch * CHW:(ch + 1) * CHW],
                start=True, stop=True,
            )
            nc.scalar.activation(
                dst[:, ch * CHW:(ch + 1) * CHW], bp, AF.Copy)

        def bcast_row(src, tag):
            """[1, NTOK] f32/f16 row -> [128, NTOK] f16 via PE broadcast."""
            dst = rowm.tile([128, NTOK], F16, name=f"b_{tag}", tag=tag)
            for ch in range(NCH):
                bcast_ch(src, dst, ch)
            return dst

        # nx16[ct]: f16 conv input, shared by all 4 groups' units:
        # [9 zeros | vproj*alpha]; the halo contribution to the first 9
        # outputs is a host-computed f16 correction (hc) accumulated via an
        # identity matmul.
        PADW = HALO + NTOK
        nx8s = {}

        def emit_nx8(ct):
            nx8 = npool.tile([128, PADW], F16, name=f"nx16_{ct}",
                             tag=f"nx16_{ct}", bufs=1)
            nc.gpsimd.memset(nx8[:, 0:HALO], 0.0)
            nc.vector.tensor_mul(nx8[:, HALO:HALO + NTOK],
                                 vproj16[:, ct, :], ab16)
            nx8s[ct] = nx8

        def emit_val(gct, gb16, on_pool=False):
            ct = gct % NCT
            val = vpool.tile([128, NTOK], F16, name="val", tag="val")
            if on_pool:
                nc.gpsimd.tensor_mul(val, vproj16[:, ct, :], gb16)
            else:
                nc.vector.tensor_mul(val, vproj16[:, ct, :], gb16)
            return val

        def emit_e_conv_pe(gct, pools=None):
            """f16 conv taps + halo-fix matmul."""
            ct = gct % NCT
            nx8 = nx8s[ct]
            dg_t = dgpool.tile([128, KT * 128], F16, name="dg_t", tag="dg")
            nc.sync.dma_start(out=dg_t, in_=dg16[gct])
            accs = []
            for ch in range(NCH):
                pool = (pools[ch % len(pools)] if pools else epsum)
                acc = pool.tile([128, CHW], F32, name="acc", tag="mm")
                for k in range(KT):
                    base = ch * CHW + k * DIL
                    nc.tensor.matmul(
                        acc,
                        dg_t[:, k * 128:(k + 1) * 128],
                        nx8[:, base:base + CHW],
                        start=(k == 0), stop=(k == KT - 1 and ch == 1),
                        skip_group_check=True,
                    )
                if ch == 0:
                    nc.tensor.matmul(
                        acc[:, 0:HALO], id16_sb, hc_sb[:, gct, :],
                        start=False, stop=True, skip_group_check=True,
                    )
                accs.append(acc)
            return accs

        def emit_e_conv_dve(gct):
            """f16 conv as DVE scalar-ptr MAC chains (+ in-place halo fix)."""
            ct = gct % NCT
            nx8 = nx8s[ct]
            outs = []
            for ch in range(NCH):
                prev = None
                for k in range(KT):
                    win = nx8[:, ch * CHW + k * DIL:ch * CHW + k * DIL + CHW]
                    a = cacc.tile([128, CHW], F16, name=f"ca{k}", tag=f"ca{k}")
                    wcol = cwf_sb[:, gct * KT + k:gct * KT + k + 1]
                    if k == 0:
                        nc.vector.tensor_scalar_mul(a, win, wcol)
                    else:
                        nc.vector.scalar_tensor_tensor(
                            a, win, wcol, prev, op0=ALU.mult, op1=ALU.add)
                    prev = a
                if ch == 0:
                    nc.vector.tensor_tensor(prev[:, 0:HALO], prev[:, 0:HALO],
                                            hc_sb[:, gct, :], op=ALU.add)
                outs.append(prev)
            return outs

        def emit_silu(accs):
            sacc = opool.tile([128, NTOK], F16, name="sacc", tag="sacc")
            for ch in range(NCH):
                nc.scalar.activation(sacc[:, ch * CHW:(ch + 1) * CHW],
                                     accs[ch], AF.Silu)
            return sacc

        def emit_resid_out(gct, val, sacc, engine="pool"):
            ot = opool.tile([128, NTOK], F16, name="ot", tag="ot")
            if engine == "dve":
                nc.vector.tensor_tensor(ot, val, sacc, op=ALU.add)
            else:
                nc.gpsimd.tensor_add(ot, val, sacc)
            nc.sync.dma_start(out=out_d[gct * 128:(gct + 1) * 128, :], in_=ot)

        # ---------- pipeline ----------
        # conv+silu only needs the shared ab16; val/resid needs gate(g).
        # Window g: C(g) + chain(g-1) + full units of group g-1 + a few
        # group-3 conv units pulled early; tail: 3 conv units cover chain(3),
        # then group-3 val/resid.
        sacc3 = {}      # gct -> long-lived sacc for group-3 units
        ab16 = None
        TAIL3 = [29, 30, 31]
        EARLY3 = {0: [24, 25, 26], 1: [27], 2: [28], 3: []}

        def conv_unit(u, long_lived=False, pools=None, defer_silu=False,
                      dve=False):
            if dve:
                accs = emit_e_conv_dve(u)
            else:
                accs = emit_e_conv_pe(u, pools=pools)
            if defer_silu:
                return accs
            if long_lived:
                sacc = opool.tile([128, NTOK], F16, name=f"sacc{u}",
                                  tag=f"sacc3_{u}", bufs=1)
            else:
                sacc = opool.tile([128, NTOK], F16, name=f"sacc{u}",
                                  tag="sacc")
            for ch in range(NCH):
                nc.scalar.activation(sacc[:, ch * CHW:(ch + 1) * CHW],
                                     accs[ch], AF.Silu)
            if long_lived:
                sacc3[u] = sacc
            return sacc

        def full_unit(u, gb16, dve=False):
            sacc = conv_unit(u, dve=dve)
            val = emit_val(u, gb16)
            emit_resid_out(u, val, sacc, engine="pool" if dve else "dve")

        gate_prev = None
        d_tiles = {}
        for g in range(3):
            dr_prev = None
            gb16 = None
            units = list(range((g - 1) * 8, g * 8)) if g else []
            for i, gg in enumerate(range(g * 4, (g + 1) * 4)):
                if i == 0 and g:
                    d_tiles[g - 1] = make_d_tiles(g - 1)
                    emit_d_s3(g - 1, d_tiles[g - 1])
                cur = emit_c_kq(g, gg)
                if i == 0:
                    if g == 0:
                        alpha16 = emit_alpha()
                    else:
                        gate_prev = emit_d(g - 1, d_tiles[g - 1],
                                           mul_eng=nc.gpsimd)
                if dr_prev is not None:
                    emit_dr(g, *dr_prev, last_gg=False)
                dr_prev = cur
                if i == 1:
                    if g == 0:
                        ab16 = bcast_row(alpha16, "ab16")
                        for ct in (0, 1, 2):
                            emit_nx8(ct)
                    elif g == 1:
                        for ct in (6, 7):
                            emit_nx8(ct)
                    batch = []
                elif i == 2:
                    if g:
                        gb16 = bcast_row(gate_prev, f"gb{g - 1}")
                        batch = units[0:3]
                    else:
                        emit_nx8(3)
                        batch = EARLY3[0][0:2]
                elif i == 3:
                    if g == 0:
                        emit_nx8(4)
                        emit_nx8(5)
                    batch = units[3:6] if g else EARLY3[0][2:3]
                else:
                    batch = []
                for u in batch:
                    if g:
                        full_unit(u, gb16, dve=(u % 8 in (0, 2, 4)))
                    else:
                        conv_unit(u, long_lived=True, dve=True)
            emit_dr(g, *dr_prev, last_gg=True)
            if g:
                for u in units[6:8]:
                    full_unit(u, gb16, dve=(u % 8 == 6))
                for u in EARLY3[g]:
                    conv_unit(u, long_lived=True)

        # ---------- window 3: chunk-split passes ----------
        # pass p computes C(3) for token chunk p only, so the stage-3 gate
        # chain + group-3 epilogue for chunk 0 hide under pass 1.
        units = list(range(16, 24))
        d_tiles[2] = make_d_tiles(2)
        emit_d_s3(2, d_tiles[2])
        T3 = None
        gb3 = rowm.tile([128, NTOK], F16, name="b_gb3", tag="gb3")

        def epi3_ch(u, ch):
            ct = u % NCT
            cols = slice(ch * CHW, (ch + 1) * CHW)
            val = vpool.tile([128, CHW], F16, name="val3", tag="val")
            nc.vector.tensor_mul(val, vproj16[:, ct, cols], gb3[:, cols])
            ot = opool.tile([128, CHW], F16, name="ot3", tag="ot")
            nc.vector.tensor_tensor(ot, val, sacc3[u][:, cols], op=ALU.add)
            nc.sync.dma_start(out=out_d[u * 128:(u + 1) * 128, cols], in_=ot)

        for p in range(2):
            dr_prev = None
            for i, gg in enumerate(range(12, 16)):
                cur = emit_c_kq1(3, gg, p)
                if p == 0 and i == 0:
                    gate2 = emit_d(2, d_tiles[2], mul_eng=nc.gpsimd)
                if p == 1 and i == 0:
                    T3 = make_d_tiles(3)
                    emit_d_s3(3, T3, chs=(0,))
                    gate3 = emit_d(3, T3, mul_eng=nc.vector, chs=(0,))
                if dr_prev is not None:
                    emit_dr1(3, *dr_prev, p, last_gg=False)
                dr_prev = cur
                if p == 0:
                    if i == 2:
                        gb2 = bcast_row(gate2, "gb2")
                        batch = units[0:3]
                    elif i == 3:
                        batch = units[3:6]
                    else:
                        batch = []
                    for u in batch:
                        full_unit(u, gb2, dve=(u % 8 in (0, 2, 4)))
                else:
                    if i == 1:
                        bcast_ch(gate3, gb3, 0)
                        for u in units[6:8]:
                            full_unit(u, gb2, dve=(u % 8 == 6))
                    elif i == 2:
                        for u in range(24, 28):
                            epi3_ch(u, 0)
                    elif i == 3:
                        epi3_ch(28, 0)
            emit_dr1(3, *dr_prev, p, last_gg=True)

        # ---------- tail: chunk 1 of the group-3 gate + epilogue ----------
        # TAIL3 conv matmuls cover the chain; their silus follow its Act ops
        emit_d_s3(3, T3, chs=(1,))
        acc_pools = [epsum, mmp]
        emit_d(3, T3, mul_eng=nc.vector, chs=(1,))
        tail_accs = [conv_unit(u, pools=acc_pools, defer_silu=True)
                     for u in TAIL3]
        bcast_ch(gate3, gb3, 1)
        for j, u in enumerate(TAIL3):
            sacc = opool.tile([128, NTOK], F16, name=f"sacc{u}",
                              tag=f"sacc3_{u}", bufs=1)
            for ch in range(NCH):
                nc.scalar.activation(sacc[:, ch * CHW:(ch + 1) * CHW],
                                     tail_accs[j][ch], AF.Silu)
            sacc3[u] = sacc
        for u in TAIL3:
            epi3_ch(u, 0)
        for u in range(24, 32):
            epi3_ch(u, 1)

        for p in (cacc, dgpool, opool, vpool, npool, rowsc, rowm, kqpool, scr,
                  epsum, sump, mmp, qpool, kwpool, consts):
            p.release()
    return nc


def host_prep(embeddings, hidden_states, key_w, key_b, value_w, value_b,
              w_key_norm, w_query_norm, w_norm, conv_weight):
    """Build the per-core input maps."""
    f32, f16 = np.float32, np.float16
    e4 = ml_dtypes.float8_e4m3fn
    embeddings = np.asarray(embeddings, f32)
    hidden_states = np.asarray(hidden_states, f32)
    key_w = np.asarray(key_w, f32)
    key_b = np.asarray(key_b, f32)
    value_w = np.asarray(value_w, f32)
    value_b = np.asarray(value_b, f32)
    w_key_norm = np.asarray(w_key_norm, f32)
    w_query_norm = np.asarray(w_query_norm, f32)
    w_norm = np.asarray(w_norm, f32)
    conv_weight = np.asarray(conv_weight, f32)

    kwT = np.ascontiguousarray(key_w.T).astype(f16)        # [E, GC]
    vwT = np.ascontiguousarray(value_w.T).astype(f16)      # [E, C]
    keyb_r = np.ascontiguousarray(key_b.reshape(NGCT, 128).T)  # [128, NGCT]
    valb_r = np.ascontiguousarray(value_b.reshape(NCT, 128).T)
    wkq = (w_key_norm * w_query_norm).reshape(GC)

    # one-hot lhsT tables. ONE shared [16, NTOK] psum accumulator with
    # disjoint rows: ak_g = row g, aq_g = 4+g, dot_g = 8+g, sv = 12.
    # (engines can only address 32-aligned partition bases, so rows are
    #  matmul-extracted after an Act bounce of the block to partition 0)
    lk8 = np.zeros((128, 9, 2, 16), f32)
    for g in range(G):
        lk8[:, g, :, g] = 1.0          # ksq -> row g
        lk8[:, 4 + g, :, 4 + g] = 1.0  # qsq -> row 4+g
    lk8[:, 8, :, 12] = 1.0             # vsq -> row 12 (sv)
    lk8 = lk8.reshape(128, 288).astype(e4)

    lkq = np.zeros((NGCT, 128, 16), f32)
    for gct in range(NGCT):
        g = gct // NCT
        lkq[gct, :, 8 + g] = wkq[gct * 128:(gct + 1) * 128]

    aux16 = np.zeros((128, 16 + 256), f16)
    aux16[:, 12] = 1.0        # lv one-hot: vsq -> row 12 (sv)
    for j in range(2):
        aux16[j, 16 + j * 128:16 + (j + 1) * 128] = 1.0

    # ceps: cols 0-3 = per-stage bias vectors (+C*EPS on ak/aq rows);
    #        cols 8+r = f32 one-hot row selectors (identity)
    ceps_h = np.zeros((128, 24), f32)
    for g in range(G):
        ceps_h[g, g] = float(C) * EPS
        ceps_h[4 + g, g] = float(C) * EPS
    for r in range(16):
        ceps_h[r, 8 + r] = 1.0
    ceps_h[0, 6] = NORM_EPS
    ceps_h[0, 7] = 1e-60

    # f16 diagonal conv weights + identity for the halo-fix matmul.
    cwf = (conv_weight.reshape(G, C, KT) * w_norm[:, :, None]).astype(f32)
    dg = np.zeros((NGCT, 128, KT * 128), f16)
    idx = np.arange(128)
    for gct in range(NGCT):
        g, ct = gct // NCT, gct % NCT
        for k in range(KT):
            dg[gct, idx, k * 128 + idx] = cwf[g, ct * 128 + idx, k].astype(f16)
    id16_h = np.zeros((128, 128), f16)
    id16_h[idx, idx] = 1.0
    cwf_r = np.zeros((128, NGCT * KT), f32)
    for gct in range(NGCT):
        g, ct = gct // NCT, gct % NCT
        for k in range(KT):
            cwf_r[:, gct * KT + k] = cwf[g, ct * 128:(ct + 1) * 128, k]

    in_maps = []
    for core in range(NCORES):
        b = core // (NCORES // B)
        t0 = (core % (NCORES // B)) * NTOK
        emb_s = embeddings[b, t0:t0 + NTOK]                # [NTOK, E]
        hid_s = hidden_states[b, t0:t0 + NTOK].reshape(NTOK, GC)
        emb_c = np.ascontiguousarray(emb_s.T).astype(f16)  # [E, NTOK]
        hid_c = np.ascontiguousarray(hid_s.T).astype(f16)  # [GC, NTOK]

        # halo: nhat (= value / rms_v, w_norm NOT applied) for the 9
        # preceding tokens feeds a host-computed conv correction hc for the
        # first 9 output tokens; zeros at the sequence start.
        if t0 == 0:
            hc_c = np.zeros((128, NGCT * HALO), f16)
        else:
            th = slice(t0 - HALO, t0)
            e9 = embeddings[b, th]                          # [9, E]
            k9 = (e9 @ key_w.T + key_b).reshape(HALO, G, C)
            q9 = hidden_states[b, th]                       # [9, G, C]
            rk = np.sqrt((k9 * k9).mean(-1) + EPS)
            rq = np.sqrt((q9 * q9).mean(-1) + EPS)
            d9 = np.einsum("tgc,gc,tgc,gc->tg", k9, w_key_norm, q9,
                           w_query_norm)
            graw = d9 / (rk * rq) / np.sqrt(f32(C))
            g9 = 1.0 / (1.0 + np.exp(-(np.where(graw >= 0, 1.0, -1.0)
                                       * np.sqrt(np.maximum(np.abs(graw),
                                                            1e-6)))))
            vp9 = e9 @ value_w.T + value_b                  # [9, C]
            val9 = vp9[:, None, :] * g9[..., None].astype(f32)
            rv9 = np.sqrt((val9 * val9).mean(-1) + NORM_EPS)
            nhat9 = val9 / rv9[..., None]                   # [9, G, C]
            # hc[c, gct, t] = sum_{k: t+k*DIL<9} cwf[g,c,k]*nhat9[t+k*DIL,g,c]
            hcf = np.zeros((HALO, G, C), f32)
            for t in range(HALO):
                for k in range(KT):
                    ix = t + k * DIL
                    if ix < HALO:
                        hcf[t] += cwf[:, :, k] * nhat9[ix]
            hg = hcf.transpose(1, 2, 0).reshape(NGCT, 128, HALO)
            hc_c = np.ascontiguousarray(
                hg.transpose(1, 0, 2).reshape(128, NGCT * HALO)).astype(f16)

        in_maps.append({
            "emb16": emb_c, "hidT": hid_c, "kwT": kwT, "vwT": vwT,
            "keyb": keyb_r, "valb": valb_r,
            "lk8": lk8, "lkq": lkq, "aux16": aux16, "ceps": ceps_h,
            "dg16": dg, "cwf": cwf_r, "hc": hc_c, "id16": id16_h,
        })
    return in_maps


_NC_CACHE = [None]
LAST_RESULT = [None]


def kernel(**inputs) -> np.ndarray:
    in_maps = host_prep(**inputs)
    if _NC_CACHE[0] is None:
        _NC_CACHE[0] = build_program()
    nc = _NC_CACHE[0]
    res = run_bass_kernel_spmd(nc, in_maps, list(range(NCORES)))
    LAST_RESULT[0] = res
    out = np.empty((B, T, G, C), np.float32)
    for core in range(NCORES):
        b = core // (NCORES // B)
        t0 = (core % (NCORES // B)) * NTOK
        oc = np.asarray(res.results[core]["out"]).astype(np.float32)
        out[b, t0:t0 + NTOK] = oc.reshape(G, C, NTOK).transpose(2, 0, 1)
    return out


# revision 113
# speedup vs baseline: 1.0011x; 1.0011x over previous
"""Trainium2 Bass kernel for the EngramNew module (dense_cnn), v3.

Sharding: B*T = 8192 tokens split across 8 cores (1024 tokens each); the conv
halo of (K-1)*DIL = 9 tokens is precomputed host-side.  On-device layout is
channels-on-partitions / tokens-on-free: [G*C, T_core].

v10 design vs the v2 baseline (291.8us -> 225.1us):
 - shared rms_v normalizer: rms_v = sqrt(gate^2*mean(vproj^2)+eps)
   ~= gate*sqrt(mean(vproj^2)+eps) since gate = sigmoid(..) > 0, so the
   conv input (normed) = vproj*alpha with ONE shared alpha row; only the
   residual needs the per-group gate.  This decouples the whole conv
   pipeline from the gate chains (error <= ~1e-3, verified vs reference).
 - ONE [16,NTOK] PSUM accumulator shared by all four gate-sum stages via
   disjoint one-hot rows (ak_g=g, aq_g=4+g, dot_g=8+g, sv=12), reset once:
   no PSUM bank-rotation WAR stalls.  Rows are bounced to partition 0 by
   an Act copy + f32r one-hot extraction matmuls (engines can only address
   32-aligned partition bases).
 - per-ct conv input tiles (zero prefix + vproj*alpha) shared by all 4
   groups; the halo contribution to the first 9 outputs is a host-computed
   correction added via an identity matmul.  15 of 32 conv units run as
   DVE MAC chains, the rest as PE diag matmuls.
 - C(3) is split into two per-chunk passes so chunk 0 of the stage-3 gate
   chain + epilogue hides under the chunk-1 pass; 3 conv units are held
   back as PE cover for the chunk-1 chain.
 - kq / ksq+qsq(fp8 DR) / vsq(fp8 DR) reductions all deferred by one
   gg/vv so their producers never stall the PE sum matmuls.
 - startup: split vw/emb DMAs ordered first + 4-psum et-outer first vproj
   pass so PE starts at ~4us instead of 13us.
"""

import os
import sys

for _p in ("/opt/trn_rl_repo",):
    if _p not in sys.path:
        sys.path.insert(0, _p)

import numpy as np
import ml_dtypes

import concourse.bass as bass
from concourse import mybir
from concourse.tile import TileContext
from concourse.bass_utils import run_bass_kernel_spmd
import bass_rust

F32 = mybir.dt.float32
F32R = mybir.dt.float32r
F16 = mybir.dt.float16
FP8 = mybir.dt.float8e4
AF = mybir.ActivationFunctionType
ALU = mybir.AluOpType
DR = mybir.MatmulPerfMode.DoubleRow

# Problem constants (hardcoded per spec nn_EngramNew_2070174237244)
B, T, G, C, E = 2, 4096, 4, 1024, 1024
GC = G * C
KT, DIL = 4, 3          # conv taps / dilation
EPS = 1e-5
NORM_EPS = 1e-5
NCORES = 8
NTOK = (B * T) // NCORES    # 1024 tokens per core
HALO = (KT - 1) * DIL       # 9
NET = E // 128              # 8 e-tiles
NGCT = GC // 128            # 32 gc-tiles
NCT = C // 128              # 8 c-tiles
CHW = 512                   # token chunk width (1 PSUM bank of fp32)
NCH = NTOK // CHW           # 2 chunks



class PatchedTileContext(TileContext):
    """This walrus build allows only one sem wait per instruction (two on
    EventSemaphore). Tile attaches as many waits as an instruction needs,
    so after scheduling we hoist excess waits onto no-op instructions
    inserted just before the owner on the same engine (engines are strict
    FIFO, so observing the sems earlier is equivalent)."""

    def _split_excess_waits(self):
        nc = self.nc

        def make_nop(engine):
            bi = nc.engines[engine].nop()
            bb = nc.cur_bb.bb
            lst = list(bb.instructions)
            assert lst[-1] is bi.ins
            bb.instructions = lst[:-1]
            return bi.ins

        # Phase 1: snapshot every block BEFORE creating any nop, so nops
        # appended to cur_bb can never leak into the iteration or the rebuilt
        # lists (cur_bb may be one of the blocks being processed).
        snapshots = []
        for f in nc.m.functions:
            for blk in f.blocks:
                snapshots.append((blk, list(blk.instructions)))

        for blk, insts in snapshots:
            out = []
            changed = False
            for ins in insts:
                si = ins.sync_info
                waits = list(si.on_wait) if (si and si.on_wait) else []
                cap = 2 if isinstance(ins, mybir.InstEventSemaphore) else 1
                if len(waits) > cap:
                    changed = True
                    for w in waits[cap:]:
                        nop = make_nop(ins.engine)
                        nop.sync_info = bass_rust.SyncInfo(
                            on_wait=[w], on_update=[]
                        )
                        out.append(nop)
                    upd = list(si.on_update) if si.on_update else []
                    ins.sync_info = bass_rust.SyncInfo(
                        on_wait=waits[:cap], on_update=upd
                    )
                out.append(ins)
            if changed:
                blk.instructions = out

    def _drain_and_barrier(self, tick_clock, wait_clock):
        super()._drain_and_barrier(tick_clock, wait_clock)
        self._split_excess_waits()


def _r(ap):
    return ap.bitcast(F32R)


def build_program():
    nc = bass.Bass()

    # ---- DRAM parameters ----
    emb16 = nc.declare_dram_parameter("emb16", [E, NTOK], F16, isOutput=False)
    hidT = nc.declare_dram_parameter("hidT", [GC, NTOK], F16, isOutput=False)
    kwT = nc.declare_dram_parameter("kwT", [E, GC], F16, isOutput=False)
    vwT = nc.declare_dram_parameter("vwT", [E, C], F16, isOutput=False)
    keyb = nc.declare_dram_parameter("keyb", [128, NGCT], F32, isOutput=False)
    valb = nc.declare_dram_parameter("valb", [128, NCT], F32, isOutput=False)
    lk8 = nc.declare_dram_parameter("lk8", [128, 9 * 2 * 16], FP8,
                                    isOutput=False)
    lkq = nc.declare_dram_parameter("lkq", [NGCT, 128, 16], F32, isOutput=False)
    aux16 = nc.declare_dram_parameter("aux16", [128, 16 + 256], F16,
                                      isOutput=False)
    ceps = nc.declare_dram_parameter("ceps", [128, 24], F32, isOutput=False)
    dg16 = nc.declare_dram_parameter("dg16", [NGCT, 128, KT * 128], F16,
                                     isOutput=False)
    cwf = nc.declare_dram_parameter("cwf", [128, NGCT * KT], F32,
                                    isOutput=False)
    hc = nc.declare_dram_parameter("hc", [128, NGCT * HALO], F16,
                                   isOutput=False)
    id16 = nc.declare_dram_parameter("id16", [128, 128], F16, isOutput=False)
    out_d = nc.declare_dram_parameter("out", [GC, NTOK], F16, isOutput=True)

    with PatchedTileContext(nc) as tc:
        consts = tc.alloc_tile_pool(name="consts", bufs=1)
        kwpool = tc.alloc_tile_pool(name="kwpool", bufs=2)
        qpool = tc.alloc_tile_pool(name="qpool", bufs=3)
        mmp = tc.alloc_tile_pool(name="mmp", bufs=3, space=bass.MemorySpace.PSUM)
        sump = tc.alloc_tile_pool(name="sump", bufs=1, space=bass.MemorySpace.PSUM)
        epsum = tc.alloc_tile_pool(name="epsum", bufs=3,
                                   space=bass.MemorySpace.PSUM)
        scr = tc.alloc_tile_pool(name="scr", bufs=4)
        kqpool = tc.alloc_tile_pool(name="kqpool", bufs=4)
        rowm = tc.alloc_tile_pool(name="rowm", bufs=1)
        rowsc = tc.alloc_tile_pool(name="rowsc", bufs=9)
        npool = tc.alloc_tile_pool(name="npool", bufs=3)
        vpool = tc.alloc_tile_pool(name="vpool", bufs=3)
        opool = tc.alloc_tile_pool(name="opool", bufs=4)
        dgpool = tc.alloc_tile_pool(name="dgpool", bufs=3)
        cacc = tc.alloc_tile_pool(name="cacc", bufs=2)

        # ---- load order: vw(vv0) first, then emb per-et, then small consts
        vw_t0 = kwpool.tile([128, NET, 256], F16, name="vw_t0", tag="w")
        for eh in range(2):
            nc.sync.dma_start(
                out=vw_t0[:, eh * 4:(eh + 1) * 4, :],
                in_=vwT.rearrange("(et p) c -> p et c", p=128)[
                    :, eh * 4:(eh + 1) * 4, 0:256],
            )
        emb_all = consts.tile([128, NET, NTOK], F16)
        for et in range(NET):
            nc.sync.dma_start(out=emb_all[:, et, :],
                              in_=emb16[et * 128:(et + 1) * 128, :])
        valb_sb = consts.tile([128, NCT], F32)
        nc.sync.dma_start(out=valb_sb, in_=valb[:, :])
        aux_sb = consts.tile([128, 16 + 256], F16)
        nc.sync.dma_start(out=aux_sb, in_=aux16[:, :])
        ceps_sb = consts.tile([128, 24], F32)
        nc.sync.dma_start(out=ceps_sb, in_=ceps[:, :])
        cepr_sb = consts.tile([128, 24], F32R)
        nc.sync.dma_start(out=cepr_sb, in_=_r(ceps[:, :]))
        keyb_sb = consts.tile([128, NGCT], F32)
        nc.sync.dma_start(out=keyb_sb, in_=keyb[:, :])
        lk8_sb = consts.tile([128, 9, 2, 16], FP8)
        nc.sync.dma_start(out=lk8_sb,
                          in_=lk8.rearrange("p (q i c) -> p q i c", i=2, c=16))
        lkq_sb = consts.tile([128, NGCT, 16], F32R)
        nc.sync.dma_start(out=lkq_sb, in_=_r(lkq.rearrange("n p m -> p n m")))
        cwf_sb = consts.tile([128, NGCT * KT], F32)
        nc.sync.dma_start(out=cwf_sb, in_=cwf[:, :])
        hc_sb = consts.tile([128, NGCT, HALO], F16)
        nc.sync.dma_start(out=hc_sb,
                          in_=hc.rearrange("p (n h) -> p n h", h=HALO))
        id16_sb = consts.tile([128, 128], F16)
        nc.sync.dma_start(out=id16_sb, in_=id16[:, :])
        vproj16 = consts.tile([128, NCT, NTOK], F16)
        bc2_sb = aux_sb[0:1, 16:16 + 128]

        # ---- gate sums: ONE [16, NTOK] psum shared by all stages via
        # disjoint one-hot rows: ak_g = row g, aq_g = 4+g, dot_g = 8+g,
        # sv = 12. Reset once (B's first vsq sum); everything accumulates.
        sums_all = sump.tile([16, NTOK], F32, name="sums_all", tag="sums")
        first_sum = [True] * NCH

        def sum_mm(stage, lhsT, rhs, ch, last=False, perf_mode=None):
            st = first_sum[ch]
            first_sum[ch] = False
            nc.tensor.matmul(
                sums_all[:, ch * CHW:(ch + 1) * CHW],
                lhsT, rhs, start=st, stop=last,
                perf_mode=perf_mode, skip_group_check=True,
            )

        # ---------- stage B: vproj = value_w @ emb + value_b ----------
        # vsq in fp8 (feeds only alpha), DoubleRow-reduced, deferred one vv
        pend_vsq = None

        def flush_vsq(v8):
            for ch in range(NCH):
                cols = slice(ch * CHW, (ch + 1) * CHW)
                sum_mm(3, lk8_sb[:, 8, :, :], v8[:, :, cols], ch,
                       perf_mode=DR)

        for vv in range(NCT // 2):
            if vv == 0:
                vw_t = vw_t0
            else:
                vw_t = kwpool.tile([128, NET, 256], F16, name="vw_t", tag="w")
                nc.sync.dma_start(
                    out=vw_t,
                    in_=vwT.rearrange("(et p) c -> p et c", p=128)[
                        :, :, vv * 256:(vv + 1) * 256],
                )
            vsq = scr.tile([128, 2, NTOK], FP8, name="vsq8", tag="p8")
            if vv == 0:
                # et-outer across 4 psums so PE rate-matches the emb DMAs
                ps4 = [mmp.tile([128, CHW], F32, name=f"psB0_{i}", tag="mm")
                       for i in range(3)]
                ps4.append(epsum.tile([128, CHW], F32, name="psB0_3",
                                      tag="mm"))
                for et in range(NET):
                    for i in range(4):
                        s2, ch = i // 2, i % 2
                        nc.tensor.matmul(
                            ps4[i],
                            vw_t[:, et, s2 * 128:(s2 + 1) * 128],
                            emb_all[:, et, ch * CHW:(ch + 1) * CHW],
                            start=(et == 0), stop=(et == NET - 1),
                        )
                for i in range(4):
                    s2, ch = i // 2, i % 2
                    ct = vv * 2 + s2
                    cols = slice(ch * CHW, (ch + 1) * CHW)
                    nc.scalar.activation(
                        vproj16[:, ct, cols], ps4[i],
                        AF.Identity, bias=valb_sb[:, ct:ct + 1], scale=1.0,
                    )
                    nc.scalar.activation(
                        vsq[:, s2, cols], ps4[i], AF.Square,
                        bias=valb_sb[:, ct:ct + 1], scale=1.0,
                    )
            else:
                for s2 in range(2):
                    ct = vv * 2 + s2
                    for ch in range(NCH):
                        cols = slice(ch * CHW, (ch + 1) * CHW)
                        ps = mmp.tile([128, CHW], F32, name="psB", tag="mm")
                        for et in range(NET):
                            nc.tensor.matmul(
                                ps,
                                vw_t[:, et, s2 * 128:(s2 + 1) * 128],
                                emb_all[:, et, ch * CHW:(ch + 1) * CHW],
                                start=(et == 0), stop=(et == NET - 1),
                            )
                        nc.scalar.activation(
                            vproj16[:, ct, cols], ps,
                            AF.Identity, bias=valb_sb[:, ct:ct + 1], scale=1.0,
                        )
                        nc.scalar.activation(
                            vsq[:, s2, cols], ps, AF.Square,
                            bias=valb_sb[:, ct:ct + 1], scale=1.0,
                        )
                if pend_vsq is not None:
                    flush_vsq(pend_vsq)
                    pend_vsq = None
            pend_vsq = vsq
        flush_vsq(pend_vsq)

        # ---------- stage C for one group-pair ----------
        def emit_c_kq(stage, gg):
            """k path for double-gct gg (two gc tiles); DR sums deferred."""
            kw_t = kwpool.tile([128, NET, 256], F16, name="kw_t", tag="w")
            nc.sync.dma_start(
                out=kw_t,
                in_=kwT.rearrange("(et p) c -> p et c", p=128)[
                    :, :, gg * 256:(gg + 1) * 256],
            )
            ksqp = scr.tile([128, 2, NTOK], FP8, name="ksqp", tag="p8")
            qsqp = scr.tile([128, 2, NTOK], FP8, name="qsqp", tag="p8")
            kqs = []
            for s2 in range(2):
                gct = gg * 2 + s2
                q_sb = qpool.tile([128, NTOK], F16, name="q_sb", tag="q")
                nc.sync.dma_start(
                    out=q_sb, in_=hidT[gct * 128:(gct + 1) * 128, :]
                )
                kq = kqpool.tile([128, NTOK], F32R, name="kq", tag="kq")
                for ch in range(NCH):
                    ps = mmp.tile([128, CHW], F32, name="psC", tag="mm")
                    for et in range(NET):
                        nc.tensor.matmul(
                            ps,
                            kw_t[:, et, s2 * 128:(s2 + 1) * 128],
                            emb_all[:, et, ch * CHW:(ch + 1) * CHW],
                            start=(et == 0), stop=(et == NET - 1),
                        )
                    cols = slice(ch * CHW, (ch + 1) * CHW)
                    nc.scalar.activation(
                        ksqp[:, s2, cols], ps, AF.Square,
                        bias=keyb_sb[:, gct:gct + 1], scale=1.0,
                    )
                    nc.gpsimd.tensor_mul(qsqp[:, s2, cols], q_sb[:, cols],
                                         q_sb[:, cols])
                    nc.vector.scalar_tensor_tensor(
                        kq[:, cols], ps, keyb_sb[:, gct:gct + 1],
                        q_sb[:, cols], op0=ALU.add, op1=ALU.mult,
                    )
                kqs.append((gct, kq))
            return ksqp, qsqp, kqs

        def emit_dr(stage, ksqp, qsqp, kqs, last_gg):
            for gct, kq in kqs:
                for ch in range(NCH):
                    sum_mm(stage, lkq_sb[:, gct, :],
                           kq[:, ch * CHW:(ch + 1) * CHW], ch)
            for ch in range(NCH):
                cols = slice(ch * CHW, (ch + 1) * CHW)
                sum_mm(stage, lk8_sb[:, stage, :, :], ksqp[:, :, cols], ch,
                       perf_mode=DR)
                sum_mm(stage, lk8_sb[:, 4 + stage, :, :], qsqp[:, :, cols],
                       ch, last=last_gg, perf_mode=DR)

        def emit_c_kq1(stage, gg, ch):
            """Single-chunk variant (window-3 ch-split passes)."""
            cols = slice(ch * CHW, (ch + 1) * CHW)
            kw_t = kwpool.tile([128, NET, 256], F16, name="kw_t", tag="w")
            nc.sync.dma_start(
                out=kw_t,
                in_=kwT.rearrange("(et p) c -> p et c", p=128)[
                    :, :, gg * 256:(gg + 1) * 256],
            )
            ksqp = scr.tile([128, 2, CHW], FP8, name="ksqp1", tag="p8")
            qsqp = scr.tile([128, 2, CHW], FP8, name="qsqp1", tag="p8")
            kqs = []
            for s2 in range(2):
                gct = gg * 2 + s2
                q_sb = qpool.tile([128, CHW], F16, name="q_sb1", tag="q")
                nc.sync.dma_start(
                    out=q_sb, in_=hidT[gct * 128:(gct + 1) * 128, cols]
                )
                kq = kqpool.tile([128, CHW], F32R, name="kq1", tag="kq")
                ps = mmp.tile([128, CHW], F32, name="psC", tag="mm")
                for et in range(NET):
                    nc.tensor.matmul(
                        ps,
                        kw_t[:, et, s2 * 128:(s2 + 1) * 128],
                        emb_all[:, et, cols],
                        start=(et == 0), stop=(et == NET - 1),
                    )
                nc.scalar.activation(
                    ksqp[:, s2, :], ps, AF.Square,
                    bias=keyb_sb[:, gct:gct + 1], scale=1.0,
                )
                nc.gpsimd.tensor_mul(qsqp[:, s2, :], q_sb, q_sb)
                nc.vector.scalar_tensor_tensor(
                    kq, ps, keyb_sb[:, gct:gct + 1],
                    q_sb, op0=ALU.add, op1=ALU.mult,
                )
                kqs.append((gct, kq))
            return ksqp, qsqp, kqs

        def emit_dr1(stage, ksqp, qsqp, kqs, ch, last_gg):
            for gct, kq in kqs:
                sum_mm(stage, lkq_sb[:, gct, :], kq, ch)
            sum_mm(stage, lk8_sb[:, stage, :, :], ksqp, ch, perf_mode=DR)
            sum_mm(stage, lk8_sb[:, 4 + stage, :, :], qsqp, ch, last=last_gg,
                   perf_mode=DR)

        # ---------- stage D ----------
        # Shared rms_v normalizer: rms_v = sqrt(gate^2*mean(vproj^2)+eps)
        # ~= gate*sqrt(mean(vproj^2)+eps) since gate=sigmoid(..)>0, so the
        # conv input normed = vproj*alpha with ONE shared alpha row; only the
        # residual (value = vproj*gate) needs the per-group gate.
        def emit_alpha():
            # sv (= sum vproj^2) sits at psum row 12: bounce the block to
            # SBUF and matmul-extract the row to partition 0.
            s3a = rowm.tile([16, NTOK], F32R, name="s3a", tag="svz")
            aln = rowsc.tile([1, NTOK], F32, name="aln", tag="rs")
            alpha16 = rowm.tile([1, NTOK], F16, name="alpha16", tag="alpha16")
            nc.scalar.activation(s3a, sums_all[:, :], AF.Copy)
            for ch in range(NCH):
                cols = slice(ch * CHW, (ch + 1) * CHW)
                p = epsum.tile([1, CHW], F32, name="svx", tag="mm")
                nc.tensor.matmul(p, cepr_sb[0:16, 20:21], s3a[:, cols],
                                 start=True, stop=True)
                nc.scalar.activation(aln[:, cols], p, AF.Ln,
                                     bias=ceps_sb[0:1, 6:7],
                                     scale=1.0 / float(C))
            nc.scalar.activation(alpha16, aln, AF.Exp, scale=-0.5)
            return alpha16

        def make_d_tiles(stage):
            T = {}
            for nm in ("p4", "lnp", "lnd", "lng", "sqg", "sgn", "ss4", "ab4",
                       "akr"):
                T[nm] = rowsc.tile([1, NTOK], F32, name=f"{nm}{stage}",
                                   tag="rs")
            T["gate16"] = rowm.tile([1, NTOK], F16, name=f"gate16{stage}",
                                    tag="gate16")
            T["s3"] = rowm.tile([16, NTOK], F32R, name=f"s3_{stage}",
                                tag="ext")
            return T

        def emit_d_s3(stage, T, chs=(0, 1)):
            """Psum sums -> partition-0-based SBUF bounce (+ stage biases)."""
            for ch in chs:
                sl = slice(ch * CHW, (ch + 1) * CHW)
                nc.scalar.activation(T["s3"][:, sl], sums_all[:, sl],
                                     AF.Identity,
                                     bias=ceps_sb[0:16, stage:stage + 1],
                                     scale=1.0)

        def emit_d(stage, T, mul_eng=None, chs=(0, 1)):
            """Per-group gate chain: gate = sigmoid(sign(dot)*sqrt(|graw|)).

            Engines only address partitions at 32-boundaries, so the psum
            region is Act-copied (aligned base -> partition 0) to s3, and
            rows 1+ are pulled to partition-0 psum via one-hot matmuls.
            Row layout: stages 0-2: [ak, aq, dot]; stage 3: [sv, aq, dot, ak].
            """
            me = mul_eng if mul_eng is not None else nc.vector
            s3 = T["s3"]
            p4, lnp, lnd, lng, sqg, sgn, ss4, ab4, gate16 = (
                T["p4"], T["lnp"], T["lnd"], T["lng"], T["sqg"], T["sgn"],
                T["ss4"], T["ab4"], T["gate16"])
            akr = T["akr"]
            if chs == (0, 1):
                sls = [slice(0, NTOK)]
            else:
                sls = [slice(ch * CHW, (ch + 1) * CHW) for ch in chs]

            def extract(row, ch):
                sel = cepr_sb[0:16, 8 + row:9 + row]
                p = epsum.tile([1, CHW], F32, name=f"x{row}_{stage}",
                               tag="mm")
                nc.tensor.matmul(p, sel,
                                 s3[:, ch * CHW:(ch + 1) * CHW],
                                 start=True, stop=True)
                return p

            # first layer reads the [1, CHW] psums (partition 0), per chunk
            for ch in chs:
                cols = slice(ch * CHW, (ch + 1) * CHW)
                ak_ps = extract(stage, ch)
                aq_ps = extract(4 + stage, ch)
                dot_ps = extract(8 + stage, ch)
                nc.scalar.activation(akr[:, cols], ak_ps, AF.Copy)
                nc.scalar.activation(ab4[:, cols], dot_ps, AF.Square)
                nc.scalar.activation(sgn[:, cols], dot_ps, AF.Sign)
                nc.vector.tensor_mul(p4[:, cols], akr[:, cols], aq_ps)
            # 2ln|dot| and ln(p4/C); 2ln|graw| = 2ln|dot| - ln(p4/C)
            # (plain subtract so the mul engine can be Pool)
            for sl in sls:
                nc.scalar.activation(lnd[:, sl], ab4[:, sl], AF.Ln,
                                     bias=ceps_sb[0:1, 7:8])
            for sl in sls:
                nc.scalar.activation(lnp[:, sl], p4[:, sl], AF.Ln,
                                     scale=1.0 / float(C))
            for sl in sls:
                me.tensor_sub(lng[:, sl], lnd[:, sl], lnp[:, sl])
            for sl in sls:
                nc.scalar.activation(sqg[:, sl], lng[:, sl], AF.Exp,
                                     scale=0.25)
            for sl in sls:
                me.tensor_mul(ss4[:, sl], sqg[:, sl], sgn[:, sl])
            for sl in sls:
                nc.scalar.activation(gate16[:, sl], ss4[:, sl], AF.Sigmoid)
            return gate16

        # ---------- stage E ----------
        def bcast_ch(src, dst, ch):
            bp = epsum.tile([128, CHW], F32, name="bp", tag="mm")
            nc.tensor.matmul(
                bp, bc2_sb[0:1, 0:128],
                src[:, ch * CHW:(ch + 1) * CHW],
                start=True, stop=True,
            )
            nc.scalar.activation(
                dst[:, ch * CHW:(ch + 1) * CHW], bp, AF.Copy)

        def bcast_row(src, tag):
            """[1, NTOK] f32/f16 row -> [128, NTOK] f16 via PE broadcast."""
            dst = rowm.tile([128, NTOK], F16, name=f"b_{tag}", tag=tag)
            for ch in range(NCH):
                bcast_ch(src, dst, ch)
            return dst

        # nx16[ct]: f16 conv input, shared by all 4 groups' units:
        # [9 zeros | vproj*alpha]; the halo contribution to the first 9
        # outputs is a host-computed f16 correction (hc) accumulated via an
        # identity matmul.
        PADW = HALO + NTOK
        nx8s = {}

        def emit_nx8(ct):
            nx8 = npool.tile([128, PADW], F16, name=f"nx16_{ct}",
                             tag=f"nx16_{ct}", bufs=1)
            nc.gpsimd.memset(nx8[:, 0:HALO], 0.0)
            nc.vector.tensor_mul(nx8[:, HALO:HALO + NTOK],
                                 vproj16[:, ct, :], ab16)
            nx8s[ct] = nx8

        def emit_val(gct, gb16, on_pool=False):
            ct = gct % NCT
            val = vpool.tile([128, NTOK], F16, name="val", tag="val")
            if on_pool:
                nc.gpsimd.tensor_mul(val, vproj16[:, ct, :], gb16)
            else:
                nc.vector.tensor_mul(val, vproj16[:, ct, :], gb16)
            return val

        def emit_e_conv_pe(gct, pools=None):
            """f16 conv taps + halo-fix matmul."""
            ct = gct % NCT
            nx8 = nx8s[ct]
            dg_t = dgpool.tile([128, KT * 128], F16, name="dg_t", tag="dg")
            nc.sync.dma_start(out=dg_t, in_=dg16[gct])
            accs = []
            for ch in range(NCH):
                pool = (pools[ch % len(pools)] if pools else epsum)
                acc = pool.tile([128, CHW], F32, name="acc", tag="mm")
                for k in range(KT):
                    base = ch * CHW + k * DIL
                    nc.tensor.matmul(
                        acc,
                        dg_t[:, k * 128:(k + 1) * 128],
                        nx8[:, base:base + CHW],
                        start=(k == 0), stop=(k == KT - 1 and ch == 1),
                        skip_group_check=True,
                    )
                if ch == 0:
                    nc.tensor.matmul(
                        acc[:, 0:HALO], id16_sb, hc_sb[:, gct, :],
                        start=False, stop=True, skip_group_check=True,
                    )
                accs.append(acc)
            return accs

        def emit_e_conv_dve(gct):
            """f16 conv as DVE scalar-ptr MAC chains (+ in-place halo fix)."""
            ct = gct % NCT
            nx8 = nx8s[ct]
            outs = []
            for ch in range(NCH):
                prev = None
                for k in range(KT):
                    win = nx8[:, ch * CHW + k * DIL:ch * CHW + k * DIL + CHW]
                    a = cacc.tile([128, CHW], F16, name=f"ca{k}", tag=f"ca{k}")
                    wcol = cwf_sb[:, gct * KT + k:gct * KT + k + 1]
                    if k == 0:
                        nc.vector.tensor_scalar_mul(a, win, wcol)
                    else:
                        nc.vector.scalar_tensor_tensor(
                            a, win, wcol, prev, op0=ALU.mult, op1=ALU.add)
                    prev = a
                if ch == 0:
                    nc.vector.tensor_tensor(prev[:, 0:HALO], prev[:, 0:HALO],
                                            hc_sb[:, gct, :], op=ALU.add)
                outs.append(prev)
            return outs

        def emit_silu(accs):
            sacc = opool.tile([128, NTOK], F16, name="sacc", tag="sacc")
            for ch in range(NCH):
                nc.scalar.activation(sacc[:, ch * CHW:(ch + 1) * CHW],
                                     accs[ch], AF.Silu)
            return sacc

        def emit_resid_out(gct, val, sacc, engine="pool"):
            ot = opool.tile([128, NTOK], F16, name="ot", tag="ot")
            if engine == "dve":
                nc.vector.tensor_tensor(ot, val, sacc, op=ALU.add)
            else:
                nc.gpsimd.tensor_add(ot, val, sacc)
            nc.sync.dma_start(out=out_d[gct * 128:(gct + 1) * 128, :], in_=ot)

        # ---------- pipeline ----------
        # conv+silu only needs the shared ab16; val/resid needs gate(g).
        # Window g: C(g) + chain(g-1) + full units of group g-1 + a few
        # group-3 conv units pulled early; tail: 3 conv units cover chain(3),
        # then group-3 val/resid.
        sacc3 = {}      # gct -> long-lived sacc for group-3 units
        ab16 = None
        TAIL3 = [29, 30, 31]
        EARLY3 = {0: [24, 25, 26], 1: [27], 2: [28], 3: []}

        def conv_unit(u, long_lived=False, pools=None, defer_silu=False,
                      dve=False):
            if dve:
                accs = emit_e_conv_dve(u)
            else:
                accs = emit_e_conv_pe(u, pools=pools)
            if defer_silu:
                return accs
            if long_lived:
                sacc = opool.tile([128, NTOK], F16, name=f"sacc{u}",
                                  tag=f"sacc3_{u}", bufs=1)
            else:
                sacc = opool.tile([128, NTOK], F16, name=f"sacc{u}",
                                  tag="sacc")
            for ch in range(NCH):
                nc.scalar.activation(sacc[:, ch * CHW:(ch + 1) * CHW],
                                     accs[ch], AF.Silu)
            if long_lived:
                sacc3[u] = sacc
            return sacc

        def full_unit(u, gb16, dve=False):
            sacc = conv_unit(u, dve=dve)
            val = emit_val(u, gb16)
            emit_resid_out(u, val, sacc, engine="pool" if dve else "dve")

        gate_prev = None
        d_tiles = {}
        for g in range(3):
            dr_prev = None
            gb16 = None
            units = list(range((g - 1) * 8, g * 8)) if g else []
            for i, gg in enumerate(range(g * 4, (g + 1) * 4)):
                if i == 0 and g:
                    d_tiles[g - 1] = make_d_tiles(g - 1)
                    emit_d_s3(g - 1, d_tiles[g - 1])
                cur = emit_c_kq(g, gg)
                if i == 0:
                    if g == 0:
                        alpha16 = emit_alpha()
                    else:
                        gate_prev = emit_d(g - 1, d_tiles[g - 1],
                                           mul_eng=nc.gpsimd)
                if dr_prev is not None:
                    emit_dr(g, *dr_prev, last_gg=False)
                dr_prev = cur
                if i == 1:
                    if g == 0:
                        ab16 = bcast_row(alpha16, "ab16")
                        for ct in (0, 1, 2):
                            emit_nx8(ct)
                    elif g == 1:
                        for ct in (6, 7):
                            emit_nx8(ct)
                    batch = []
                elif i == 2:
                    if g:
                        gb16 = bcast_row(gate_prev, f"gb{g - 1}")
                        batch = units[0:3]
                    else:
                        emit_nx8(3)
                        batch = EARLY3[0][0:2]
                elif i == 3:
                    if g == 0:
                        emit_nx8(4)
                        emit_nx8(5)
                    batch = units[3:6] if g else EARLY3[0][2:3]
                else:
                    batch = []
                for u in batch:
                    if g:
                        full_unit(u, gb16, dve=(u % 8 in (0, 2, 4)))
                    else:
                        conv_unit(u, long_lived=True, dve=True)
            emit_dr(g, *dr_prev, last_gg=True)
            if g:
                for u in units[6:8]:
                    full_unit(u, gb16, dve=(u % 8 == 6))
                for u in EARLY3[g]:
                    conv_unit(u, long_lived=True)

        # ---------- window 3: chunk-split passes ----------
        # pass p computes C(3) for token chunk p only, so the stage-3 gate
        # chain + group-3 epilogue for chunk 0 hide under pass 1.
        units = list(range(16, 24))
        d_tiles[2] = make_d_tiles(2)
        emit_d_s3(2, d_tiles[2])
        T3 = None
        gb3 = rowm.tile([128, NTOK], F16, name="b_gb3", tag="gb3")

        def epi3_ch(u, ch):
            ct = u % NCT
            cols = slice(ch * CHW, (ch + 1) * CHW)
            val = vpool.tile([128, CHW], F16, name="val3", tag="val")
            nc.vector.tensor_mul(val, vproj16[:, ct, cols], gb3[:, cols])
            ot = opool.tile([128, CHW], F16, name="ot3", tag="ot")
            nc.vector.tensor_tensor(ot, val, sacc3[u][:, cols], op=ALU.add)
            nc.sync.dma_start(out=out_d[u * 128:(u + 1) * 128, cols], in_=ot)

        for p in range(2):
            dr_prev = None
            for i, gg in enumerate(range(12, 16)):
                cur = emit_c_kq1(3, gg, p)
                if p == 0 and i == 0:
                    gate2 = emit_d(2, d_tiles[2], mul_eng=nc.gpsimd)
                if p == 1 and i == 0:
                    T3 = make_d_tiles(3)
                    emit_d_s3(3, T3, chs=(0,))
                    gate3 = emit_d(3, T3, mul_eng=nc.vector, chs=(0,))
                if dr_prev is not None:
                    emit_dr1(3, *dr_prev, p, last_gg=False)
                dr_prev = cur
                if p == 0:
                    if i == 2:
                        gb2 = bcast_row(gate2, "gb2")
                        batch = units[0:3]
                    elif i == 3:
                        batch = units[3:6]
                    else:
                        batch = []
                    for u in batch:
                        full_unit(u, gb2, dve=(u % 8 in (0, 2, 4)))
                else:
                    if i == 1:
                        bcast_ch(gate3, gb3, 0)
                        for u in units[6:8]:
                            full_unit(u, gb2, dve=(u % 8 == 6))
                    elif i == 2:
                        for u in range(24, 28):
                            epi3_ch(u, 0)
                    elif i == 3:
                        epi3_ch(28, 0)
            emit_dr1(3, *dr_prev, p, last_gg=True)

        # ---------- tail: chunk 1 of the group-3 gate + epilogue ----------
        # TAIL3 conv matmuls cover the chain; their silus follow its Act ops
        emit_d_s3(3, T3, chs=(1,))
        acc_pools = [epsum, mmp]
        emit_d(3, T3, mul_eng=nc.vector, chs=(1,))
        tail_accs = [conv_unit(u, pools=acc_pools, defer_silu=True)
                     for u in TAIL3]
        bcast_ch(gate3, gb3, 1)
        for j, u in enumerate(TAIL3):
            sacc = opool.tile([128, NTOK], F16, name=f"sacc{u}",
                              tag=f"sacc3_{u}", bufs=1)
            for ch in range(NCH):
                nc.scalar.activation(sacc[:, ch * CHW:(ch + 1) * CHW],
                                     tail_accs[j][ch], AF.Silu)
            sacc3[u] = sacc
        for u in TAIL3:
            epi3_ch(u, 0)
        for u in range(24, 32):
            epi3_ch(u, 1)

        for p in (cacc, dgpool, opool, vpool, npool, rowsc, rowm, kqpool, scr,
                  epsum, sump, mmp, qpool, kwpool, consts):
            p.release()
    return nc


def host_prep(embeddings, hidden_states, key_w, key_b, value_w, value_b,
              w_key_norm, w_query_norm, w_norm, conv_weight):
    """Build the per-core input maps."""
    f32, f16 = np.float32, np.float16
    e4 = ml_dtypes.float8_e4m3fn
    embeddings = np.asarray(embeddings, f32)
    hidden_states = np.asarray(hidden_states, f32)
    key_w = np.asarray(key_w, f32)
    key_b = np.asarray(key_b, f32)
    value_w = np.asarray(value_w, f32)
    value_b = np.asarray(value_b, f32)
    w_key_norm = np.asarray(w_key_norm, f32)
    w_query_norm = np.asarray(w_query_norm, f32)
    w_norm = np.asarray(w_norm, f32)
    conv_weight = np.asarray(conv_weight, f32)

    kwT = np.ascontiguousarray(key_w.T).astype(f16)        # [E, GC]
    vwT = np.ascontiguousarray(value_w.T).astype(f16)      # [E, C]
    keyb_r = np.ascontiguousarray(key_b.reshape(NGCT, 128).T)  # [128, NGCT]
    valb_r = np.ascontiguousarray(value_b.reshape(NCT, 128).T)
    wkq = (w_key_norm * w_query_norm).reshape(GC)

    # one-hot lhsT tables. ONE shared [16, NTOK] psum accumulator with
    # disjoint rows: ak_g = row g, aq_g = 4+g, dot_g = 8+g, sv = 12.
    # (engines can only address 32-aligned partition bases, so rows are
    #  matmul-extracted after an Act bounce of the block to partition 0)
    lk8 = np.zeros((128, 9, 2, 16), f32)
    for g in range(G):
        lk8[:, g, :, g] = 1.0          # ksq -> row g
        lk8[:, 4 + g, :, 4 + g] = 1.0  # qsq -> row 4+g
    lk8[:, 8, :, 12] = 1.0             # vsq -> row 12 (sv)
    lk8 = lk8.reshape(128, 288).astype(e4)

    lkq = np.zeros((NGCT, 128, 16), f32)
    for gct in range(NGCT):
        g = gct // NCT
        lkq[gct, :, 8 + g] = wkq[gct * 128:(gct + 1) * 128]

    aux16 = np.zeros((128, 16 + 256), f16)
    aux16[:, 12] = 1.0        # lv one-hot: vsq -> row 12 (sv)
    for j in range(2):
        aux16[j, 16 + j * 128:16 + (j + 1) * 128] = 1.0

    # ceps: cols 0-3 = per-stage bias vectors (+C*EPS on ak/aq rows);
    #        cols 8+r = f32 one-hot row selectors (identity)
    ceps_h = np.zeros((128, 24), f32)
    for g in range(G):
        ceps_h[g, g] = float(C) * EPS
        ceps_h[4 + g, g] = float(C) * EPS
    for r in range(16):
        ceps_h[r, 8 + r] = 1.0
    ceps_h[0, 6] = NORM_EPS
    ceps_h[0, 7] = 1e-60

    # f16 diagonal conv weights + identity for the halo-fix matmul.
    cwf = (conv_weight.reshape(G, C, KT) * w_norm[:, :, None]).astype(f32)
    dg = np.zeros((NGCT, 128, KT * 128), f16)
    idx = np.arange(128)
    for gct in range(NGCT):
        g, ct = gct // NCT, gct % NCT
        for k in range(KT):
            dg[gct, idx, k * 128 + idx] = cwf[g, ct * 128 + idx, k].astype(f16)
    id16_h = np.zeros((128, 128), f16)
    id16_h[idx, idx] = 1.0
    cwf_r = np.zeros((128, NGCT * KT), f32)
    for gct in range(NGCT):
        g, ct = gct // NCT, gct % NCT
        for k in range(KT):
            cwf_r[:, gct * KT + k] = cwf[g, ct * 128:(ct + 1) * 128, k]

    in_maps = []
    for core in range(NCORES):
        b = core // (NCORES // B)
        t0 = (core % (NCORES // B)) * NTOK
        emb_s = embeddings[b, t0:t0 + NTOK]                # [NTOK, E]
        hid_s = hidden_states[b, t0:t0 + NTOK].reshape(NTOK, GC)
        emb_c = np.ascontiguousarray(emb_s.T).astype(f16)  # [E, NTOK]
        hid_c = np.ascontiguousarray(hid_s.T).astype(f16)  # [GC, NTOK]

        # halo: nhat (= value / rms_v, w_norm NOT applied) for the 9
        # preceding tokens feeds a host-computed conv correction hc for the
        # first 9 output tokens; zeros at the sequence start.
        if t0 == 0:
            hc_c = np.zeros((128, NGCT * HALO), f16)
        else:
            th = slice(t0 - HALO, t0)
            e9 = embeddings[b, th]                          # [9, E]
            k9 = (e9 @ key_w.T + key_b).reshape(HALO, G, C)
            q9 = hidden_states[b, th]                       # [9, G, C]
            rk = np.sqrt((k9 * k9).mean(-1) + EPS)
            rq = np.sqrt((q9 * q9).mean(-1) + EPS)
            d9 = np.einsum("tgc,gc,tgc,gc->tg", k9, w_key_norm, q9,
                           w_query_norm)
            graw = d9 / (rk * rq) / np.sqrt(f32(C))
            g9 = 1.0 / (1.0 + np.exp(-(np.where(graw >= 0, 1.0, -1.0)
                                       * np.sqrt(np.maximum(np.abs(graw),
                                                            1e-6)))))
            vp9 = e9 @ value_w.T + value_b                  # [9, C]
            val9 = vp9[:, None, :] * g9[..., None].astype(f32)
            rv9 = np.sqrt((val9 * val9).mean(-1) + NORM_EPS)
            nhat9 = val9 / rv9[..., None]                   # [9, G, C]
            # hc[c, gct, t] = sum_{k: t+k*DIL<9} cwf[g,c,k]*nhat9[t+k*DIL,g,c]
            hcf = np.zeros((HALO, G, C), f32)
            for t in range(HALO):
                for k in range(KT):
                    ix = t + k * DIL
                    if ix < HALO:
                        hcf[t] += cwf[:, :, k] * nhat9[ix]
            hg = hcf.transpose(1, 2, 0).reshape(NGCT, 128, HALO)
            hc_c = np.ascontiguousarray(
                hg.transpose(1, 0, 2).reshape(128, NGCT * HALO)).astype(f16)

        in_maps.append({
            "emb16": emb_c, "hidT": hid_c, "kwT": kwT, "vwT": vwT,
            "keyb": keyb_r, "valb": valb_r,
            "lk8": lk8, "lkq": lkq, "aux16": aux16, "ceps": ceps_h,
            "dg16": dg, "cwf": cwf_r, "hc": hc_c, "id16": id16_h,
        })
    return in_maps


_NC_CACHE = [None]
LAST_RESULT = [None]


def kernel(**inputs) -> np.ndarray:
    in_maps = host_prep(**inputs)
    if _NC_CACHE[0] is None:
        _NC_CACHE[0] = build_program()
    nc = _NC_CACHE[0]
    res = run_bass_kernel_spmd(nc, in_maps, list(range(NCORES)))
    LAST_RESULT[0] = res
    out = np.empty((B, T, G, C), np.float32)
    for core in range(NCORES):
        b = core // (NCORES // B)
        t0 = (core % (NCORES // B)) * NTOK
        oc = np.asarray(res.results[core]["out"]).astype(np.float32)
        out[b, t0:t0 + NTOK] = oc.reshape(G, C, NTOK).transpose(2, 0, 1)
    return out


# revision 114
# speedup vs baseline: 1.0030x; 1.0018x over previous
"""Trainium2 Bass kernel for the EngramNew module (dense_cnn), v3.

Sharding: B*T = 8192 tokens split across 8 cores (1024 tokens each); the conv
halo of (K-1)*DIL = 9 tokens is precomputed host-side.  On-device layout is
channels-on-partitions / tokens-on-free: [G*C, T_core].

v10 design vs the v2 baseline (291.8us -> 225.1us):
 - shared rms_v normalizer: rms_v = sqrt(gate^2*mean(vproj^2)+eps)
   ~= gate*sqrt(mean(vproj^2)+eps) since gate = sigmoid(..) > 0, so the
   conv input (normed) = vproj*alpha with ONE shared alpha row; only the
   residual needs the per-group gate.  This decouples the whole conv
   pipeline from the gate chains (error <= ~1e-3, verified vs reference).
 - ONE [16,NTOK] PSUM accumulator shared by all four gate-sum stages via
   disjoint one-hot rows (ak_g=g, aq_g=4+g, dot_g=8+g, sv=12), reset once:
   no PSUM bank-rotation WAR stalls.  Rows are bounced to partition 0 by
   an Act copy + f32r one-hot extraction matmuls (engines can only address
   32-aligned partition bases).
 - per-ct conv input tiles (zero prefix + vproj*alpha) shared by all 4
   groups; the halo contribution to the first 9 outputs is a host-computed
   correction added via an identity matmul.  15 of 32 conv units run as
   DVE MAC chains, the rest as PE diag matmuls.
 - C(3) is split into two per-chunk passes so chunk 0 of the stage-3 gate
   chain + epilogue hides under the chunk-1 pass; 3 conv units are held
   back as PE cover for the chunk-1 chain.
 - kq / ksq+qsq(fp8 DR) / vsq(fp8 DR) reductions all deferred by one
   gg/vv so their producers never stall the PE sum matmuls.
 - startup: split vw/emb DMAs ordered first + 4-psum et-outer first vproj
   pass so PE starts at ~4us instead of 13us.
"""

import os
import sys

for _p in ("/opt/trn_rl_repo",):
    if _p not in sys.path:
        sys.path.insert(0, _p)

import numpy as np
import ml_dtypes

import concourse.bass as bass
from concourse import mybir
from concourse.tile import TileContext
from concourse.bass_utils import run_bass_kernel_spmd
import bass_rust

F32 = mybir.dt.float32
F32R = mybir.dt.float32r
F16 = mybir.dt.float16
FP8 = mybir.dt.float8e4
AF = mybir.ActivationFunctionType
ALU = mybir.AluOpType
DR = mybir.MatmulPerfMode.DoubleRow

# Problem constants (hardcoded per spec nn_EngramNew_2070174237244)
B, T, G, C, E = 2, 4096, 4, 1024, 1024
GC = G * C
KT, DIL = 4, 3          # conv taps / dilation
EPS = 1e-5
NORM_EPS = 1e-5
NCORES = 8
NTOK = (B * T) // NCORES    # 1024 tokens per core
HALO = (KT - 1) * DIL       # 9
NET = E // 128              # 8 e-tiles
NGCT = GC // 128            # 32 gc-tiles
NCT = C // 128              # 8 c-tiles
CHW = 512                   # token chunk width (1 PSUM bank of fp32)
NCH = NTOK // CHW           # 2 chunks



class PatchedTileContext(TileContext):
    """This walrus build allows only one sem wait per instruction (two on
    EventSemaphore). Tile attaches as many waits as an instruction needs,
    so after scheduling we hoist excess waits onto no-op instructions
    inserted just before the owner on the same engine (engines are strict
    FIFO, so observing the sems earlier is equivalent)."""

    def _split_excess_waits(self):
        nc = self.nc

        def make_nop(engine):
            bi = nc.engines[engine].nop()
            bb = nc.cur_bb.bb
            lst = list(bb.instructions)
            assert lst[-1] is bi.ins
            bb.instructions = lst[:-1]
            return bi.ins

        # Phase 1: snapshot every block BEFORE creating any nop, so nops
        # appended to cur_bb can never leak into the iteration or the rebuilt
        # lists (cur_bb may be one of the blocks being processed).
        snapshots = []
        for f in nc.m.functions:
            for blk in f.blocks:
                snapshots.append((blk, list(blk.instructions)))

        for blk, insts in snapshots:
            out = []
            changed = False
            for ins in insts:
                si = ins.sync_info
                waits = list(si.on_wait) if (si and si.on_wait) else []
                cap = 2 if isinstance(ins, mybir.InstEventSemaphore) else 1
                if len(waits) > cap:
                    changed = True
                    for w in waits[cap:]:
                        nop = make_nop(ins.engine)
                        nop.sync_info = bass_rust.SyncInfo(
                            on_wait=[w], on_update=[]
                        )
                        out.append(nop)
                    upd = list(si.on_update) if si.on_update else []
                    ins.sync_info = bass_rust.SyncInfo(
                        on_wait=waits[:cap], on_update=upd
                    )
                out.append(ins)
            if changed:
                blk.instructions = out

    def _drain_and_barrier(self, tick_clock, wait_clock):
        super()._drain_and_barrier(tick_clock, wait_clock)
        self._split_excess_waits()


def _r(ap):
    return ap.bitcast(F32R)


def build_program():
    nc = bass.Bass()

    # ---- DRAM parameters ----
    emb16 = nc.declare_dram_parameter("emb16", [E, NTOK], F16, isOutput=False)
    hidT = nc.declare_dram_parameter("hidT", [GC, NTOK], F16, isOutput=False)
    kwT = nc.declare_dram_parameter("kwT", [E, GC], F16, isOutput=False)
    vwT = nc.declare_dram_parameter("vwT", [E, C], F16, isOutput=False)
    keyb = nc.declare_dram_parameter("keyb", [128, NGCT], F32, isOutput=False)
    valb = nc.declare_dram_parameter("valb", [128, NCT], F32, isOutput=False)
    lk8 = nc.declare_dram_parameter("lk8", [128, 9 * 2 * 16], FP8,
                                    isOutput=False)
    lkq = nc.declare_dram_parameter("lkq", [NGCT, 128, 16], F32, isOutput=False)
    aux16 = nc.declare_dram_parameter("aux16", [128, 16 + 256], F16,
                                      isOutput=False)
    ceps = nc.declare_dram_parameter("ceps", [128, 24], F32, isOutput=False)
    dg16 = nc.declare_dram_parameter("dg16", [NGCT, 128, KT * 128], F16,
                                     isOutput=False)
    cwf = nc.declare_dram_parameter("cwf", [128, NGCT * KT], F32,
                                    isOutput=False)
    hc = nc.declare_dram_parameter("hc", [128, NGCT * HALO], F16,
                                   isOutput=False)
    id16 = nc.declare_dram_parameter("id16", [128, 128], F16, isOutput=False)
    out_d = nc.declare_dram_parameter("out", [GC, NTOK], F16, isOutput=True)

    with PatchedTileContext(nc) as tc:
        consts = tc.alloc_tile_pool(name="consts", bufs=1)
        kwpool = tc.alloc_tile_pool(name="kwpool", bufs=2)
        qpool = tc.alloc_tile_pool(name="qpool", bufs=3)
        mmp = tc.alloc_tile_pool(name="mmp", bufs=3, space=bass.MemorySpace.PSUM)
        sump = tc.alloc_tile_pool(name="sump", bufs=1, space=bass.MemorySpace.PSUM)
        epsum = tc.alloc_tile_pool(name="epsum", bufs=3,
                                   space=bass.MemorySpace.PSUM)
        scr = tc.alloc_tile_pool(name="scr", bufs=4)
        kqpool = tc.alloc_tile_pool(name="kqpool", bufs=4)
        rowm = tc.alloc_tile_pool(name="rowm", bufs=1)
        rowsc = tc.alloc_tile_pool(name="rowsc", bufs=9)
        npool = tc.alloc_tile_pool(name="npool", bufs=3)
        vpool = tc.alloc_tile_pool(name="vpool", bufs=3)
        opool = tc.alloc_tile_pool(name="opool", bufs=4)
        dgpool = tc.alloc_tile_pool(name="dgpool", bufs=3)
        cacc = tc.alloc_tile_pool(name="cacc", bufs=2)

        # ---- load order: vw(vv0) first, then emb per-et, then small consts
        vw_t0 = kwpool.tile([128, NET, 256], F16, name="vw_t0", tag="w")
        for eh in range(2):
            nc.sync.dma_start(
                out=vw_t0[:, eh * 4:(eh + 1) * 4, :],
                in_=vwT.rearrange("(et p) c -> p et c", p=128)[
                    :, eh * 4:(eh + 1) * 4, 0:256],
            )
        emb_all = consts.tile([128, NET, NTOK], F16)
        for et in range(NET):
            nc.sync.dma_start(out=emb_all[:, et, :],
                              in_=emb16[et * 128:(et + 1) * 128, :])
        vw_t1 = kwpool.tile([128, NET, 256], F16, name="vw_t1", tag="w")
        nc.sync.dma_start(
            out=vw_t1,
            in_=vwT.rearrange("(et p) c -> p et c", p=128)[:, :, 256:512],
        )
        valb_sb = consts.tile([128, NCT], F32)
        nc.sync.dma_start(out=valb_sb, in_=valb[:, :])
        aux_sb = consts.tile([128, 16 + 256], F16)
        nc.sync.dma_start(out=aux_sb, in_=aux16[:, :])
        ceps_sb = consts.tile([128, 24], F32)
        nc.sync.dma_start(out=ceps_sb, in_=ceps[:, :])
        cepr_sb = consts.tile([128, 24], F32R)
        nc.sync.dma_start(out=cepr_sb, in_=_r(ceps[:, :]))
        keyb_sb = consts.tile([128, NGCT], F32)
        nc.sync.dma_start(out=keyb_sb, in_=keyb[:, :])
        lk8_sb = consts.tile([128, 9, 2, 16], FP8)
        nc.sync.dma_start(out=lk8_sb,
                          in_=lk8.rearrange("p (q i c) -> p q i c", i=2, c=16))
        lkq_sb = consts.tile([128, NGCT, 16], F32R)
        nc.sync.dma_start(out=lkq_sb, in_=_r(lkq.rearrange("n p m -> p n m")))
        cwf_sb = consts.tile([128, NGCT * KT], F32)
        nc.sync.dma_start(out=cwf_sb, in_=cwf[:, :])
        hc_sb = consts.tile([128, NGCT, HALO], F16)
        nc.sync.dma_start(out=hc_sb,
                          in_=hc.rearrange("p (n h) -> p n h", h=HALO))
        id16_sb = consts.tile([128, 128], F16)
        nc.sync.dma_start(out=id16_sb, in_=id16[:, :])
        vproj16 = consts.tile([128, NCT, NTOK], F16)
        bc2_sb = aux_sb[0:1, 16:16 + 128]

        # ---- gate sums: ONE [16, NTOK] psum shared by all stages via
        # disjoint one-hot rows: ak_g = row g, aq_g = 4+g, dot_g = 8+g,
        # sv = 12. Reset once (B's first vsq sum); everything accumulates.
        sums_all = sump.tile([16, NTOK], F32, name="sums_all", tag="sums")
        first_sum = [True] * NCH

        def sum_mm(stage, lhsT, rhs, ch, last=False, perf_mode=None):
            st = first_sum[ch]
            first_sum[ch] = False
            nc.tensor.matmul(
                sums_all[:, ch * CHW:(ch + 1) * CHW],
                lhsT, rhs, start=st, stop=last,
                perf_mode=perf_mode, skip_group_check=True,
            )

        # ---------- stage B: vproj = value_w @ emb + value_b ----------
        # vsq in fp8 (feeds only alpha), DoubleRow-reduced, deferred one vv
        pend_vsq = None

        def flush_vsq(v8):
            for ch in range(NCH):
                cols = slice(ch * CHW, (ch + 1) * CHW)
                sum_mm(3, lk8_sb[:, 8, :, :], v8[:, :, cols], ch,
                       perf_mode=DR)

        for vv in range(NCT // 2):
            if vv == 0:
                vw_t = vw_t0
            elif vv == 1:
                vw_t = vw_t1
            else:
                vw_t = kwpool.tile([128, NET, 256], F16, name="vw_t", tag="w")
                nc.sync.dma_start(
                    out=vw_t,
                    in_=vwT.rearrange("(et p) c -> p et c", p=128)[
                        :, :, vv * 256:(vv + 1) * 256],
                )
            vsq = scr.tile([128, 2, NTOK], FP8, name="vsq8", tag="p8")
            if vv == 0:
                # et-outer across 4 psums so PE rate-matches the emb DMAs
                ps4 = [mmp.tile([128, CHW], F32, name=f"psB0_{i}", tag="mm")
                       for i in range(3)]
                ps4.append(epsum.tile([128, CHW], F32, name="psB0_3",
                                      tag="mm"))
                for et in range(NET):
                    for i in range(4):
                        s2, ch = i // 2, i % 2
                        nc.tensor.matmul(
                            ps4[i],
                            vw_t[:, et, s2 * 128:(s2 + 1) * 128],
                            emb_all[:, et, ch * CHW:(ch + 1) * CHW],
                            start=(et == 0), stop=(et == NET - 1),
                        )
                for i in range(4):
                    s2, ch = i // 2, i % 2
                    ct = vv * 2 + s2
                    cols = slice(ch * CHW, (ch + 1) * CHW)
                    nc.scalar.activation(
                        vproj16[:, ct, cols], ps4[i],
                        AF.Identity, bias=valb_sb[:, ct:ct + 1], scale=1.0,
                    )
                    nc.scalar.activation(
                        vsq[:, s2, cols], ps4[i], AF.Square,
                        bias=valb_sb[:, ct:ct + 1], scale=1.0,
                    )
            else:
                for s2 in range(2):
                    ct = vv * 2 + s2
                    for ch in range(NCH):
                        cols = slice(ch * CHW, (ch + 1) * CHW)
                        ps = mmp.tile([128, CHW], F32, name="psB", tag="mm")
                        for et in range(NET):
                            nc.tensor.matmul(
                                ps,
                                vw_t[:, et, s2 * 128:(s2 + 1) * 128],
                                emb_all[:, et, ch * CHW:(ch + 1) * CHW],
                                start=(et == 0), stop=(et == NET - 1),
                            )
                        nc.scalar.activation(
                            vproj16[:, ct, cols], ps,
                            AF.Identity, bias=valb_sb[:, ct:ct + 1], scale=1.0,
                        )
                        nc.scalar.activation(
                            vsq[:, s2, cols], ps, AF.Square,
                            bias=valb_sb[:, ct:ct + 1], scale=1.0,
                        )
                if pend_vsq is not None:
                    flush_vsq(pend_vsq)
                    pend_vsq = None
            pend_vsq = vsq
        flush_vsq(pend_vsq)

        # ---------- stage C for one group-pair ----------
        def emit_c_kq(stage, gg):
            """k path for double-gct gg (two gc tiles); DR sums deferred."""
            kw_t = kwpool.tile([128, NET, 256], F16, name="kw_t", tag="w")
            nc.sync.dma_start(
                out=kw_t,
                in_=kwT.rearrange("(et p) c -> p et c", p=128)[
                    :, :, gg * 256:(gg + 1) * 256],
            )
            ksqp = scr.tile([128, 2, NTOK], FP8, name="ksqp", tag="p8")
            qsqp = scr.tile([128, 2, NTOK], FP8, name="qsqp", tag="p8")
            kqs = []
            for s2 in range(2):
                gct = gg * 2 + s2
                q_sb = qpool.tile([128, NTOK], F16, name="q_sb", tag="q")
                nc.sync.dma_start(
                    out=q_sb, in_=hidT[gct * 128:(gct + 1) * 128, :]
                )
                kq = kqpool.tile([128, NTOK], F32R, name="kq", tag="kq")
                for ch in range(NCH):
                    ps = mmp.tile([128, CHW], F32, name="psC", tag="mm")
                    for et in range(NET):
                        nc.tensor.matmul(
                            ps,
                            kw_t[:, et, s2 * 128:(s2 + 1) * 128],
                            emb_all[:, et, ch * CHW:(ch + 1) * CHW],
                            start=(et == 0), stop=(et == NET - 1),
                        )
                    cols = slice(ch * CHW, (ch + 1) * CHW)
                    nc.scalar.activation(
                        ksqp[:, s2, cols], ps, AF.Square,
                        bias=keyb_sb[:, gct:gct + 1], scale=1.0,
                    )
                    nc.gpsimd.tensor_mul(qsqp[:, s2, cols], q_sb[:, cols],
                                         q_sb[:, cols])
                    nc.vector.scalar_tensor_tensor(
                        kq[:, cols], ps, keyb_sb[:, gct:gct + 1],
                        q_sb[:, cols], op0=ALU.add, op1=ALU.mult,
                    )
                kqs.append((gct, kq))
            return ksqp, qsqp, kqs

        def emit_dr(stage, ksqp, qsqp, kqs, last_gg):
            for gct, kq in kqs:
                for ch in range(NCH):
                    sum_mm(stage, lkq_sb[:, gct, :],
                           kq[:, ch * CHW:(ch + 1) * CHW], ch)
            for ch in range(NCH):
                cols = slice(ch * CHW, (ch + 1) * CHW)
                sum_mm(stage, lk8_sb[:, stage, :, :], ksqp[:, :, cols], ch,
                       perf_mode=DR)
                sum_mm(stage, lk8_sb[:, 4 + stage, :, :], qsqp[:, :, cols],
                       ch, last=last_gg, perf_mode=DR)

        def emit_c_kq1(stage, gg, ch):
            """Single-chunk variant (window-3 ch-split passes)."""
            cols = slice(ch * CHW, (ch + 1) * CHW)
            kw_t = kwpool.tile([128, NET, 256], F16, name="kw_t", tag="w")
            nc.sync.dma_start(
                out=kw_t,
                in_=kwT.rearrange("(et p) c -> p et c", p=128)[
                    :, :, gg * 256:(gg + 1) * 256],
            )
            ksqp = scr.tile([128, 2, CHW], FP8, name="ksqp1", tag="p8")
            qsqp = scr.tile([128, 2, CHW], FP8, name="qsqp1", tag="p8")
            kqs = []
            for s2 in range(2):
                gct = gg * 2 + s2
                q_sb = qpool.tile([128, CHW], F16, name="q_sb1", tag="q")
                nc.sync.dma_start(
                    out=q_sb, in_=hidT[gct * 128:(gct + 1) * 128, cols]
                )
                kq = kqpool.tile([128, CHW], F32R, name="kq1", tag="kq")
                ps = mmp.tile([128, CHW], F32, name="psC", tag="mm")
                for et in range(NET):
                    nc.tensor.matmul(
                        ps,
                        kw_t[:, et, s2 * 128:(s2 + 1) * 128],
                        emb_all[:, et, cols],
                        start=(et == 0), stop=(et == NET - 1),
                    )
                nc.scalar.activation(
                    ksqp[:, s2, :], ps, AF.Square,
                    bias=keyb_sb[:, gct:gct + 1], scale=1.0,
                )
                nc.gpsimd.tensor_mul(qsqp[:, s2, :], q_sb, q_sb)
                nc.vector.scalar_tensor_tensor(
                    kq, ps, keyb_sb[:, gct:gct + 1],
                    q_sb, op0=ALU.add, op1=ALU.mult,
                )
                kqs.append((gct, kq))
            return ksqp, qsqp, kqs

        def emit_dr1(stage, ksqp, qsqp, kqs, ch, last_gg):
            for gct, kq in kqs:
                sum_mm(stage, lkq_sb[:, gct, :], kq, ch)
            sum_mm(stage, lk8_sb[:, stage, :, :], ksqp, ch, perf_mode=DR)
            sum_mm(stage, lk8_sb[:, 4 + stage, :, :], qsqp, ch, last=last_gg,
                   perf_mode=DR)

        # ---------- stage D ----------
        # Shared rms_v normalizer: rms_v = sqrt(gate^2*mean(vproj^2)+eps)
        # ~= gate*sqrt(mean(vproj^2)+eps) since gate=sigmoid(..)>0, so the
        # conv input normed = vproj*alpha with ONE shared alpha row; only the
        # residual (value = vproj*gate) needs the per-group gate.
        def emit_alpha():
            # sv (= sum vproj^2) sits at psum row 12: bounce the block to
            # SBUF and matmul-extract the row to partition 0.
            s3a = rowm.tile([16, NTOK], F32R, name="s3a", tag="svz")
            aln = rowsc.tile([1, NTOK], F32, name="aln", tag="rs")
            alpha16 = rowm.tile([1, NTOK], F16, name="alpha16", tag="alpha16")
            nc.scalar.activation(s3a, sums_all[:, :], AF.Copy)
            for ch in range(NCH):
                cols = slice(ch * CHW, (ch + 1) * CHW)
                p = epsum.tile([1, CHW], F32, name="svx", tag="mm")
                nc.tensor.matmul(p, cepr_sb[0:16, 20:21], s3a[:, cols],
                                 start=True, stop=True)
                nc.scalar.activation(aln[:, cols], p, AF.Ln,
                                     bias=ceps_sb[0:1, 6:7],
                                     scale=1.0 / float(C))
            nc.scalar.activation(alpha16, aln, AF.Exp, scale=-0.5)
            return alpha16

        def make_d_tiles(stage):
            T = {}
            for nm in ("p4", "lnp", "lnd", "lng", "sqg", "sgn", "ss4", "ab4",
                       "akr"):
                T[nm] = rowsc.tile([1, NTOK], F32, name=f"{nm}{stage}",
                                   tag="rs")
            T["gate16"] = rowm.tile([1, NTOK], F16, name=f"gate16{stage}",
                                    tag="gate16")
            T["s3"] = rowm.tile([16, NTOK], F32R, name=f"s3_{stage}",
                                tag="ext")
            return T

        def emit_d_s3(stage, T, chs=(0, 1)):
            """Psum sums -> partition-0-based SBUF bounce (+ stage biases)."""
            for ch in chs:
                sl = slice(ch * CHW, (ch + 1) * CHW)
                nc.scalar.activation(T["s3"][:, sl], sums_all[:, sl],
                                     AF.Identity,
                                     bias=ceps_sb[0:16, stage:stage + 1],
                                     scale=1.0)

        def emit_d(stage, T, mul_eng=None, chs=(0, 1)):
            """Per-group gate chain: gate = sigmoid(sign(dot)*sqrt(|graw|)).

            Engines only address partitions at 32-boundaries, so the psum
            region is Act-copied (aligned base -> partition 0) to s3, and
            rows 1+ are pulled to partition-0 psum via one-hot matmuls.
            Row layout: stages 0-2: [ak, aq, dot]; stage 3: [sv, aq, dot, ak].
            """
            me = mul_eng if mul_eng is not None else nc.vector
            s3 = T["s3"]
            p4, lnp, lnd, lng, sqg, sgn, ss4, ab4, gate16 = (
                T["p4"], T["lnp"], T["lnd"], T["lng"], T["sqg"], T["sgn"],
                T["ss4"], T["ab4"], T["gate16"])
            akr = T["akr"]
            if chs == (0, 1):
                sls = [slice(0, NTOK)]
            else:
                sls = [slice(ch * CHW, (ch + 1) * CHW) for ch in chs]

            def extract(row, ch):
                sel = cepr_sb[0:16, 8 + row:9 + row]
                p = epsum.tile([1, CHW], F32, name=f"x{row}_{stage}",
                               tag="mm")
                nc.tensor.matmul(p, sel,
                                 s3[:, ch * CHW:(ch + 1) * CHW],
                                 start=True, stop=True)
                return p

            # first layer reads the [1, CHW] psums (partition 0), per chunk
            for ch in chs:
                cols = slice(ch * CHW, (ch + 1) * CHW)
                ak_ps = extract(stage, ch)
                aq_ps = extract(4 + stage, ch)
                dot_ps = extract(8 + stage, ch)
                nc.scalar.activation(akr[:, cols], ak_ps, AF.Copy)
                nc.scalar.activation(ab4[:, cols], dot_ps, AF.Square)
                nc.scalar.activation(sgn[:, cols], dot_ps, AF.Sign)
                nc.vector.tensor_mul(p4[:, cols], akr[:, cols], aq_ps)
            # 2ln|dot| and ln(p4/C); 2ln|graw| = 2ln|dot| - ln(p4/C)
            # (plain subtract so the mul engine can be Pool)
            for sl in sls:
                nc.scalar.activation(lnd[:, sl], ab4[:, sl], AF.Ln,
                                     bias=ceps_sb[0:1, 7:8])
            for sl in sls:
                nc.scalar.activation(lnp[:, sl], p4[:, sl], AF.Ln,
                                     scale=1.0 / float(C))
            for sl in sls:
                me.tensor_sub(lng[:, sl], lnd[:, sl], lnp[:, sl])
            for sl in sls:
                nc.scalar.activation(sqg[:, sl], lng[:, sl], AF.Exp,
                                     scale=0.25)
            for sl in sls:
                me.tensor_mul(ss4[:, sl], sqg[:, sl], sgn[:, sl])
            for sl in sls:
                nc.scalar.activation(gate16[:, sl], ss4[:, sl], AF.Sigmoid)
            return gate16

        # ---------- stage E ----------
        def bcast_ch(src, dst, ch):
            bp = epsum.tile([128, CHW], F32, name="bp", tag="mm")
            nc.tensor.matmul(
                bp, bc2_sb[0:1, 0:128],
                src[:, ch * CHW:(ch + 1) * CHW],
                start=True, stop=True,
            )
            nc.scalar.activation(
                dst[:, ch * CHW:(ch + 1) * CHW], bp, AF.Copy)

        def bcast_row(src, tag):
            """[1, NTOK] f32/f16 row -> [128, NTOK] f16 via PE broadcast."""
            dst = rowm.tile([128, NTOK], F16, name=f"b_{tag}", tag=tag)
            for ch in range(NCH):
                bcast_ch(src, dst, ch)
            return dst

        # nx16[ct]: f16 conv input, shared by all 4 groups' units:
        # [9 zeros | vproj*alpha]; the halo contribution to the first 9
        # outputs is a host-computed f16 correction (hc) accumulated via an
        # identity matmul.
        PADW = HALO + NTOK
        nx8s = {}

        def emit_nx8(ct):
            nx8 = npool.tile([128, PADW], F16, name=f"nx16_{ct}",
                             tag=f"nx16_{ct}", bufs=1)
            nc.gpsimd.memset(nx8[:, 0:HALO], 0.0)
            nc.vector.tensor_mul(nx8[:, HALO:HALO + NTOK],
                                 vproj16[:, ct, :], ab16)
            nx8s[ct] = nx8

        def emit_val(gct, gb16, on_pool=False):
            ct = gct % NCT
            val = vpool.tile([128, NTOK], F16, name="val", tag="val")
            if on_pool:
                nc.gpsimd.tensor_mul(val, vproj16[:, ct, :], gb16)
            else:
                nc.vector.tensor_mul(val, vproj16[:, ct, :], gb16)
            return val

        def emit_e_conv_pe(gct, pools=None):
            """f16 conv taps + halo-fix matmul."""
            ct = gct % NCT
            nx8 = nx8s[ct]
            dg_t = dgpool.tile([128, KT * 128], F16, name="dg_t", tag="dg")
            nc.sync.dma_start(out=dg_t, in_=dg16[gct])
            accs = []
            for ch in range(NCH):
                pool = (pools[ch % len(pools)] if pools else epsum)
                acc = pool.tile([128, CHW], F32, name="acc", tag="mm")
                for k in range(KT):
                    base = ch * CHW + k * DIL
                    nc.tensor.matmul(
                        acc,
                        dg_t[:, k * 128:(k + 1) * 128],
                        nx8[:, base:base + CHW],
                        start=(k == 0), stop=(k == KT - 1 and ch == 1),
                        skip_group_check=True,
                    )
                if ch == 0:
                    nc.tensor.matmul(
                        acc[:, 0:HALO], id16_sb, hc_sb[:, gct, :],
                        start=False, stop=True, skip_group_check=True,
                    )
                accs.append(acc)
            return accs

        def emit_e_conv_dve(gct):
            """f16 conv as DVE scalar-ptr MAC chains (+ in-place halo fix)."""
            ct = gct % NCT
            nx8 = nx8s[ct]
            outs = []
            for ch in range(NCH):
                prev = None
                for k in range(KT):
                    win = nx8[:, ch * CHW + k * DIL:ch * CHW + k * DIL + CHW]
                    a = cacc.tile([128, CHW], F16, name=f"ca{k}", tag=f"ca{k}")
                    wcol = cwf_sb[:, gct * KT + k:gct * KT + k + 1]
                    if k == 0:
                        nc.vector.tensor_scalar_mul(a, win, wcol)
                    else:
                        nc.vector.scalar_tensor_tensor(
                            a, win, wcol, prev, op0=ALU.mult, op1=ALU.add)
                    prev = a
                if ch == 0:
                    nc.vector.tensor_tensor(prev[:, 0:HALO], prev[:, 0:HALO],
                                            hc_sb[:, gct, :], op=ALU.add)
                outs.append(prev)
            return outs

        def emit_silu(accs):
            sacc = opool.tile([128, NTOK], F16, name="sacc", tag="sacc")
            for ch in range(NCH):
                nc.scalar.activation(sacc[:, ch * CHW:(ch + 1) * CHW],
                                     accs[ch], AF.Silu)
            return sacc

        def emit_resid_out(gct, val, sacc, engine="pool"):
            ot = opool.tile([128, NTOK], F16, name="ot", tag="ot")
            if engine == "dve":
                nc.vector.tensor_tensor(ot, val, sacc, op=ALU.add)
            else:
                nc.gpsimd.tensor_add(ot, val, sacc)
            nc.sync.dma_start(out=out_d[gct * 128:(gct + 1) * 128, :], in_=ot)

        # ---------- pipeline ----------
        # conv+silu only needs the shared ab16; val/resid needs gate(g).
        # Window g: C(g) + chain(g-1) + full units of group g-1 + a few
        # group-3 conv units pulled early; tail: 3 conv units cover chain(3),
        # then group-3 val/resid.
        sacc3 = {}      # gct -> long-lived sacc for group-3 units
        ab16 = None
        TAIL3 = [29, 30, 31]
        EARLY3 = {0: [24, 25, 26], 1: [27], 2: [28], 3: []}

        def conv_unit(u, long_lived=False, pools=None, defer_silu=False,
                      dve=False):
            if dve:
                accs = emit_e_conv_dve(u)
            else:
                accs = emit_e_conv_pe(u, pools=pools)
            if defer_silu:
                return accs
            if long_lived:
                sacc = opool.tile([128, NTOK], F16, name=f"sacc{u}",
                                  tag=f"sacc3_{u}", bufs=1)
            else:
                sacc = opool.tile([128, NTOK], F16, name=f"sacc{u}",
                                  tag="sacc")
            for ch in range(NCH):
                nc.scalar.activation(sacc[:, ch * CHW:(ch + 1) * CHW],
                                     accs[ch], AF.Silu)
            if long_lived:
                sacc3[u] = sacc
            return sacc

        def full_unit(u, gb16, dve=False):
            sacc = conv_unit(u, dve=dve)
            val = emit_val(u, gb16)
            emit_resid_out(u, val, sacc, engine="pool" if dve else "dve")

        gate_prev = None
        d_tiles = {}
        for g in range(3):
            dr_prev = None
            gb16 = None
            units = list(range((g - 1) * 8, g * 8)) if g else []
            for i, gg in enumerate(range(g * 4, (g + 1) * 4)):
                if i == 0 and g:
                    d_tiles[g - 1] = make_d_tiles(g - 1)
                    emit_d_s3(g - 1, d_tiles[g - 1])
                cur = emit_c_kq(g, gg)
                if i == 0:
                    if g == 0:
                        alpha16 = emit_alpha()
                    else:
                        gate_prev = emit_d(g - 1, d_tiles[g - 1],
                                           mul_eng=nc.gpsimd)
                if dr_prev is not None:
                    emit_dr(g, *dr_prev, last_gg=False)
                dr_prev = cur
                if i == 1:
                    if g == 0:
                        ab16 = bcast_row(alpha16, "ab16")
                        for ct in (0, 1, 2):
                            emit_nx8(ct)
                    elif g == 1:
                        for ct in (6, 7):
                            emit_nx8(ct)
                    batch = []
                elif i == 2:
                    if g:
                        gb16 = bcast_row(gate_prev, f"gb{g - 1}")
                        batch = units[0:3]
                    else:
                        emit_nx8(3)
                        batch = EARLY3[0][0:2]
                elif i == 3:
                    if g == 0:
                        emit_nx8(4)
                        emit_nx8(5)
                    batch = units[3:6] if g else EARLY3[0][2:3]
                else:
                    batch = []
                for u in batch:
                    if g:
                        full_unit(u, gb16, dve=(u % 8 in (0, 2, 4)))
                    else:
                        conv_unit(u, long_lived=True, dve=True)
            emit_dr(g, *dr_prev, last_gg=True)
            if g:
                for u in units[6:8]:
                    full_unit(u, gb16, dve=(u % 8 == 6))
                for u in EARLY3[g]:
                    conv_unit(u, long_lived=True)

        # ---------- window 3: chunk-split passes ----------
        # pass p computes C(3) for token chunk p only, so the stage-3 gate
        # chain + group-3 epilogue for chunk 0 hide under pass 1.
        units = list(range(16, 24))
        d_tiles[2] = make_d_tiles(2)
        emit_d_s3(2, d_tiles[2])
        T3 = None
        gb3 = rowm.tile([128, NTOK], F16, name="b_gb3", tag="gb3")

        def epi3_ch(u, ch):
            ct = u % NCT
            cols = slice(ch * CHW, (ch + 1) * CHW)
            val = vpool.tile([128, CHW], F16, name="val3", tag="val")
            nc.vector.tensor_mul(val, vproj16[:, ct, cols], gb3[:, cols])
            ot = opool.tile([128, CHW], F16, name="ot3", tag="ot")
            nc.vector.tensor_tensor(ot, val, sacc3[u][:, cols], op=ALU.add)
            nc.sync.dma_start(out=out_d[u * 128:(u + 1) * 128, cols], in_=ot)

        for p in range(2):
            dr_prev = None
            for i, gg in enumerate(range(12, 16)):
                cur = emit_c_kq1(3, gg, p)
                if p == 0 and i == 0:
                    gate2 = emit_d(2, d_tiles[2], mul_eng=nc.gpsimd)
                if p == 1 and i == 0:
                    T3 = make_d_tiles(3)
                    emit_d_s3(3, T3, chs=(0,))
                    gate3 = emit_d(3, T3, mul_eng=nc.vector, chs=(0,))
                if dr_prev is not None:
                    emit_dr1(3, *dr_prev, p, last_gg=False)
                dr_prev = cur
                if p == 0:
                    if i == 2:
                        gb2 = bcast_row(gate2, "gb2")
                        batch = units[0:3]
                    elif i == 3:
                        batch = units[3:6]
                    else:
                        batch = []
                    for u in batch:
                        full_unit(u, gb2, dve=(u % 8 in (0, 2, 4)))
                else:
                    if i == 1:
                        bcast_ch(gate3, gb3, 0)
                        for u in units[6:8]:
                            full_unit(u, gb2, dve=(u % 8 == 6))
                    elif i == 2:
                        for u in range(24, 28):
                            epi3_ch(u, 0)
                    elif i == 3:
                        epi3_ch(28, 0)
            emit_dr1(3, *dr_prev, p, last_gg=True)

        # ---------- tail: chunk 1 of the group-3 gate + epilogue ----------
        # TAIL3 conv matmuls cover the chain; their silus follow its Act ops
        emit_d_s3(3, T3, chs=(1,))
        acc_pools = [epsum, mmp]
        emit_d(3, T3, mul_eng=nc.vector, chs=(1,))
        tail_accs = [conv_unit(u, pools=acc_pools, defer_silu=True)
                     for u in TAIL3]
        bcast_ch(gate3, gb3, 1)
        for j, u in enumerate(TAIL3):
            sacc = opool.tile([128, NTOK], F16, name=f"sacc{u}",
                              tag=f"sacc3_{u}", bufs=1)
            for ch in range(NCH):
                nc.scalar.activation(sacc[:, ch * CHW:(ch + 1) * CHW],
                                     tail_accs[j][ch], AF.Silu)
            sacc3[u] = sacc
        for u in TAIL3:
            epi3_ch(u, 0)
        for u in range(24, 32):
            epi3_ch(u, 1)

        for p in (cacc, dgpool, opool, vpool, npool, rowsc, rowm, kqpool, scr,
                  epsum, sump, mmp, qpool, kwpool, consts):
            p.release()
    return nc


def host_prep(embeddings, hidden_states, key_w, key_b, value_w, value_b,
              w_key_norm, w_query_norm, w_norm, conv_weight):
    """Build the per-core input maps."""
    f32, f16 = np.float32, np.float16
    e4 = ml_dtypes.float8_e4m3fn
    embeddings = np.asarray(embeddings, f32)
    hidden_states = np.asarray(hidden_states, f32)
    key_w = np.asarray(key_w, f32)
    key_b = np.asarray(key_b, f32)
    value_w = np.asarray(value_w, f32)
    value_b = np.asarray(value_b, f32)
    w_key_norm = np.asarray(w_key_norm, f32)
    w_query_norm = np.asarray(w_query_norm, f32)
    w_norm = np.asarray(w_norm, f32)
    conv_weight = np.asarray(conv_weight, f32)

    kwT = np.ascontiguousarray(key_w.T).astype(f16)        # [E, GC]
    vwT = np.ascontiguousarray(value_w.T).astype(f16)      # [E, C]
    keyb_r = np.ascontiguousarray(key_b.reshape(NGCT, 128).T)  # [128, NGCT]
    valb_r = np.ascontiguousarray(value_b.reshape(NCT, 128).T)
    wkq = (w_key_norm * w_query_norm).reshape(GC)

    # one-hot lhsT tables. ONE shared [16, NTOK] psum accumulator with
    # disjoint rows: ak_g = row g, aq_g = 4+g, dot_g = 8+g, sv = 12.
    # (engines can only address 32-aligned partition bases, so rows are
    #  matmul-extracted after an Act bounce of the block to partition 0)
    lk8 = np.zeros((128, 9, 2, 16), f32)
    for g in range(G):
        lk8[:, g, :, g] = 1.0          # ksq -> row g
        lk8[:, 4 + g, :, 4 + g] = 1.0  # qsq -> row 4+g
    lk8[:, 8, :, 12] = 1.0             # vsq -> row 12 (sv)
    lk8 = lk8.reshape(128, 288).astype(e4)

    lkq = np.zeros((NGCT, 128, 16), f32)
    for gct in range(NGCT):
        g = gct // NCT
        lkq[gct, :, 8 + g] = wkq[gct * 128:(gct + 1) * 128]

    aux16 = np.zeros((128, 16 + 256), f16)
    aux16[:, 12] = 1.0        # lv one-hot: vsq -> row 12 (sv)
    for j in range(2):
        aux16[j, 16 + j * 128:16 + (j + 1) * 128] = 1.0

    # ceps: cols 0-3 = per-stage bias vectors (+C*EPS on ak/aq rows);
    #        cols 8+r = f32 one-hot row selectors (identity)
    ceps_h = np.zeros((128, 24), f32)
    for g in range(G):
        ceps_h[g, g] = float(C) * EPS
        ceps_h[4 + g, g] = float(C) * EPS
    for r in range(16):
        ceps_h[r, 8 + r] = 1.0
    ceps_h[0, 6] = NORM_EPS
    ceps_h[0, 7] = 1e-60

    # f16 diagonal conv weights + identity for the halo-fix matmul.
    cwf = (conv_weight.reshape(G, C, KT) * w_norm[:, :, None]).astype(f32)
    dg = np.zeros((NGCT, 128, KT * 128), f16)
    idx = np.arange(128)
    for gct in range(NGCT):
        g, ct = gct // NCT, gct % NCT
        for k in range(KT):
            dg[gct, idx, k * 128 + idx] = cwf[g, ct * 128 + idx, k].astype(f16)
    id16_h = np.zeros((128, 128), f16)
    id16_h[idx, idx] = 1.0
    cwf_r = np.zeros((128, NGCT * KT), f32)
    for gct in range(NGCT):
        g, ct = gct // NCT, gct % NCT
        for k in range(KT):
            cwf_r[:, gct * KT + k] = cwf[g, ct * 128:(ct + 1) * 128, k]

    in_maps = []
    for core in range(NCORES):
        b = core // (NCORES // B)
        t0 = (core % (NCORES // B)) * NTOK
        emb_s = embeddings[b, t0:t0 + NTOK]                # [NTOK, E]
        hid_s = hidden_states[b, t0:t0 + NTOK].reshape(NTOK, GC)
        emb_c = np.ascontiguousarray(emb_s.T).astype(f16)  # [E, NTOK]
        hid_c = np.ascontiguousarray(hid_s.T).astype(f16)  # [GC, NTOK]

        # halo: nhat (= value / rms_v, w_norm NOT applied) for the 9
        # preceding tokens feeds a host-computed conv correction hc for the
        # first 9 output tokens; zeros at the sequence start.
        if t0 == 0:
            hc_c = np.zeros((128, NGCT * HALO), f16)
        else:
            th = slice(t0 - HALO, t0)
            e9 = embeddings[b, th]                          # [9, E]
            k9 = (e9 @ key_w.T + key_b).reshape(HALO, G, C)
            q9 = hidden_states[b, th]                       # [9, G, C]
            rk = np.sqrt((k9 * k9).mean(-1) + EPS)
            rq = np.sqrt((q9 * q9).mean(-1) + EPS)
            d9 = np.einsum("tgc,gc,tgc,gc->tg", k9, w_key_norm, q9,
                           w_query_norm)
            graw = d9 / (rk * rq) / np.sqrt(f32(C))
            g9 = 1.0 / (1.0 + np.exp(-(np.where(graw >= 0, 1.0, -1.0)
                                       * np.sqrt(np.maximum(np.abs(graw),
                                                            1e-6)))))
            vp9 = e9 @ value_w.T + value_b                  # [9, C]
            val9 = vp9[:, None, :] * g9[..., None].astype(f32)
            rv9 = np.sqrt((val9 * val9).mean(-1) + NORM_EPS)
            nhat9 = val9 / rv9[..., None]                   # [9, G, C]
            # hc[c, gct, t] = sum_{k: t+k*DIL<9} cwf[g,c,k]*nhat9[t+k*DIL,g,c]
            hcf = np.zeros((HALO, G, C), f32)
            for t in range(HALO):
                for k in range(KT):
                    ix = t + k * DIL
                    if ix < HALO:
                        hcf[t] += cwf[:, :, k] * nhat9[ix]
            hg = hcf.transpose(1, 2, 0).reshape(NGCT, 128, HALO)
            hc_c = np.ascontiguousarray(
                hg.transpose(1, 0, 2).reshape(128, NGCT * HALO)).astype(f16)

        in_maps.append({
            "emb16": emb_c, "hidT": hid_c, "kwT": kwT, "vwT": vwT,
            "keyb": keyb_r, "valb": valb_r,
            "lk8": lk8, "lkq": lkq, "aux16": aux16, "ceps": ceps_h,
            "dg16": dg, "cwf": cwf_r, "hc": hc_c, "id16": id16_h,
        })
    return in_maps


_NC_CACHE = [None]
LAST_RESULT = [None]


def kernel(**inputs) -> np.ndarray:
    in_maps = host_prep(**inputs)
    if _NC_CACHE[0] is None:
        _NC_CACHE[0] = build_program()
    nc = _NC_CACHE[0]
    res = run_bass_kernel_spmd(nc, in_maps, list(range(NCORES)))
    LAST_RESULT[0] = res
    out = np.empty((B, T, G, C), np.float32)
    for core in range(NCORES):
        b = core // (NCORES // B)
        t0 = (core % (NCORES // B)) * NTOK
        oc = np.asarray(res.results[core]["out"]).astype(np.float32)
        out[b, t0:t0 + NTOK] = oc.reshape(G, C, NTOK).transpose(2, 0, 1)
    return out


# revision 116
# speedup vs baseline: 1.0039x; 1.0009x over previous
"""Trainium2 Bass kernel for the EngramNew module (dense_cnn), v3.

Sharding: B*T = 8192 tokens split across 8 cores (1024 tokens each); the conv
halo of (K-1)*DIL = 9 tokens is precomputed host-side.  On-device layout is
channels-on-partitions / tokens-on-free: [G*C, T_core].

v10 design vs the v2 baseline (291.8us -> 225.1us):
 - shared rms_v normalizer: rms_v = sqrt(gate^2*mean(vproj^2)+eps)
   ~= gate*sqrt(mean(vproj^2)+eps) since gate = sigmoid(..) > 0, so the
   conv input (normed) = vproj*alpha with ONE shared alpha row; only the
   residual needs the per-group gate.  This decouples the whole conv
   pipeline from the gate chains (error <= ~1e-3, verified vs reference).
 - ONE [16,NTOK] PSUM accumulator shared by all four gate-sum stages via
   disjoint one-hot rows (ak_g=g, aq_g=4+g, dot_g=8+g, sv=12), reset once:
   no PSUM bank-rotation WAR stalls.  Rows are bounced to partition 0 by
   an Act copy + f32r one-hot extraction matmuls (engines can only address
   32-aligned partition bases).
 - per-ct conv input tiles (zero prefix + vproj*alpha) shared by all 4
   groups; the halo contribution to the first 9 outputs is a host-computed
   correction added via an identity matmul.  15 of 32 conv units run as
   DVE MAC chains, the rest as PE diag matmuls.
 - C(3) is split into two per-chunk passes so chunk 0 of the stage-3 gate
   chain + epilogue hides under the chunk-1 pass; 3 conv units are held
   back as PE cover for the chunk-1 chain.
 - kq / ksq+qsq(fp8 DR) / vsq(fp8 DR) reductions all deferred by one
   gg/vv so their producers never stall the PE sum matmuls.
 - startup: split vw/emb DMAs ordered first + 4-psum et-outer first vproj
   pass so PE starts at ~4us instead of 13us.
"""

import os
import sys

for _p in ("/opt/trn_rl_repo",):
    if _p not in sys.path:
        sys.path.insert(0, _p)

import numpy as np
import ml_dtypes

import concourse.bass as bass
from concourse import mybir
from concourse.tile import TileContext
from concourse.bass_utils import run_bass_kernel_spmd
import bass_rust

F32 = mybir.dt.float32
F32R = mybir.dt.float32r
F16 = mybir.dt.float16
FP8 = mybir.dt.float8e4
AF = mybir.ActivationFunctionType
ALU = mybir.AluOpType
DR = mybir.MatmulPerfMode.DoubleRow

# Problem constants (hardcoded per spec nn_EngramNew_2070174237244)
B, T, G, C, E = 2, 4096, 4, 1024, 1024
GC = G * C
KT, DIL = 4, 3          # conv taps / dilation
EPS = 1e-5
NORM_EPS = 1e-5
NCORES = 8
NTOK = (B * T) // NCORES    # 1024 tokens per core
HALO = (KT - 1) * DIL       # 9
NET = E // 128              # 8 e-tiles
NGCT = GC // 128            # 32 gc-tiles
NCT = C // 128              # 8 c-tiles
CHW = 512                   # token chunk width (1 PSUM bank of fp32)
NCH = NTOK // CHW           # 2 chunks



class PatchedTileContext(TileContext):
    """This walrus build allows only one sem wait per instruction (two on
    EventSemaphore). Tile attaches as many waits as an instruction needs,
    so after scheduling we hoist excess waits onto no-op instructions
    inserted just before the owner on the same engine (engines are strict
    FIFO, so observing the sems earlier is equivalent)."""

    def _split_excess_waits(self):
        nc = self.nc

        def make_nop(engine):
            bi = nc.engines[engine].nop()
            bb = nc.cur_bb.bb
            lst = list(bb.instructions)
            assert lst[-1] is bi.ins
            bb.instructions = lst[:-1]
            return bi.ins

        # Phase 1: snapshot every block BEFORE creating any nop, so nops
        # appended to cur_bb can never leak into the iteration or the rebuilt
        # lists (cur_bb may be one of the blocks being processed).
        snapshots = []
        for f in nc.m.functions:
            for blk in f.blocks:
                snapshots.append((blk, list(blk.instructions)))

        for blk, insts in snapshots:
            out = []
            changed = False
            for ins in insts:
                si = ins.sync_info
                waits = list(si.on_wait) if (si and si.on_wait) else []
                cap = 2 if isinstance(ins, mybir.InstEventSemaphore) else 1
                if len(waits) > cap:
                    changed = True
                    for w in waits[cap:]:
                        nop = make_nop(ins.engine)
                        nop.sync_info = bass_rust.SyncInfo(
                            on_wait=[w], on_update=[]
                        )
                        out.append(nop)
                    upd = list(si.on_update) if si.on_update else []
                    ins.sync_info = bass_rust.SyncInfo(
                        on_wait=waits[:cap], on_update=upd
                    )
                out.append(ins)
            if changed:
                blk.instructions = out

    def _drain_and_barrier(self, tick_clock, wait_clock):
        super()._drain_and_barrier(tick_clock, wait_clock)
        self._split_excess_waits()


def _r(ap):
    return ap.bitcast(F32R)


def build_program():
    nc = bass.Bass()

    # ---- DRAM parameters ----
    emb16 = nc.declare_dram_parameter("emb16", [E, NTOK], F16, isOutput=False)
    hidT = nc.declare_dram_parameter("hidT", [GC, NTOK], F16, isOutput=False)
    kwT = nc.declare_dram_parameter("kwT", [E, GC], F16, isOutput=False)
    vwT = nc.declare_dram_parameter("vwT", [E, C], F16, isOutput=False)
    keyb = nc.declare_dram_parameter("keyb", [128, NGCT], F32, isOutput=False)
    valb = nc.declare_dram_parameter("valb", [128, NCT], F32, isOutput=False)
    lk8 = nc.declare_dram_parameter("lk8", [128, 9 * 2 * 16], FP8,
                                    isOutput=False)
    lkq = nc.declare_dram_parameter("lkq", [NGCT, 128, 16], F32, isOutput=False)
    aux16 = nc.declare_dram_parameter("aux16", [128, 16 + 256], F16,
                                      isOutput=False)
    ceps = nc.declare_dram_parameter("ceps", [128, 24], F32, isOutput=False)
    dg16 = nc.declare_dram_parameter("dg16", [NGCT, 128, KT * 128], F16,
                                     isOutput=False)
    cwf = nc.declare_dram_parameter("cwf", [128, NGCT * KT], F32,
                                    isOutput=False)
    hc = nc.declare_dram_parameter("hc", [128, NGCT * HALO], F16,
                                   isOutput=False)
    id16 = nc.declare_dram_parameter("id16", [128, 128], F16, isOutput=False)
    out_d = nc.declare_dram_parameter("out", [GC, NTOK], F16, isOutput=True)

    with PatchedTileContext(nc) as tc:
        consts = tc.alloc_tile_pool(name="consts", bufs=1)
        kwpool = tc.alloc_tile_pool(name="kwpool", bufs=2)
        qpool = tc.alloc_tile_pool(name="qpool", bufs=3)
        mmp = tc.alloc_tile_pool(name="mmp", bufs=3, space=bass.MemorySpace.PSUM)
        sump = tc.alloc_tile_pool(name="sump", bufs=1, space=bass.MemorySpace.PSUM)
        epsum = tc.alloc_tile_pool(name="epsum", bufs=3,
                                   space=bass.MemorySpace.PSUM)
        scr = tc.alloc_tile_pool(name="scr", bufs=4)
        kqpool = tc.alloc_tile_pool(name="kqpool", bufs=4)
        rowm = tc.alloc_tile_pool(name="rowm", bufs=1)
        rowsc = tc.alloc_tile_pool(name="rowsc", bufs=9)
        npool = tc.alloc_tile_pool(name="npool", bufs=3)
        vpool = tc.alloc_tile_pool(name="vpool", bufs=3)
        opool = tc.alloc_tile_pool(name="opool", bufs=4)
        dgpool = tc.alloc_tile_pool(name="dgpool", bufs=3)
        cacc = tc.alloc_tile_pool(name="cacc", bufs=3)

        # ---- load order: vw(vv0) first, then emb per-et, then small consts
        vw_t0 = kwpool.tile([128, NET, 256], F16, name="vw_t0", tag="w")
        for eh in range(2):
            nc.sync.dma_start(
                out=vw_t0[:, eh * 4:(eh + 1) * 4, :],
                in_=vwT.rearrange("(et p) c -> p et c", p=128)[
                    :, eh * 4:(eh + 1) * 4, 0:256],
            )
        emb_all = consts.tile([128, NET, NTOK], F16)
        for et in range(NET):
            nc.sync.dma_start(out=emb_all[:, et, :],
                              in_=emb16[et * 128:(et + 1) * 128, :])
        vw_t1 = kwpool.tile([128, NET, 256], F16, name="vw_t1", tag="w")
        nc.sync.dma_start(
            out=vw_t1,
            in_=vwT.rearrange("(et p) c -> p et c", p=128)[:, :, 256:512],
        )
        valb_sb = consts.tile([128, NCT], F32)
        nc.sync.dma_start(out=valb_sb, in_=valb[:, :])
        aux_sb = consts.tile([128, 16 + 256], F16)
        nc.sync.dma_start(out=aux_sb, in_=aux16[:, :])
        ceps_sb = consts.tile([128, 24], F32)
        nc.sync.dma_start(out=ceps_sb, in_=ceps[:, :])
        cepr_sb = consts.tile([128, 24], F32R)
        nc.sync.dma_start(out=cepr_sb, in_=_r(ceps[:, :]))
        keyb_sb = consts.tile([128, NGCT], F32)
        nc.sync.dma_start(out=keyb_sb, in_=keyb[:, :])
        lk8_sb = consts.tile([128, 9, 2, 16], FP8)
        nc.sync.dma_start(out=lk8_sb,
                          in_=lk8.rearrange("p (q i c) -> p q i c", i=2, c=16))
        lkq_sb = consts.tile([128, NGCT, 16], F32R)
        nc.sync.dma_start(out=lkq_sb, in_=_r(lkq.rearrange("n p m -> p n m")))
        cwf_sb = consts.tile([128, NGCT * KT], F32)
        nc.sync.dma_start(out=cwf_sb, in_=cwf[:, :])
        hc_sb = consts.tile([128, NGCT, HALO], F16)
        nc.sync.dma_start(out=hc_sb,
                          in_=hc.rearrange("p (n h) -> p n h", h=HALO))
        id16_sb = consts.tile([128, 128], F16)
        nc.sync.dma_start(out=id16_sb, in_=id16[:, :])
        vproj16 = consts.tile([128, NCT, NTOK], F16)
        bc2_sb = aux_sb[0:1, 16:16 + 128]

        # ---- gate sums: ONE [16, NTOK] psum shared by all stages via
        # disjoint one-hot rows: ak_g = row g, aq_g = 4+g, dot_g = 8+g,
        # sv = 12. Reset once (B's first vsq sum); everything accumulates.
        sums_all = sump.tile([16, NTOK], F32, name="sums_all", tag="sums")
        first_sum = [True] * NCH

        def sum_mm(stage, lhsT, rhs, ch, last=False, perf_mode=None):
            st = first_sum[ch]
            first_sum[ch] = False
            nc.tensor.matmul(
                sums_all[:, ch * CHW:(ch + 1) * CHW],
                lhsT, rhs, start=st, stop=last,
                perf_mode=perf_mode, skip_group_check=True,
            )

        # ---------- stage B: vproj = value_w @ emb + value_b ----------
        # vsq in fp8 (feeds only alpha), DoubleRow-reduced, deferred one vv
        pend_vsq = None

        def flush_vsq(v8):
            for ch in range(NCH):
                cols = slice(ch * CHW, (ch + 1) * CHW)
                sum_mm(3, lk8_sb[:, 8, :, :], v8[:, :, cols], ch,
                       perf_mode=DR)

        for vv in range(NCT // 2):
            if vv == 0:
                vw_t = vw_t0
            elif vv == 1:
                vw_t = vw_t1
            else:
                vw_t = kwpool.tile([128, NET, 256], F16, name="vw_t", tag="w")
                nc.sync.dma_start(
                    out=vw_t,
                    in_=vwT.rearrange("(et p) c -> p et c", p=128)[
                        :, :, vv * 256:(vv + 1) * 256],
                )
            vsq = scr.tile([128, 2, NTOK], FP8, name="vsq8", tag="p8")
            if vv == 0:
                # et-outer across 4 psums so PE rate-matches the emb DMAs
                ps4 = [mmp.tile([128, CHW], F32, name=f"psB0_{i}", tag="mm")
                       for i in range(3)]
                ps4.append(epsum.tile([128, CHW], F32, name="psB0_3",
                                      tag="mm"))
                for et in range(NET):
                    for i in range(4):
                        s2, ch = i // 2, i % 2
                        nc.tensor.matmul(
                            ps4[i],
                            vw_t[:, et, s2 * 128:(s2 + 1) * 128],
                            emb_all[:, et, ch * CHW:(ch + 1) * CHW],
                            start=(et == 0), stop=(et == NET - 1),
                        )
                for i in range(4):
                    s2, ch = i // 2, i % 2
                    ct = vv * 2 + s2
                    cols = slice(ch * CHW, (ch + 1) * CHW)
                    nc.scalar.activation(
                        vproj16[:, ct, cols], ps4[i],
                        AF.Identity, bias=valb_sb[:, ct:ct + 1], scale=1.0,
                    )
                    nc.scalar.activation(
                        vsq[:, s2, cols], ps4[i], AF.Square,
                        bias=valb_sb[:, ct:ct + 1], scale=1.0,
                    )
            else:
                for s2 in range(2):
                    ct = vv * 2 + s2
                    for ch in range(NCH):
                        cols = slice(ch * CHW, (ch + 1) * CHW)
                        ps = mmp.tile([128, CHW], F32, name="psB", tag="mm")
                        for et in range(NET):
                            nc.tensor.matmul(
                                ps,
                                vw_t[:, et, s2 * 128:(s2 + 1) * 128],
                                emb_all[:, et, ch * CHW:(ch + 1) * CHW],
                                start=(et == 0), stop=(et == NET - 1),
                            )
                        nc.scalar.activation(
                            vproj16[:, ct, cols], ps,
                            AF.Identity, bias=valb_sb[:, ct:ct + 1], scale=1.0,
                        )
                        nc.scalar.activation(
                            vsq[:, s2, cols], ps, AF.Square,
                            bias=valb_sb[:, ct:ct + 1], scale=1.0,
                        )
                if pend_vsq is not None:
                    flush_vsq(pend_vsq)
                    pend_vsq = None
            pend_vsq = vsq
        flush_vsq(pend_vsq)

        # ---------- stage C for one group-pair ----------
        def emit_c_kq(stage, gg):
            """k path for double-gct gg (two gc tiles); DR sums deferred."""
            kw_t = kwpool.tile([128, NET, 256], F16, name="kw_t", tag="w")
            nc.sync.dma_start(
                out=kw_t,
                in_=kwT.rearrange("(et p) c -> p et c", p=128)[
                    :, :, gg * 256:(gg + 1) * 256],
            )
            ksqp = scr.tile([128, 2, NTOK], FP8, name="ksqp", tag="p8")
            qsqp = scr.tile([128, 2, NTOK], FP8, name="qsqp", tag="p8")
            kqs = []
            for s2 in range(2):
                gct = gg * 2 + s2
                q_sb = qpool.tile([128, NTOK], F16, name="q_sb", tag="q")
                nc.sync.dma_start(
                    out=q_sb, in_=hidT[gct * 128:(gct + 1) * 128, :]
                )
                kq = kqpool.tile([128, NTOK], F32R, name="kq", tag="kq")
                for ch in range(NCH):
                    ps = mmp.tile([128, CHW], F32, name="psC", tag="mm")
                    for et in range(NET):
                        nc.tensor.matmul(
                            ps,
                            kw_t[:, et, s2 * 128:(s2 + 1) * 128],
                            emb_all[:, et, ch * CHW:(ch + 1) * CHW],
                            start=(et == 0), stop=(et == NET - 1),
                        )
                    cols = slice(ch * CHW, (ch + 1) * CHW)
                    nc.scalar.activation(
                        ksqp[:, s2, cols], ps, AF.Square,
                        bias=keyb_sb[:, gct:gct + 1], scale=1.0,
                    )
                    nc.gpsimd.tensor_mul(qsqp[:, s2, cols], q_sb[:, cols],
                                         q_sb[:, cols])
                    nc.vector.scalar_tensor_tensor(
                        kq[:, cols], ps, keyb_sb[:, gct:gct + 1],
                        q_sb[:, cols], op0=ALU.add, op1=ALU.mult,
                    )
                kqs.append((gct, kq))
            return ksqp, qsqp, kqs

        def emit_dr(stage, ksqp, qsqp, kqs, last_gg):
            for gct, kq in kqs:
                for ch in range(NCH):
                    sum_mm(stage, lkq_sb[:, gct, :],
                           kq[:, ch * CHW:(ch + 1) * CHW], ch)
            for ch in range(NCH):
                cols = slice(ch * CHW, (ch + 1) * CHW)
                sum_mm(stage, lk8_sb[:, stage, :, :], ksqp[:, :, cols], ch,
                       perf_mode=DR)
                sum_mm(stage, lk8_sb[:, 4 + stage, :, :], qsqp[:, :, cols],
                       ch, last=last_gg, perf_mode=DR)

        def emit_c_kq1(stage, gg, ch):
            """Single-chunk variant (window-3 ch-split passes)."""
            cols = slice(ch * CHW, (ch + 1) * CHW)
            kw_t = kwpool.tile([128, NET, 256], F16, name="kw_t", tag="w")
            nc.sync.dma_start(
                out=kw_t,
                in_=kwT.rearrange("(et p) c -> p et c", p=128)[
                    :, :, gg * 256:(gg + 1) * 256],
            )
            ksqp = scr.tile([128, 2, CHW], FP8, name="ksqp1", tag="p8")
            qsqp = scr.tile([128, 2, CHW], FP8, name="qsqp1", tag="p8")
            kqs = []
            for s2 in range(2):
                gct = gg * 2 + s2
                q_sb = qpool.tile([128, CHW], F16, name="q_sb1", tag="q")
                nc.sync.dma_start(
                    out=q_sb, in_=hidT[gct * 128:(gct + 1) * 128, cols]
                )
                kq = kqpool.tile([128, CHW], F32R, name="kq1", tag="kq")
                ps = mmp.tile([128, CHW], F32, name="psC", tag="mm")
                for et in range(NET):
                    nc.tensor.matmul(
                        ps,
                        kw_t[:, et, s2 * 128:(s2 + 1) * 128],
                        emb_all[:, et, cols],
                        start=(et == 0), stop=(et == NET - 1),
                    )
                nc.scalar.activation(
                    ksqp[:, s2, :], ps, AF.Square,
                    bias=keyb_sb[:, gct:gct + 1], scale=1.0,
                )
                nc.gpsimd.tensor_mul(qsqp[:, s2, :], q_sb, q_sb)
                nc.vector.scalar_tensor_tensor(
                    kq, ps, keyb_sb[:, gct:gct + 1],
                    q_sb, op0=ALU.add, op1=ALU.mult,
                )
                kqs.append((gct, kq))
            return ksqp, qsqp, kqs

        def emit_dr1(stage, ksqp, qsqp, kqs, ch, last_gg):
            for gct, kq in kqs:
                sum_mm(stage, lkq_sb[:, gct, :], kq, ch)
            sum_mm(stage, lk8_sb[:, stage, :, :], ksqp, ch, perf_mode=DR)
            sum_mm(stage, lk8_sb[:, 4 + stage, :, :], qsqp, ch, last=last_gg,
                   perf_mode=DR)

        # ---------- stage D ----------
        # Shared rms_v normalizer: rms_v = sqrt(gate^2*mean(vproj^2)+eps)
        # ~= gate*sqrt(mean(vproj^2)+eps) since gate=sigmoid(..)>0, so the
        # conv input normed = vproj*alpha with ONE shared alpha row; only the
        # residual (value = vproj*gate) needs the per-group gate.
        def emit_alpha():
            # sv (= sum vproj^2) sits at psum row 12: bounce the block to
            # SBUF and matmul-extract the row to partition 0.
            s3a = rowm.tile([16, NTOK], F32R, name="s3a", tag="svz")
            aln = rowsc.tile([1, NTOK], F32, name="aln", tag="rs")
            alpha16 = rowm.tile([1, NTOK], F16, name="alpha16", tag="alpha16")
            nc.scalar.activation(s3a, sums_all[:, :], AF.Copy)
            for ch in range(NCH):
                cols = slice(ch * CHW, (ch + 1) * CHW)
                p = epsum.tile([1, CHW], F32, name="svx", tag="mm")
                nc.tensor.matmul(p, cepr_sb[0:16, 20:21], s3a[:, cols],
                                 start=True, stop=True)
                nc.scalar.activation(aln[:, cols], p, AF.Ln,
                                     bias=ceps_sb[0:1, 6:7],
                                     scale=1.0 / float(C))
            nc.scalar.activation(alpha16, aln, AF.Exp, scale=-0.5)
            return alpha16

        def make_d_tiles(stage):
            T = {}
            for nm in ("p4", "lnp", "lnd", "lng", "sqg", "sgn", "ss4", "ab4",
                       "akr"):
                T[nm] = rowsc.tile([1, NTOK], F32, name=f"{nm}{stage}",
                                   tag="rs")
            T["gate16"] = rowm.tile([1, NTOK], F16, name=f"gate16{stage}",
                                    tag="gate16")
            T["s3"] = rowm.tile([16, NTOK], F32R, name=f"s3_{stage}",
                                tag="ext")
            return T

        def emit_d_s3(stage, T, chs=(0, 1)):
            """Psum sums -> partition-0-based SBUF bounce (+ stage biases)."""
            for ch in chs:
                sl = slice(ch * CHW, (ch + 1) * CHW)
                nc.scalar.activation(T["s3"][:, sl], sums_all[:, sl],
                                     AF.Identity,
                                     bias=ceps_sb[0:16, stage:stage + 1],
                                     scale=1.0)

        def emit_d(stage, T, mul_eng=None, chs=(0, 1)):
            """Per-group gate chain: gate = sigmoid(sign(dot)*sqrt(|graw|)).

            Engines only address partitions at 32-boundaries, so the psum
            region is Act-copied (aligned base -> partition 0) to s3, and
            rows 1+ are pulled to partition-0 psum via one-hot matmuls.
            Row layout: stages 0-2: [ak, aq, dot]; stage 3: [sv, aq, dot, ak].
            """
            me = mul_eng if mul_eng is not None else nc.vector
            s3 = T["s3"]
            p4, lnp, lnd, lng, sqg, sgn, ss4, ab4, gate16 = (
                T["p4"], T["lnp"], T["lnd"], T["lng"], T["sqg"], T["sgn"],
                T["ss4"], T["ab4"], T["gate16"])
            akr = T["akr"]
            if chs == (0, 1):
                sls = [slice(0, NTOK)]
            else:
                sls = [slice(ch * CHW, (ch + 1) * CHW) for ch in chs]

            def extract(row, ch):
                sel = cepr_sb[0:16, 8 + row:9 + row]
                p = epsum.tile([1, CHW], F32, name=f"x{row}_{stage}",
                               tag="mm")
                nc.tensor.matmul(p, sel,
                                 s3[:, ch * CHW:(ch + 1) * CHW],
                                 start=True, stop=True)
                return p

            # first layer reads the [1, CHW] psums (partition 0), per chunk
            for ch in chs:
                cols = slice(ch * CHW, (ch + 1) * CHW)
                ak_ps = extract(stage, ch)
                aq_ps = extract(4 + stage, ch)
                dot_ps = extract(8 + stage, ch)
                nc.scalar.activation(akr[:, cols], ak_ps, AF.Copy)
                nc.scalar.activation(ab4[:, cols], dot_ps, AF.Square)
                nc.scalar.activation(sgn[:, cols], dot_ps, AF.Sign)
                nc.vector.tensor_mul(p4[:, cols], akr[:, cols], aq_ps)
            # 2ln|dot| and ln(p4/C); 2ln|graw| = 2ln|dot| - ln(p4/C)
            # (plain subtract so the mul engine can be Pool)
            for sl in sls:
                nc.scalar.activation(lnd[:, sl], ab4[:, sl], AF.Ln,
                                     bias=ceps_sb[0:1, 7:8])
            for sl in sls:
                nc.scalar.activation(lnp[:, sl], p4[:, sl], AF.Ln,
                                     scale=1.0 / float(C))
            for sl in sls:
                me.tensor_sub(lng[:, sl], lnd[:, sl], lnp[:, sl])
            for sl in sls:
                nc.scalar.activation(sqg[:, sl], lng[:, sl], AF.Exp,
                                     scale=0.25)
            for sl in sls:
                me.tensor_mul(ss4[:, sl], sqg[:, sl], sgn[:, sl])
            for sl in sls:
                nc.scalar.activation(gate16[:, sl], ss4[:, sl], AF.Sigmoid)
            return gate16

        # ---------- stage E ----------
        def bcast_ch(src, dst, ch):
            bp = epsum.tile([128, CHW], F32, name="bp", tag="mm")
            nc.tensor.matmul(
                bp, bc2_sb[0:1, 0:128],
                src[:, ch * CHW:(ch + 1) * CHW],
                start=True, stop=True,
            )
            nc.scalar.activation(
                dst[:, ch * CHW:(ch + 1) * CHW], bp, AF.Copy)

        def bcast_row(src, tag):
            """[1, NTOK] f32/f16 row -> [128, NTOK] f16 via PE broadcast."""
            dst = rowm.tile([128, NTOK], F16, name=f"b_{tag}", tag=tag)
            for ch in range(NCH):
                bcast_ch(src, dst, ch)
            return dst

        # nx16[ct]: f16 conv input, shared by all 4 groups' units:
        # [9 zeros | vproj*alpha]; the halo contribution to the first 9
        # outputs is a host-computed f16 correction (hc) accumulated via an
        # identity matmul.
        PADW = HALO + NTOK
        nx8s = {}

        def emit_nx8(ct):
            nx8 = npool.tile([128, PADW], F16, name=f"nx16_{ct}",
                             tag=f"nx16_{ct}", bufs=1)
            nc.gpsimd.memset(nx8[:, 0:HALO], 0.0)
            nc.vector.tensor_mul(nx8[:, HALO:HALO + NTOK],
                                 vproj16[:, ct, :], ab16)
            nx8s[ct] = nx8

        def emit_val(gct, gb16, on_pool=False):
            ct = gct % NCT
            val = vpool.tile([128, NTOK], F16, name="val", tag="val")
            if on_pool:
                nc.gpsimd.tensor_mul(val, vproj16[:, ct, :], gb16)
            else:
                nc.vector.tensor_mul(val, vproj16[:, ct, :], gb16)
            return val

        def emit_e_conv_pe(gct, pools=None):
            """f16 conv taps + halo-fix matmul."""
            ct = gct % NCT
            nx8 = nx8s[ct]
            dg_t = dgpool.tile([128, KT * 128], F16, name="dg_t", tag="dg")
            nc.sync.dma_start(out=dg_t, in_=dg16[gct])
            accs = []
            for ch in range(NCH):
                pool = (pools[ch % len(pools)] if pools else epsum)
                acc = pool.tile([128, CHW], F32, name="acc", tag="mm")
                for k in range(KT):
                    base = ch * CHW + k * DIL
                    nc.tensor.matmul(
                        acc,
                        dg_t[:, k * 128:(k + 1) * 128],
                        nx8[:, base:base + CHW],
                        start=(k == 0), stop=(k == KT - 1 and ch == 1),
                        skip_group_check=True,
                    )
                if ch == 0:
                    nc.tensor.matmul(
                        acc[:, 0:HALO], id16_sb, hc_sb[:, gct, :],
                        start=False, stop=True, skip_group_check=True,
                    )
                accs.append(acc)
            return accs

        def emit_e_conv_dve(gct):
            """f16 conv as DVE scalar-ptr MAC chains (+ in-place halo fix)."""
            ct = gct % NCT
            nx8 = nx8s[ct]
            outs = []
            for ch in range(NCH):
                prev = None
                for k in range(KT):
                    win = nx8[:, ch * CHW + k * DIL:ch * CHW + k * DIL + CHW]
                    a = cacc.tile([128, CHW], F16, name=f"ca{k}", tag=f"ca{k}")
                    wcol = cwf_sb[:, gct * KT + k:gct * KT + k + 1]
                    if k == 0:
                        nc.vector.tensor_scalar_mul(a, win, wcol)
                    else:
                        nc.vector.scalar_tensor_tensor(
                            a, win, wcol, prev, op0=ALU.mult, op1=ALU.add)
                    prev = a
                if ch == 0:
                    nc.vector.tensor_tensor(prev[:, 0:HALO], prev[:, 0:HALO],
                                            hc_sb[:, gct, :], op=ALU.add)
                outs.append(prev)
            return outs

        def emit_silu(accs):
            sacc = opool.tile([128, NTOK], F16, name="sacc", tag="sacc")
            for ch in range(NCH):
                nc.scalar.activation(sacc[:, ch * CHW:(ch + 1) * CHW],
                                     accs[ch], AF.Silu)
            return sacc

        def emit_resid_out(gct, val, sacc, engine="pool"):
            ot = opool.tile([128, NTOK], F16, name="ot", tag="ot")
            if engine == "dve":
                nc.vector.tensor_tensor(ot, val, sacc, op=ALU.add)
            else:
                nc.gpsimd.tensor_add(ot, val, sacc)
            nc.sync.dma_start(out=out_d[gct * 128:(gct + 1) * 128, :], in_=ot)

        # ---------- pipeline ----------
        # conv+silu only needs the shared ab16; val/resid needs gate(g).
        # Window g: C(g) + chain(g-1) + full units of group g-1 + a few
        # group-3 conv units pulled early; tail: 3 conv units cover chain(3),
        # then group-3 val/resid.
        sacc3 = {}      # gct -> long-lived sacc for group-3 units
        ab16 = None
        TAIL3 = [29, 30, 31]
        EARLY3 = {0: [24, 25, 26], 1: [27], 2: [28], 3: []}

        def conv_unit(u, long_lived=False, pools=None, defer_silu=False,
                      dve=False):
            if dve:
                accs = emit_e_conv_dve(u)
            else:
                accs = emit_e_conv_pe(u, pools=pools)
            if defer_silu:
                return accs
            if long_lived:
                sacc = opool.tile([128, NTOK], F16, name=f"sacc{u}",
                                  tag=f"sacc3_{u}", bufs=1)
            else:
                sacc = opool.tile([128, NTOK], F16, name=f"sacc{u}",
                                  tag="sacc")
            for ch in range(NCH):
                nc.scalar.activation(sacc[:, ch * CHW:(ch + 1) * CHW],
                                     accs[ch], AF.Silu)
            if long_lived:
                sacc3[u] = sacc
            return sacc

        def full_unit(u, gb16, dve=False):
            sacc = conv_unit(u, dve=dve)
            val = emit_val(u, gb16)
            emit_resid_out(u, val, sacc, engine="pool" if dve else "dve")

        gate_prev = None
        d_tiles = {}
        for g in range(3):
            dr_prev = None
            gb16 = None
            units = list(range((g - 1) * 8, g * 8)) if g else []
            for i, gg in enumerate(range(g * 4, (g + 1) * 4)):
                if i == 0 and g:
                    d_tiles[g - 1] = make_d_tiles(g - 1)
                    emit_d_s3(g - 1, d_tiles[g - 1])
                cur = emit_c_kq(g, gg)
                if i == 0:
                    if g == 0:
                        alpha16 = emit_alpha()
                    else:
                        gate_prev = emit_d(g - 1, d_tiles[g - 1],
                                           mul_eng=nc.gpsimd)
                if dr_prev is not None:
                    emit_dr(g, *dr_prev, last_gg=False)
                dr_prev = cur
                if i == 1:
                    if g == 0:
                        ab16 = bcast_row(alpha16, "ab16")
                        for ct in (0, 1, 2):
                            emit_nx8(ct)
                    elif g == 1:
                        for ct in (6, 7):
                            emit_nx8(ct)
                    batch = []
                elif i == 2:
                    if g:
                        gb16 = bcast_row(gate_prev, f"gb{g - 1}")
                        batch = units[0:3]
                    else:
                        emit_nx8(3)
                        batch = EARLY3[0][0:2]
                elif i == 3:
                    if g == 0:
                        emit_nx8(4)
                        emit_nx8(5)
                    batch = units[3:6] if g else EARLY3[0][2:3]
                else:
                    batch = []
                for u in batch:
                    if g:
                        full_unit(u, gb16, dve=(u % 8 in (0, 2, 4)))
                    else:
                        conv_unit(u, long_lived=True, dve=True)
            emit_dr(g, *dr_prev, last_gg=True)
            if g:
                for u in units[6:8]:
                    full_unit(u, gb16, dve=(u % 8 == 6))
                for u in EARLY3[g]:
                    conv_unit(u, long_lived=True)

        # ---------- window 3: chunk-split passes ----------
        # pass p computes C(3) for token chunk p only, so the stage-3 gate
        # chain + group-3 epilogue for chunk 0 hide under pass 1.
        units = list(range(16, 24))
        d_tiles[2] = make_d_tiles(2)
        emit_d_s3(2, d_tiles[2])
        T3 = None
        gb3 = rowm.tile([128, NTOK], F16, name="b_gb3", tag="gb3")

        def epi3_ch(u, ch):
            ct = u % NCT
            cols = slice(ch * CHW, (ch + 1) * CHW)
            val = vpool.tile([128, CHW], F16, name="val3", tag="val")
            nc.vector.tensor_mul(val, vproj16[:, ct, cols], gb3[:, cols])
            ot = opool.tile([128, CHW], F16, name="ot3", tag="ot")
            nc.vector.tensor_tensor(ot, val, sacc3[u][:, cols], op=ALU.add)
            nc.sync.dma_start(out=out_d[u * 128:(u + 1) * 128, cols], in_=ot)

        for p in range(2):
            dr_prev = None
            for i, gg in enumerate(range(12, 16)):
                cur = emit_c_kq1(3, gg, p)
                if p == 0 and i == 0:
                    gate2 = emit_d(2, d_tiles[2], mul_eng=nc.gpsimd)
                if p == 1 and i == 0:
                    T3 = make_d_tiles(3)
                    emit_d_s3(3, T3, chs=(0,))
                    gate3 = emit_d(3, T3, mul_eng=nc.vector, chs=(0,))
                if dr_prev is not None:
                    emit_dr1(3, *dr_prev, p, last_gg=False)
                dr_prev = cur
                if p == 0:
                    if i == 2:
                        gb2 = bcast_row(gate2, "gb2")
                        batch = units[0:3]
                    elif i == 3:
                        batch = units[3:6]
                    else:
                        batch = []
                    for u in batch:
                        full_unit(u, gb2, dve=(u % 8 in (0, 2, 4)))
                else:
                    if i == 1:
                        bcast_ch(gate3, gb3, 0)
                        for u in units[6:8]:
                            full_unit(u, gb2, dve=(u % 8 == 6))
                    elif i == 2:
                        for u in range(24, 28):
                            epi3_ch(u, 0)
                    elif i == 3:
                        epi3_ch(28, 0)
            emit_dr1(3, *dr_prev, p, last_gg=True)

        # ---------- tail: chunk 1 of the group-3 gate + epilogue ----------
        # TAIL3 conv matmuls cover the chain; their silus follow its Act ops
        emit_d_s3(3, T3, chs=(1,))
        acc_pools = [epsum, mmp]
        emit_d(3, T3, mul_eng=nc.vector, chs=(1,))
        tail_accs = [conv_unit(u, pools=acc_pools, defer_silu=True)
                     for u in TAIL3]
        bcast_ch(gate3, gb3, 1)
        for j, u in enumerate(TAIL3):
            sacc = opool.tile([128, NTOK], F16, name=f"sacc{u}",
                              tag=f"sacc3_{u}", bufs=1)
            for ch in range(NCH):
                nc.scalar.activation(sacc[:, ch * CHW:(ch + 1) * CHW],
                                     tail_accs[j][ch], AF.Silu)
            sacc3[u] = sacc
        for u in TAIL3:
            epi3_ch(u, 0)
        for u in range(24, 32):
            epi3_ch(u, 1)

        for p in (cacc, dgpool, opool, vpool, npool, rowsc, rowm, kqpool, scr,
                  epsum, sump, mmp, qpool, kwpool, consts):
            p.release()
    return nc


def host_prep(embeddings, hidden_states, key_w, key_b, value_w, value_b,
              w_key_norm, w_query_norm, w_norm, conv_weight):
    """Build the per-core input maps."""
    f32, f16 = np.float32, np.float16
    e4 = ml_dtypes.float8_e4m3fn
    embeddings = np.asarray(embeddings, f32)
    hidden_states = np.asarray(hidden_states, f32)
    key_w = np.asarray(key_w, f32)
    key_b = np.asarray(key_b, f32)
    value_w = np.asarray(value_w, f32)
    value_b = np.asarray(value_b, f32)
    w_key_norm = np.asarray(w_key_norm, f32)
    w_query_norm = np.asarray(w_query_norm, f32)
    w_norm = np.asarray(w_norm, f32)
    conv_weight = np.asarray(conv_weight, f32)

    kwT = np.ascontiguousarray(key_w.T).astype(f16)        # [E, GC]
    vwT = np.ascontiguousarray(value_w.T).astype(f16)      # [E, C]
    keyb_r = np.ascontiguousarray(key_b.reshape(NGCT, 128).T)  # [128, NGCT]
    valb_r = np.ascontiguousarray(value_b.reshape(NCT, 128).T)
    wkq = (w_key_norm * w_query_norm).reshape(GC)

    # one-hot lhsT tables. ONE shared [16, NTOK] psum accumulator with
    # disjoint rows: ak_g = row g, aq_g = 4+g, dot_g = 8+g, sv = 12.
    # (engines can only address 32-aligned partition bases, so rows are
    #  matmul-extracted after an Act bounce of the block to partition 0)
    lk8 = np.zeros((128, 9, 2, 16), f32)
    for g in range(G):
        lk8[:, g, :, g] = 1.0          # ksq -> row g
        lk8[:, 4 + g, :, 4 + g] = 1.0  # qsq -> row 4+g
    lk8[:, 8, :, 12] = 1.0             # vsq -> row 12 (sv)
    lk8 = lk8.reshape(128, 288).astype(e4)

    lkq = np.zeros((NGCT, 128, 16), f32)
    for gct in range(NGCT):
        g = gct // NCT
        lkq[gct, :, 8 + g] = wkq[gct * 128:(gct + 1) * 128]

    aux16 = np.zeros((128, 16 + 256), f16)
    aux16[:, 12] = 1.0        # lv one-hot: vsq -> row 12 (sv)
    for j in range(2):
        aux16[j, 16 + j * 128:16 + (j + 1) * 128] = 1.0

    # ceps: cols 0-3 = per-stage bias vectors (+C*EPS on ak/aq rows);
    #        cols 8+r = f32 one-hot row selectors (identity)
    ceps_h = np.zeros((128, 24), f32)
    for g in range(G):
        ceps_h[g, g] = float(C) * EPS
        ceps_h[4 + g, g] = float(C) * EPS
    for r in range(16):
        ceps_h[r, 8 + r] = 1.0
    ceps_h[0, 6] = NORM_EPS
    ceps_h[0, 7] = 1e-60

    # f16 diagonal conv weights + identity for the halo-fix matmul.
    cwf = (conv_weight.reshape(G, C, KT) * w_norm[:, :, None]).astype(f32)
    dg = np.zeros((NGCT, 128, KT * 128), f16)
    idx = np.arange(128)
    for gct in range(NGCT):
        g, ct = gct // NCT, gct % NCT
        for k in range(KT):
            dg[gct, idx, k * 128 + idx] = cwf[g, ct * 128 + idx, k].astype(f16)
    id16_h = np.zeros((128, 128), f16)
    id16_h[idx, idx] = 1.0
    cwf_r = np.zeros((128, NGCT * KT), f32)
    for gct in range(NGCT):
        g, ct = gct // NCT, gct % NCT
        for k in range(KT):
            cwf_r[:, gct * KT + k] = cwf[g, ct * 128:(ct + 1) * 128, k]

    in_maps = []
    for core in range(NCORES):
        b = core // (NCORES // B)
        t0 = (core % (NCORES // B)) * NTOK
        emb_s = embeddings[b, t0:t0 + NTOK]                # [NTOK, E]
        hid_s = hidden_states[b, t0:t0 + NTOK].reshape(NTOK, GC)
        emb_c = np.ascontiguousarray(emb_s.T).astype(f16)  # [E, NTOK]
        hid_c = np.ascontiguousarray(hid_s.T).astype(f16)  # [GC, NTOK]

        # halo: nhat (= value / rms_v, w_norm NOT applied) for the 9
        # preceding tokens feeds a host-computed conv correction hc for the
        # first 9 output tokens; zeros at the sequence start.
        if t0 == 0:
            hc_c = np.zeros((128, NGCT * HALO), f16)
        else:
            th = slice(t0 - HALO, t0)
            e9 = embeddings[b, th]                          # [9, E]
            k9 = (e9 @ key_w.T + key_b).reshape(HALO, G, C)
            q9 = hidden_states[b, th]                       # [9, G, C]
            rk = np.sqrt((k9 * k9).mean(-1) + EPS)
            rq = np.sqrt((q9 * q9).mean(-1) + EPS)
            d9 = np.einsum("tgc,gc,tgc,gc->tg", k9, w_key_norm, q9,
                           w_query_norm)
            graw = d9 / (rk * rq) / np.sqrt(f32(C))
            g9 = 1.0 / (1.0 + np.exp(-(np.where(graw >= 0, 1.0, -1.0)
                                       * np.sqrt(np.maximum(np.abs(graw),
                                                            1e-6)))))
            vp9 = e9 @ value_w.T + value_b                  # [9, C]
            val9 = vp9[:, None, :] * g9[..., None].astype(f32)
            rv9 = np.sqrt((val9 * val9).mean(-1) + NORM_EPS)
            nhat9 = val9 / rv9[..., None]                   # [9, G, C]
            # hc[c, gct, t] = sum_{k: t+k*DIL<9} cwf[g,c,k]*nhat9[t+k*DIL,g,c]
            hcf = np.zeros((HALO, G, C), f32)
            for t in range(HALO):
                for k in range(KT):
                    ix = t + k * DIL
                    if ix < HALO:
                        hcf[t] += cwf[:, :, k] * nhat9[ix]
            hg = hcf.transpose(1, 2, 0).reshape(NGCT, 128, HALO)
            hc_c = np.ascontiguousarray(
                hg.transpose(1, 0, 2).reshape(128, NGCT * HALO)).astype(f16)

        in_maps.append({
            "emb16": emb_c, "hidT": hid_c, "kwT": kwT, "vwT": vwT,
            "keyb": keyb_r, "valb": valb_r,
            "lk8": lk8, "lkq": lkq, "aux16": aux16, "ceps": ceps_h,
            "dg16": dg, "cwf": cwf_r, "hc": hc_c, "id16": id16_h,
        })
    return in_maps


_NC_CACHE = [None]
LAST_RESULT = [None]


def kernel(**inputs) -> np.ndarray:
    in_maps = host_prep(**inputs)
    if _NC_CACHE[0] is None:
        _NC_CACHE[0] = build_program()
    nc = _NC_CACHE[0]
    res = run_bass_kernel_spmd(nc, in_maps, list(range(NCORES)))
    LAST_RESULT[0] = res
    out = np.empty((B, T, G, C), np.float32)
    for core in range(NCORES):
        b = core // (NCORES // B)
        t0 = (core % (NCORES // B)) * NTOK
        oc = np.asarray(res.results[core]["out"]).astype(np.float32)
        out[b, t0:t0 + NTOK] = oc.reshape(G, C, NTOK).transpose(2, 0, 1)
    return out


# revision 117
# speedup vs baseline: 1.0074x; 1.0035x over previous
"""Trainium2 Bass kernel for the EngramNew module (dense_cnn), v3.

Sharding: B*T = 8192 tokens split across 8 cores (1024 tokens each); the conv
halo of (K-1)*DIL = 9 tokens is precomputed host-side.  On-device layout is
channels-on-partitions / tokens-on-free: [G*C, T_core].

v10 design vs the v2 baseline (291.8us -> 225.1us):
 - shared rms_v normalizer: rms_v = sqrt(gate^2*mean(vproj^2)+eps)
   ~= gate*sqrt(mean(vproj^2)+eps) since gate = sigmoid(..) > 0, so the
   conv input (normed) = vproj*alpha with ONE shared alpha row; only the
   residual needs the per-group gate.  This decouples the whole conv
   pipeline from the gate chains (error <= ~1e-3, verified vs reference).
 - ONE [16,NTOK] PSUM accumulator shared by all four gate-sum stages via
   disjoint one-hot rows (ak_g=g, aq_g=4+g, dot_g=8+g, sv=12), reset once:
   no PSUM bank-rotation WAR stalls.  Rows are bounced to partition 0 by
   an Act copy + f32r one-hot extraction matmuls (engines can only address
   32-aligned partition bases).
 - per-ct conv input tiles (zero prefix + vproj*alpha) shared by all 4
   groups; the halo contribution to the first 9 outputs is a host-computed
   correction added via an identity matmul.  15 of 32 conv units run as
   DVE MAC chains, the rest as PE diag matmuls.
 - C(3) is split into two per-chunk passes so chunk 0 of the stage-3 gate
   chain + epilogue hides under the chunk-1 pass; 3 conv units are held
   back as PE cover for the chunk-1 chain.
 - kq / ksq+qsq(fp8 DR) / vsq(fp8 DR) reductions all deferred by one
   gg/vv so their producers never stall the PE sum matmuls.
 - startup: split vw/emb DMAs ordered first + 4-psum et-outer first vproj
   pass so PE starts at ~4us instead of 13us.
"""

import os
import sys

for _p in ("/opt/trn_rl_repo",):
    if _p not in sys.path:
        sys.path.insert(0, _p)

import numpy as np
import ml_dtypes

import concourse.bass as bass
from concourse import mybir
from concourse.tile import TileContext
from concourse.bass_utils import run_bass_kernel_spmd
import bass_rust

F32 = mybir.dt.float32
F32R = mybir.dt.float32r
F16 = mybir.dt.float16
FP8 = mybir.dt.float8e4
AF = mybir.ActivationFunctionType
ALU = mybir.AluOpType
DR = mybir.MatmulPerfMode.DoubleRow

# Problem constants (hardcoded per spec nn_EngramNew_2070174237244)
B, T, G, C, E = 2, 4096, 4, 1024, 1024
GC = G * C
KT, DIL = 4, 3          # conv taps / dilation
EPS = 1e-5
NORM_EPS = 1e-5
NCORES = 8
NTOK = (B * T) // NCORES    # 1024 tokens per core
HALO = (KT - 1) * DIL       # 9
NET = E // 128              # 8 e-tiles
NGCT = GC // 128            # 32 gc-tiles
NCT = C // 128              # 8 c-tiles
CHW = 512                   # token chunk width (1 PSUM bank of fp32)
NCH = NTOK // CHW           # 2 chunks



class PatchedTileContext(TileContext):
    """This walrus build allows only one sem wait per instruction (two on
    EventSemaphore). Tile attaches as many waits as an instruction needs,
    so after scheduling we hoist excess waits onto no-op instructions
    inserted just before the owner on the same engine (engines are strict
    FIFO, so observing the sems earlier is equivalent)."""

    def _split_excess_waits(self):
        nc = self.nc

        def make_nop(engine):
            bi = nc.engines[engine].nop()
            bb = nc.cur_bb.bb
            lst = list(bb.instructions)
            assert lst[-1] is bi.ins
            bb.instructions = lst[:-1]
            return bi.ins

        # Phase 1: snapshot every block BEFORE creating any nop, so nops
        # appended to cur_bb can never leak into the iteration or the rebuilt
        # lists (cur_bb may be one of the blocks being processed).
        snapshots = []
        for f in nc.m.functions:
            for blk in f.blocks:
                snapshots.append((blk, list(blk.instructions)))

        for blk, insts in snapshots:
            out = []
            changed = False
            for ins in insts:
                si = ins.sync_info
                waits = list(si.on_wait) if (si and si.on_wait) else []
                cap = 2 if isinstance(ins, mybir.InstEventSemaphore) else 1
                if len(waits) > cap:
                    changed = True
                    for w in waits[cap:]:
                        nop = make_nop(ins.engine)
                        nop.sync_info = bass_rust.SyncInfo(
                            on_wait=[w], on_update=[]
                        )
                        out.append(nop)
                    upd = list(si.on_update) if si.on_update else []
                    ins.sync_info = bass_rust.SyncInfo(
                        on_wait=waits[:cap], on_update=upd
                    )
                out.append(ins)
            if changed:
                blk.instructions = out

    def _drain_and_barrier(self, tick_clock, wait_clock):
        super()._drain_and_barrier(tick_clock, wait_clock)
        self._split_excess_waits()


def _r(ap):
    return ap.bitcast(F32R)


def build_program():
    nc = bass.Bass()

    # ---- DRAM parameters ----
    emb16 = nc.declare_dram_parameter("emb16", [E, NTOK], F16, isOutput=False)
    hidT = nc.declare_dram_parameter("hidT", [GC, NTOK], F16, isOutput=False)
    kwT = nc.declare_dram_parameter("kwT", [E, GC], F16, isOutput=False)
    vwT = nc.declare_dram_parameter("vwT", [E, C], F16, isOutput=False)
    keyb = nc.declare_dram_parameter("keyb", [128, NGCT], F32, isOutput=False)
    valb = nc.declare_dram_parameter("valb", [128, NCT], F32, isOutput=False)
    lk8 = nc.declare_dram_parameter("lk8", [128, 9 * 2 * 16], FP8,
                                    isOutput=False)
    lkq = nc.declare_dram_parameter("lkq", [NGCT, 128, 16], F32, isOutput=False)
    aux16 = nc.declare_dram_parameter("aux16", [128, 16 + 256], F16,
                                      isOutput=False)
    ceps = nc.declare_dram_parameter("ceps", [128, 24], F32, isOutput=False)
    dg16 = nc.declare_dram_parameter("dg16", [NGCT, 128, KT * 128], F16,
                                     isOutput=False)
    cwf = nc.declare_dram_parameter("cwf", [128, NGCT * KT], F32,
                                    isOutput=False)
    hc = nc.declare_dram_parameter("hc", [128, NGCT * HALO], F16,
                                   isOutput=False)
    id16 = nc.declare_dram_parameter("id16", [128, 128], F16, isOutput=False)
    out_d = nc.declare_dram_parameter("out", [GC, NTOK], F16, isOutput=True)

    with PatchedTileContext(nc) as tc:
        consts = tc.alloc_tile_pool(name="consts", bufs=1)
        kwpool = tc.alloc_tile_pool(name="kwpool", bufs=2)
        qpool = tc.alloc_tile_pool(name="qpool", bufs=3)
        mmp = tc.alloc_tile_pool(name="mmp", bufs=3, space=bass.MemorySpace.PSUM)
        sump = tc.alloc_tile_pool(name="sump", bufs=1, space=bass.MemorySpace.PSUM)
        epsum = tc.alloc_tile_pool(name="epsum", bufs=3,
                                   space=bass.MemorySpace.PSUM)
        scr = tc.alloc_tile_pool(name="scr", bufs=4)
        kqpool = tc.alloc_tile_pool(name="kqpool", bufs=4)
        rowm = tc.alloc_tile_pool(name="rowm", bufs=1)
        rowsc = tc.alloc_tile_pool(name="rowsc", bufs=9)
        npool = tc.alloc_tile_pool(name="npool", bufs=3)
        vpool = tc.alloc_tile_pool(name="vpool", bufs=4)
        opool = tc.alloc_tile_pool(name="opool", bufs=5)
        dgpool = tc.alloc_tile_pool(name="dgpool", bufs=3)
        cacc = tc.alloc_tile_pool(name="cacc", bufs=3)

        # ---- load order: vw(vv0) first, then emb per-et, then small consts
        vw_t0 = kwpool.tile([128, NET, 256], F16, name="vw_t0", tag="w")
        for eh in range(2):
            nc.sync.dma_start(
                out=vw_t0[:, eh * 4:(eh + 1) * 4, :],
                in_=vwT.rearrange("(et p) c -> p et c", p=128)[
                    :, eh * 4:(eh + 1) * 4, 0:256],
            )
        emb_all = consts.tile([128, NET, NTOK], F16)
        for et in range(NET):
            nc.sync.dma_start(out=emb_all[:, et, :],
                              in_=emb16[et * 128:(et + 1) * 128, :])
        vw_t1 = kwpool.tile([128, NET, 256], F16, name="vw_t1", tag="w")
        nc.sync.dma_start(
            out=vw_t1,
            in_=vwT.rearrange("(et p) c -> p et c", p=128)[:, :, 256:512],
        )
        valb_sb = consts.tile([128, NCT], F32)
        nc.sync.dma_start(out=valb_sb, in_=valb[:, :])
        aux_sb = consts.tile([128, 16 + 256], F16)
        nc.sync.dma_start(out=aux_sb, in_=aux16[:, :])
        ceps_sb = consts.tile([128, 24], F32)
        nc.sync.dma_start(out=ceps_sb, in_=ceps[:, :])
        cepr_sb = consts.tile([128, 24], F32R)
        nc.sync.dma_start(out=cepr_sb, in_=_r(ceps[:, :]))
        keyb_sb = consts.tile([128, NGCT], F32)
        nc.sync.dma_start(out=keyb_sb, in_=keyb[:, :])
        lk8_sb = consts.tile([128, 9, 2, 16], FP8)
        nc.sync.dma_start(out=lk8_sb,
                          in_=lk8.rearrange("p (q i c) -> p q i c", i=2, c=16))
        lkq_sb = consts.tile([128, NGCT, 16], F32R)
        nc.sync.dma_start(out=lkq_sb, in_=_r(lkq.rearrange("n p m -> p n m")))
        cwf_sb = consts.tile([128, NGCT * KT], F32)
        nc.sync.dma_start(out=cwf_sb, in_=cwf[:, :])
        hc_sb = consts.tile([128, NGCT, HALO], F16)
        nc.sync.dma_start(out=hc_sb,
                          in_=hc.rearrange("p (n h) -> p n h", h=HALO))
        id16_sb = consts.tile([128, 128], F16)
        nc.sync.dma_start(out=id16_sb, in_=id16[:, :])
        vproj16 = consts.tile([128, NCT, NTOK], F16)
        bc2_sb = aux_sb[0:1, 16:16 + 128]

        # ---- gate sums: ONE [16, NTOK] psum shared by all stages via
        # disjoint one-hot rows: ak_g = row g, aq_g = 4+g, dot_g = 8+g,
        # sv = 12. Reset once (B's first vsq sum); everything accumulates.
        sums_all = sump.tile([16, NTOK], F32, name="sums_all", tag="sums")
        first_sum = [True] * NCH

        def sum_mm(stage, lhsT, rhs, ch, last=False, perf_mode=None):
            st = first_sum[ch]
            first_sum[ch] = False
            nc.tensor.matmul(
                sums_all[:, ch * CHW:(ch + 1) * CHW],
                lhsT, rhs, start=st, stop=last,
                perf_mode=perf_mode, skip_group_check=True,
            )

        # ---------- stage B: vproj = value_w @ emb + value_b ----------
        # vsq in fp8 (feeds only alpha), DoubleRow-reduced, deferred one vv
        pend_vsq = None

        def flush_vsq(v8):
            for ch in range(NCH):
                cols = slice(ch * CHW, (ch + 1) * CHW)
                sum_mm(3, lk8_sb[:, 8, :, :], v8[:, :, cols], ch,
                       perf_mode=DR)

        for vv in range(NCT // 2):
            if vv == 0:
                vw_t = vw_t0
            elif vv == 1:
                vw_t = vw_t1
            else:
                vw_t = kwpool.tile([128, NET, 256], F16, name="vw_t", tag="w")
                nc.sync.dma_start(
                    out=vw_t,
                    in_=vwT.rearrange("(et p) c -> p et c", p=128)[
                        :, :, vv * 256:(vv + 1) * 256],
                )
            vsq = scr.tile([128, 2, NTOK], FP8, name="vsq8", tag="p8")
            if vv == 0:
                # et-outer across 4 psums so PE rate-matches the emb DMAs
                ps4 = [mmp.tile([128, CHW], F32, name=f"psB0_{i}", tag="mm")
                       for i in range(3)]
                ps4.append(epsum.tile([128, CHW], F32, name="psB0_3",
                                      tag="mm"))
                for et in range(NET):
                    for i in range(4):
                        s2, ch = i // 2, i % 2
                        nc.tensor.matmul(
                            ps4[i],
                            vw_t[:, et, s2 * 128:(s2 + 1) * 128],
                            emb_all[:, et, ch * CHW:(ch + 1) * CHW],
                            start=(et == 0), stop=(et == NET - 1),
                        )
                for i in range(4):
                    s2, ch = i // 2, i % 2
                    ct = vv * 2 + s2
                    cols = slice(ch * CHW, (ch + 1) * CHW)
                    nc.scalar.activation(
                        vproj16[:, ct, cols], ps4[i],
                        AF.Identity, bias=valb_sb[:, ct:ct + 1], scale=1.0,
                    )
                    nc.scalar.activation(
                        vsq[:, s2, cols], ps4[i], AF.Square,
                        bias=valb_sb[:, ct:ct + 1], scale=1.0,
                    )
            else:
                for s2 in range(2):
                    ct = vv * 2 + s2
                    for ch in range(NCH):
                        cols = slice(ch * CHW, (ch + 1) * CHW)
                        ps = mmp.tile([128, CHW], F32, name="psB", tag="mm")
                        for et in range(NET):
                            nc.tensor.matmul(
                                ps,
                                vw_t[:, et, s2 * 128:(s2 + 1) * 128],
                                emb_all[:, et, ch * CHW:(ch + 1) * CHW],
                                start=(et == 0), stop=(et == NET - 1),
                            )
                        nc.scalar.activation(
                            vproj16[:, ct, cols], ps,
                            AF.Identity, bias=valb_sb[:, ct:ct + 1], scale=1.0,
                        )
                        nc.scalar.activation(
                            vsq[:, s2, cols], ps, AF.Square,
                            bias=valb_sb[:, ct:ct + 1], scale=1.0,
                        )
                if pend_vsq is not None:
                    flush_vsq(pend_vsq)
                    pend_vsq = None
            pend_vsq = vsq
        flush_vsq(pend_vsq)

        # ---------- stage C for one group-pair ----------
        def emit_c_kq(stage, gg):
            """k path for double-gct gg (two gc tiles); DR sums deferred."""
            kw_t = kwpool.tile([128, NET, 256], F16, name="kw_t", tag="w")
            nc.sync.dma_start(
                out=kw_t,
                in_=kwT.rearrange("(et p) c -> p et c", p=128)[
                    :, :, gg * 256:(gg + 1) * 256],
            )
            ksqp = scr.tile([128, 2, NTOK], FP8, name="ksqp", tag="p8")
            qsqp = scr.tile([128, 2, NTOK], FP8, name="qsqp", tag="p8")
            kqs = []
            for s2 in range(2):
                gct = gg * 2 + s2
                q_sb = qpool.tile([128, NTOK], F16, name="q_sb", tag="q")
                nc.sync.dma_start(
                    out=q_sb, in_=hidT[gct * 128:(gct + 1) * 128, :]
                )
                kq = kqpool.tile([128, NTOK], F32R, name="kq", tag="kq")
                for ch in range(NCH):
                    ps = mmp.tile([128, CHW], F32, name="psC", tag="mm")
                    for et in range(NET):
                        nc.tensor.matmul(
                            ps,
                            kw_t[:, et, s2 * 128:(s2 + 1) * 128],
                            emb_all[:, et, ch * CHW:(ch + 1) * CHW],
                            start=(et == 0), stop=(et == NET - 1),
                        )
                    cols = slice(ch * CHW, (ch + 1) * CHW)
                    nc.scalar.activation(
                        ksqp[:, s2, cols], ps, AF.Square,
                        bias=keyb_sb[:, gct:gct + 1], scale=1.0,
                    )
                    nc.gpsimd.tensor_mul(qsqp[:, s2, cols], q_sb[:, cols],
                                         q_sb[:, cols])
                    nc.vector.scalar_tensor_tensor(
                        kq[:, cols], ps, keyb_sb[:, gct:gct + 1],
                        q_sb[:, cols], op0=ALU.add, op1=ALU.mult,
                    )
                kqs.append((gct, kq))
            return ksqp, qsqp, kqs

        def emit_dr(stage, ksqp, qsqp, kqs, last_gg):
            for gct, kq in kqs:
                for ch in range(NCH):
                    sum_mm(stage, lkq_sb[:, gct, :],
                           kq[:, ch * CHW:(ch + 1) * CHW], ch)
            for ch in range(NCH):
                cols = slice(ch * CHW, (ch + 1) * CHW)
                sum_mm(stage, lk8_sb[:, stage, :, :], ksqp[:, :, cols], ch,
                       perf_mode=DR)
                sum_mm(stage, lk8_sb[:, 4 + stage, :, :], qsqp[:, :, cols],
                       ch, last=last_gg, perf_mode=DR)

        def emit_c_kq1(stage, gg, ch):
            """Single-chunk variant (window-3 ch-split passes)."""
            cols = slice(ch * CHW, (ch + 1) * CHW)
            kw_t = kwpool.tile([128, NET, 256], F16, name="kw_t", tag="w")
            nc.sync.dma_start(
                out=kw_t,
                in_=kwT.rearrange("(et p) c -> p et c", p=128)[
                    :, :, gg * 256:(gg + 1) * 256],
            )
            ksqp = scr.tile([128, 2, CHW], FP8, name="ksqp1", tag="p8")
            qsqp = scr.tile([128, 2, CHW], FP8, name="qsqp1", tag="p8")
            kqs = []
            for s2 in range(2):
                gct = gg * 2 + s2
                q_sb = qpool.tile([128, CHW], F16, name="q_sb1", tag="q")
                nc.sync.dma_start(
                    out=q_sb, in_=hidT[gct * 128:(gct + 1) * 128, cols]
                )
                kq = kqpool.tile([128, CHW], F32R, name="kq1", tag="kq")
                ps = mmp.tile([128, CHW], F32, name="psC", tag="mm")
                for et in range(NET):
                    nc.tensor.matmul(
                        ps,
                        kw_t[:, et, s2 * 128:(s2 + 1) * 128],
                        emb_all[:, et, cols],
                        start=(et == 0), stop=(et == NET - 1),
                    )
                nc.scalar.activation(
                    ksqp[:, s2, :], ps, AF.Square,
                    bias=keyb_sb[:, gct:gct + 1], scale=1.0,
                )
                nc.gpsimd.tensor_mul(qsqp[:, s2, :], q_sb, q_sb)
                nc.vector.scalar_tensor_tensor(
                    kq, ps, keyb_sb[:, gct:gct + 1],
                    q_sb, op0=ALU.add, op1=ALU.mult,
                )
                kqs.append((gct, kq))
            return ksqp, qsqp, kqs

        def emit_dr1(stage, ksqp, qsqp, kqs, ch, last_gg):
            for gct, kq in kqs:
                sum_mm(stage, lkq_sb[:, gct, :], kq, ch)
            sum_mm(stage, lk8_sb[:, stage, :, :], ksqp, ch, perf_mode=DR)
            sum_mm(stage, lk8_sb[:, 4 + stage, :, :], qsqp, ch, last=last_gg,
                   perf_mode=DR)

        # ---------- stage D ----------
        # Shared rms_v normalizer: rms_v = sqrt(gate^2*mean(vproj^2)+eps)
        # ~= gate*sqrt(mean(vproj^2)+eps) since gate=sigmoid(..)>0, so the
        # conv input normed = vproj*alpha with ONE shared alpha row; only the
        # residual (value = vproj*gate) needs the per-group gate.
        def emit_alpha():
            # sv (= sum vproj^2) sits at psum row 12: bounce the block to
            # SBUF and matmul-extract the row to partition 0.
            s3a = rowm.tile([16, NTOK], F32R, name="s3a", tag="svz")
            aln = rowsc.tile([1, NTOK], F32, name="aln", tag="rs")
            alpha16 = rowm.tile([1, NTOK], F16, name="alpha16", tag="alpha16")
            nc.scalar.activation(s3a, sums_all[:, :], AF.Copy)
            for ch in range(NCH):
                cols = slice(ch * CHW, (ch + 1) * CHW)
                p = epsum.tile([1, CHW], F32, name="svx", tag="mm")
                nc.tensor.matmul(p, cepr_sb[0:16, 20:21], s3a[:, cols],
                                 start=True, stop=True)
                nc.scalar.activation(aln[:, cols], p, AF.Ln,
                                     bias=ceps_sb[0:1, 6:7],
                                     scale=1.0 / float(C))
            nc.scalar.activation(alpha16, aln, AF.Exp, scale=-0.5)
            return alpha16

        def make_d_tiles(stage):
            T = {}
            for nm in ("p4", "lnp", "lnd", "lng", "sqg", "sgn", "ss4", "ab4",
                       "akr"):
                T[nm] = rowsc.tile([1, NTOK], F32, name=f"{nm}{stage}",
                                   tag="rs")
            T["gate16"] = rowm.tile([1, NTOK], F16, name=f"gate16{stage}",
                                    tag="gate16")
            T["s3"] = rowm.tile([16, NTOK], F32R, name=f"s3_{stage}",
                                tag="ext")
            return T

        def emit_d_s3(stage, T, chs=(0, 1)):
            """Psum sums -> partition-0-based SBUF bounce (+ stage biases)."""
            for ch in chs:
                sl = slice(ch * CHW, (ch + 1) * CHW)
                nc.scalar.activation(T["s3"][:, sl], sums_all[:, sl],
                                     AF.Identity,
                                     bias=ceps_sb[0:16, stage:stage + 1],
                                     scale=1.0)

        def emit_d(stage, T, mul_eng=None, chs=(0, 1)):
            """Per-group gate chain: gate = sigmoid(sign(dot)*sqrt(|graw|)).

            Engines only address partitions at 32-boundaries, so the psum
            region is Act-copied (aligned base -> partition 0) to s3, and
            rows 1+ are pulled to partition-0 psum via one-hot matmuls.
            Row layout: stages 0-2: [ak, aq, dot]; stage 3: [sv, aq, dot, ak].
            """
            me = mul_eng if mul_eng is not None else nc.vector
            s3 = T["s3"]
            p4, lnp, lnd, lng, sqg, sgn, ss4, ab4, gate16 = (
                T["p4"], T["lnp"], T["lnd"], T["lng"], T["sqg"], T["sgn"],
                T["ss4"], T["ab4"], T["gate16"])
            akr = T["akr"]
            if chs == (0, 1):
                sls = [slice(0, NTOK)]
            else:
                sls = [slice(ch * CHW, (ch + 1) * CHW) for ch in chs]

            def extract(row, ch):
                sel = cepr_sb[0:16, 8 + row:9 + row]
                p = epsum.tile([1, CHW], F32, name=f"x{row}_{stage}",
                               tag="mm")
                nc.tensor.matmul(p, sel,
                                 s3[:, ch * CHW:(ch + 1) * CHW],
                                 start=True, stop=True)
                return p

            # first layer reads the [1, CHW] psums (partition 0), per chunk
            for ch in chs:
                cols = slice(ch * CHW, (ch + 1) * CHW)
                ak_ps = extract(stage, ch)
                aq_ps = extract(4 + stage, ch)
                dot_ps = extract(8 + stage, ch)
                nc.scalar.activation(akr[:, cols], ak_ps, AF.Copy)
                nc.scalar.activation(ab4[:, cols], dot_ps, AF.Square)
                nc.scalar.activation(sgn[:, cols], dot_ps, AF.Sign)
                nc.vector.tensor_mul(p4[:, cols], akr[:, cols], aq_ps)
            # 2ln|dot| and ln(p4/C); 2ln|graw| = 2ln|dot| - ln(p4/C)
            # (plain subtract so the mul engine can be Pool)
            for sl in sls:
                nc.scalar.activation(lnd[:, sl], ab4[:, sl], AF.Ln,
                                     bias=ceps_sb[0:1, 7:8])
            for sl in sls:
                nc.scalar.activation(lnp[:, sl], p4[:, sl], AF.Ln,
                                     scale=1.0 / float(C))
            for sl in sls:
                me.tensor_sub(lng[:, sl], lnd[:, sl], lnp[:, sl])
            for sl in sls:
                nc.scalar.activation(sqg[:, sl], lng[:, sl], AF.Exp,
                                     scale=0.25)
            for sl in sls:
                me.tensor_mul(ss4[:, sl], sqg[:, sl], sgn[:, sl])
            for sl in sls:
                nc.scalar.activation(gate16[:, sl], ss4[:, sl], AF.Sigmoid)
            return gate16

        # ---------- stage E ----------
        def bcast_ch(src, dst, ch):
            bp = epsum.tile([128, CHW], F32, name="bp", tag="mm")
            nc.tensor.matmul(
                bp, bc2_sb[0:1, 0:128],
                src[:, ch * CHW:(ch + 1) * CHW],
                start=True, stop=True,
            )
            nc.scalar.activation(
                dst[:, ch * CHW:(ch + 1) * CHW], bp, AF.Copy)

        def bcast_row(src, tag):
            """[1, NTOK] f32/f16 row -> [128, NTOK] f16 via PE broadcast."""
            dst = rowm.tile([128, NTOK], F16, name=f"b_{tag}", tag=tag)
            for ch in range(NCH):
                bcast_ch(src, dst, ch)
            return dst

        # nx16[ct]: f16 conv input, shared by all 4 groups' units:
        # [9 zeros | vproj*alpha]; the halo contribution to the first 9
        # outputs is a host-computed f16 correction (hc) accumulated via an
        # identity matmul.
        PADW = HALO + NTOK
        nx8s = {}

        def emit_nx8(ct):
            nx8 = npool.tile([128, PADW], F16, name=f"nx16_{ct}",
                             tag=f"nx16_{ct}", bufs=1)
            nc.gpsimd.memset(nx8[:, 0:HALO], 0.0)
            nc.vector.tensor_mul(nx8[:, HALO:HALO + NTOK],
                                 vproj16[:, ct, :], ab16)
            nx8s[ct] = nx8

        def emit_val(gct, gb16, on_pool=False):
            ct = gct % NCT
            val = vpool.tile([128, NTOK], F16, name="val", tag="val")
            if on_pool:
                nc.gpsimd.tensor_mul(val, vproj16[:, ct, :], gb16)
            else:
                nc.vector.tensor_mul(val, vproj16[:, ct, :], gb16)
            return val

        def emit_e_conv_pe(gct, pools=None):
            """f16 conv taps + halo-fix matmul."""
            ct = gct % NCT
            nx8 = nx8s[ct]
            dg_t = dgpool.tile([128, KT * 128], F16, name="dg_t", tag="dg")
            nc.sync.dma_start(out=dg_t, in_=dg16[gct])
            accs = []
            for ch in range(NCH):
                pool = (pools[ch % len(pools)] if pools else epsum)
                acc = pool.tile([128, CHW], F32, name="acc", tag="mm")
                for k in range(KT):
                    base = ch * CHW + k * DIL
                    nc.tensor.matmul(
                        acc,
                        dg_t[:, k * 128:(k + 1) * 128],
                        nx8[:, base:base + CHW],
                        start=(k == 0), stop=(k == KT - 1 and ch == 1),
                        skip_group_check=True,
                    )
                if ch == 0:
                    nc.tensor.matmul(
                        acc[:, 0:HALO], id16_sb, hc_sb[:, gct, :],
                        start=False, stop=True, skip_group_check=True,
                    )
                accs.append(acc)
            return accs

        def emit_e_conv_dve(gct):
            """f16 conv as DVE scalar-ptr MAC chains (+ in-place halo fix)."""
            ct = gct % NCT
            nx8 = nx8s[ct]
            outs = []
            for ch in range(NCH):
                prev = None
                for k in range(KT):
                    win = nx8[:, ch * CHW + k * DIL:ch * CHW + k * DIL + CHW]
                    a = cacc.tile([128, CHW], F16, name=f"ca{k}", tag=f"ca{k}")
                    wcol = cwf_sb[:, gct * KT + k:gct * KT + k + 1]
                    if k == 0:
                        nc.vector.tensor_scalar_mul(a, win, wcol)
                    else:
                        nc.vector.scalar_tensor_tensor(
                            a, win, wcol, prev, op0=ALU.mult, op1=ALU.add)
                    prev = a
                if ch == 0:
                    nc.vector.tensor_tensor(prev[:, 0:HALO], prev[:, 0:HALO],
                                            hc_sb[:, gct, :], op=ALU.add)
                outs.append(prev)
            return outs

        def emit_silu(accs):
            sacc = opool.tile([128, NTOK], F16, name="sacc", tag="sacc")
            for ch in range(NCH):
                nc.scalar.activation(sacc[:, ch * CHW:(ch + 1) * CHW],
                                     accs[ch], AF.Silu)
            return sacc

        def emit_resid_out(gct, val, sacc, engine="pool"):
            ot = opool.tile([128, NTOK], F16, name="ot", tag="ot")
            if engine == "dve":
                nc.vector.tensor_tensor(ot, val, sacc, op=ALU.add)
            else:
                nc.gpsimd.tensor_add(ot, val, sacc)
            nc.sync.dma_start(out=out_d[gct * 128:(gct + 1) * 128, :], in_=ot)

        # ---------- pipeline ----------
        # conv+silu only needs the shared ab16; val/resid needs gate(g).
        # Window g: C(g) + chain(g-1) + full units of group g-1 + a few
        # group-3 conv units pulled early; tail: 3 conv units cover chain(3),
        # then group-3 val/resid.
        sacc3 = {}      # gct -> long-lived sacc for group-3 units
        ab16 = None
        TAIL3 = [29, 30, 31]
        EARLY3 = {0: [24, 25, 26], 1: [27], 2: [28], 3: []}

        def conv_unit(u, long_lived=False, pools=None, defer_silu=False,
                      dve=False):
            if dve:
                accs = emit_e_conv_dve(u)
            else:
                accs = emit_e_conv_pe(u, pools=pools)
            if defer_silu:
                return accs
            if long_lived:
                sacc = opool.tile([128, NTOK], F16, name=f"sacc{u}",
                                  tag=f"sacc3_{u}", bufs=1)
            else:
                sacc = opool.tile([128, NTOK], F16, name=f"sacc{u}",
                                  tag="sacc")
            for ch in range(NCH):
                nc.scalar.activation(sacc[:, ch * CHW:(ch + 1) * CHW],
                                     accs[ch], AF.Silu)
            if long_lived:
                sacc3[u] = sacc
            return sacc

        def full_unit(u, gb16, dve=False):
            sacc = conv_unit(u, dve=dve)
            val = emit_val(u, gb16)
            emit_resid_out(u, val, sacc, engine="pool" if dve else "dve")

        gate_prev = None
        d_tiles = {}
        for g in range(3):
            dr_prev = None
            gb16 = None
            units = list(range((g - 1) * 8, g * 8)) if g else []
            for i, gg in enumerate(range(g * 4, (g + 1) * 4)):
                if i == 0 and g:
                    d_tiles[g - 1] = make_d_tiles(g - 1)
                    emit_d_s3(g - 1, d_tiles[g - 1])
                cur = emit_c_kq(g, gg)
                if i == 0:
                    if g == 0:
                        alpha16 = emit_alpha()
                    else:
                        gate_prev = emit_d(g - 1, d_tiles[g - 1],
                                           mul_eng=nc.gpsimd)
                if dr_prev is not None:
                    emit_dr(g, *dr_prev, last_gg=False)
                dr_prev = cur
                if i == 1:
                    if g == 0:
                        ab16 = bcast_row(alpha16, "ab16")
                        for ct in (0, 1, 2):
                            emit_nx8(ct)
                    elif g == 1:
                        for ct in (6, 7):
                            emit_nx8(ct)
                    batch = []
                elif i == 2:
                    if g:
                        gb16 = bcast_row(gate_prev, f"gb{g - 1}")
                        batch = units[0:3]
                    else:
                        emit_nx8(3)
                        batch = EARLY3[0][0:2]
                elif i == 3:
                    if g == 0:
                        emit_nx8(4)
                        emit_nx8(5)
                    batch = units[3:6] if g else EARLY3[0][2:3]
                else:
                    batch = []
                for u in batch:
                    if g:
                        full_unit(u, gb16, dve=(u % 8 in (0, 2, 4)))
                    else:
                        conv_unit(u, long_lived=True, dve=True)
            emit_dr(g, *dr_prev, last_gg=True)
            if g:
                for u in units[6:8]:
                    full_unit(u, gb16, dve=(u % 8 == 6))
                for u in EARLY3[g]:
                    conv_unit(u, long_lived=True)

        # ---------- window 3: chunk-split passes ----------
        # pass p computes C(3) for token chunk p only, so the stage-3 gate
        # chain + group-3 epilogue for chunk 0 hide under pass 1.
        units = list(range(16, 24))
        d_tiles[2] = make_d_tiles(2)
        emit_d_s3(2, d_tiles[2])
        T3 = None
        gb3 = rowm.tile([128, NTOK], F16, name="b_gb3", tag="gb3")

        def epi3_ch(u, ch):
            ct = u % NCT
            cols = slice(ch * CHW, (ch + 1) * CHW)
            val = vpool.tile([128, CHW], F16, name="val3", tag="val")
            nc.vector.tensor_mul(val, vproj16[:, ct, cols], gb3[:, cols])
            ot = opool.tile([128, CHW], F16, name="ot3", tag="ot")
            nc.vector.tensor_tensor(ot, val, sacc3[u][:, cols], op=ALU.add)
            nc.sync.dma_start(out=out_d[u * 128:(u + 1) * 128, cols], in_=ot)

        for p in range(2):
            dr_prev = None
            for i, gg in enumerate(range(12, 16)):
                cur = emit_c_kq1(3, gg, p)
                if p == 0 and i == 0:
                    gate2 = emit_d(2, d_tiles[2], mul_eng=nc.gpsimd)
                if p == 1 and i == 0:
                    T3 = make_d_tiles(3)
                    emit_d_s3(3, T3, chs=(0,))
                    gate3 = emit_d(3, T3, mul_eng=nc.vector, chs=(0,))
                if dr_prev is not None:
                    emit_dr1(3, *dr_prev, p, last_gg=False)
                dr_prev = cur
                if p == 0:
                    if i == 2:
                        gb2 = bcast_row(gate2, "gb2")
                        batch = units[0:3]
                    elif i == 3:
                        batch = units[3:6]
                    else:
                        batch = []
                    for u in batch:
                        full_unit(u, gb2, dve=(u % 8 in (0, 2, 4)))
                else:
                    if i == 1:
                        bcast_ch(gate3, gb3, 0)
                        for u in units[6:8]:
                            full_unit(u, gb2, dve=(u % 8 == 6))
                    elif i == 2:
                        for u in range(24, 28):
                            epi3_ch(u, 0)
                    elif i == 3:
                        epi3_ch(28, 0)
            emit_dr1(3, *dr_prev, p, last_gg=True)

        # ---------- tail: chunk 1 of the group-3 gate + epilogue ----------
        # TAIL3 conv matmuls cover the chain; their silus follow its Act ops
        emit_d_s3(3, T3, chs=(1,))
        acc_pools = [epsum, mmp]
        emit_d(3, T3, mul_eng=nc.vector, chs=(1,))
        tail_accs = [conv_unit(u, pools=acc_pools, defer_silu=True)
                     for u in TAIL3]
        bcast_ch(gate3, gb3, 1)
        for j, u in enumerate(TAIL3):
            sacc = opool.tile([128, NTOK], F16, name=f"sacc{u}",
                              tag=f"sacc3_{u}", bufs=1)
            for ch in range(NCH):
                nc.scalar.activation(sacc[:, ch * CHW:(ch + 1) * CHW],
                                     tail_accs[j][ch], AF.Silu)
            sacc3[u] = sacc
        for u in TAIL3:
            epi3_ch(u, 0)
        for u in range(24, 32):
            epi3_ch(u, 1)

        for p in (cacc, dgpool, opool, vpool, npool, rowsc, rowm, kqpool, scr,
                  epsum, sump, mmp, qpool, kwpool, consts):
            p.release()
    return nc


def host_prep(embeddings, hidden_states, key_w, key_b, value_w, value_b,
              w_key_norm, w_query_norm, w_norm, conv_weight):
    """Build the per-core input maps."""
    f32, f16 = np.float32, np.float16
    e4 = ml_dtypes.float8_e4m3fn
    embeddings = np.asarray(embeddings, f32)
    hidden_states = np.asarray(hidden_states, f32)
    key_w = np.asarray(key_w, f32)
    key_b = np.asarray(key_b, f32)
    value_w = np.asarray(value_w, f32)
    value_b = np.asarray(value_b, f32)
    w_key_norm = np.asarray(w_key_norm, f32)
    w_query_norm = np.asarray(w_query_norm, f32)
    w_norm = np.asarray(w_norm, f32)
    conv_weight = np.asarray(conv_weight, f32)

    kwT = np.ascontiguousarray(key_w.T).astype(f16)        # [E, GC]
    vwT = np.ascontiguousarray(value_w.T).astype(f16)      # [E, C]
    keyb_r = np.ascontiguousarray(key_b.reshape(NGCT, 128).T)  # [128, NGCT]
    valb_r = np.ascontiguousarray(value_b.reshape(NCT, 128).T)
    wkq = (w_key_norm * w_query_norm).reshape(GC)

    # one-hot lhsT tables. ONE shared [16, NTOK] psum accumulator with
    # disjoint rows: ak_g = row g, aq_g = 4+g, dot_g = 8+g, sv = 12.
    # (engines can only address 32-aligned partition bases, so rows are
    #  matmul-extracted after an Act bounce of the block to partition 0)
    lk8 = np.zeros((128, 9, 2, 16), f32)
    for g in range(G):
        lk8[:, g, :, g] = 1.0          # ksq -> row g
        lk8[:, 4 + g, :, 4 + g] = 1.0  # qsq -> row 4+g
    lk8[:, 8, :, 12] = 1.0             # vsq -> row 12 (sv)
    lk8 = lk8.reshape(128, 288).astype(e4)

    lkq = np.zeros((NGCT, 128, 16), f32)
    for gct in range(NGCT):
        g = gct // NCT
        lkq[gct, :, 8 + g] = wkq[gct * 128:(gct + 1) * 128]

    aux16 = np.zeros((128, 16 + 256), f16)
    aux16[:, 12] = 1.0        # lv one-hot: vsq -> row 12 (sv)
    for j in range(2):
        aux16[j, 16 + j * 128:16 + (j + 1) * 128] = 1.0

    # ceps: cols 0-3 = per-stage bias vectors (+C*EPS on ak/aq rows);
    #        cols 8+r = f32 one-hot row selectors (identity)
    ceps_h = np.zeros((128, 24), f32)
    for g in range(G):
        ceps_h[g, g] = float(C) * EPS
        ceps_h[4 + g, g] = float(C) * EPS
    for r in range(16):
        ceps_h[r, 8 + r] = 1.0
    ceps_h[0, 6] = NORM_EPS
    ceps_h[0, 7] = 1e-60

    # f16 diagonal conv weights + identity for the halo-fix matmul.
    cwf = (conv_weight.reshape(G, C, KT) * w_norm[:, :, None]).astype(f32)
    dg = np.zeros((NGCT, 128, KT * 128), f16)
    idx = np.arange(128)
    for gct in range(NGCT):
        g, ct = gct // NCT, gct % NCT
        for k in range(KT):
            dg[gct, idx, k * 128 + idx] = cwf[g, ct * 128 + idx, k].astype(f16)
    id16_h = np.zeros((128, 128), f16)
    id16_h[idx, idx] = 1.0
    cwf_r = np.zeros((128, NGCT * KT), f32)
    for gct in range(NGCT):
        g, ct = gct // NCT, gct % NCT
        for k in range(KT):
            cwf_r[:, gct * KT + k] = cwf[g, ct * 128:(ct + 1) * 128, k]

    in_maps = []
    for core in range(NCORES):
        b = core // (NCORES // B)
        t0 = (core % (NCORES // B)) * NTOK
        emb_s = embeddings[b, t0:t0 + NTOK]                # [NTOK, E]
        hid_s = hidden_states[b, t0:t0 + NTOK].reshape(NTOK, GC)
        emb_c = np.ascontiguousarray(emb_s.T).astype(f16)  # [E, NTOK]
        hid_c = np.ascontiguousarray(hid_s.T).astype(f16)  # [GC, NTOK]

        # halo: nhat (= value / rms_v, w_norm NOT applied) for the 9
        # preceding tokens feeds a host-computed conv correction hc for the
        # first 9 output tokens; zeros at the sequence start.
        if t0 == 0:
            hc_c = np.zeros((128, NGCT * HALO), f16)
        else:
            th = slice(t0 - HALO, t0)
            e9 = embeddings[b, th]                          # [9, E]
            k9 = (e9 @ key_w.T + key_b).reshape(HALO, G, C)
            q9 = hidden_states[b, th]                       # [9, G, C]
            rk = np.sqrt((k9 * k9).mean(-1) + EPS)
            rq = np.sqrt((q9 * q9).mean(-1) + EPS)
            d9 = np.einsum("tgc,gc,tgc,gc->tg", k9, w_key_norm, q9,
                           w_query_norm)
            graw = d9 / (rk * rq) / np.sqrt(f32(C))
            g9 = 1.0 / (1.0 + np.exp(-(np.where(graw >= 0, 1.0, -1.0)
                                       * np.sqrt(np.maximum(np.abs(graw),
                                                            1e-6)))))
            vp9 = e9 @ value_w.T + value_b                  # [9, C]
            val9 = vp9[:, None, :] * g9[..., None].astype(f32)
            rv9 = np.sqrt((val9 * val9).mean(-1) + NORM_EPS)
            nhat9 = val9 / rv9[..., None]                   # [9, G, C]
            # hc[c, gct, t] = sum_{k: t+k*DIL<9} cwf[g,c,k]*nhat9[t+k*DIL,g,c]
            hcf = np.zeros((HALO, G, C), f32)
            for t in range(HALO):
                for k in range(KT):
                    ix = t + k * DIL
                    if ix < HALO:
                        hcf[t] += cwf[:, :, k] * nhat9[ix]
            hg = hcf.transpose(1, 2, 0).reshape(NGCT, 128, HALO)
            hc_c = np.ascontiguousarray(
                hg.transpose(1, 0, 2).reshape(128, NGCT * HALO)).astype(f16)

        in_maps.append({
            "emb16": emb_c, "hidT": hid_c, "kwT": kwT, "vwT": vwT,
            "keyb": keyb_r, "valb": valb_r,
            "lk8": lk8, "lkq": lkq, "aux16": aux16, "ceps": ceps_h,
            "dg16": dg, "cwf": cwf_r, "hc": hc_c, "id16": id16_h,
        })
    return in_maps


_NC_CACHE = [None]
LAST_RESULT = [None]


def kernel(**inputs) -> np.ndarray:
    in_maps = host_prep(**inputs)
    if _NC_CACHE[0] is None:
        _NC_CACHE[0] = build_program()
    nc = _NC_CACHE[0]
    res = run_bass_kernel_spmd(nc, in_maps, list(range(NCORES)))
    LAST_RESULT[0] = res
    out = np.empty((B, T, G, C), np.float32)
    for core in range(NCORES):
        b = core // (NCORES // B)
        t0 = (core % (NCORES // B)) * NTOK
        oc = np.asarray(res.results[core]["out"]).astype(np.float32)
        out[b, t0:t0 + NTOK] = oc.reshape(G, C, NTOK).transpose(2, 0, 1)
    return out


# revision 118
# speedup vs baseline: 1.0076x; 1.0002x over previous
"""Trainium2 Bass kernel for the EngramNew module (dense_cnn), v3.

Sharding: B*T = 8192 tokens split across 8 cores (1024 tokens each); the conv
halo of (K-1)*DIL = 9 tokens is precomputed host-side.  On-device layout is
channels-on-partitions / tokens-on-free: [G*C, T_core].

v10 design vs the v2 baseline (291.8us -> 225.1us):
 - shared rms_v normalizer: rms_v = sqrt(gate^2*mean(vproj^2)+eps)
   ~= gate*sqrt(mean(vproj^2)+eps) since gate = sigmoid(..) > 0, so the
   conv input (normed) = vproj*alpha with ONE shared alpha row; only the
   residual needs the per-group gate.  This decouples the whole conv
   pipeline from the gate chains (error <= ~1e-3, verified vs reference).
 - ONE [16,NTOK] PSUM accumulator shared by all four gate-sum stages via
   disjoint one-hot rows (ak_g=g, aq_g=4+g, dot_g=8+g, sv=12), reset once:
   no PSUM bank-rotation WAR stalls.  Rows are bounced to partition 0 by
   an Act copy + f32r one-hot extraction matmuls (engines can only address
   32-aligned partition bases).
 - per-ct conv input tiles (zero prefix + vproj*alpha) shared by all 4
   groups; the halo contribution to the first 9 outputs is a host-computed
   correction added via an identity matmul.  15 of 32 conv units run as
   DVE MAC chains, the rest as PE diag matmuls.
 - C(3) is split into two per-chunk passes so chunk 0 of the stage-3 gate
   chain + epilogue hides under the chunk-1 pass; 3 conv units are held
   back as PE cover for the chunk-1 chain.
 - kq / ksq+qsq(fp8 DR) / vsq(fp8 DR) reductions all deferred by one
   gg/vv so their producers never stall the PE sum matmuls.
 - startup: split vw/emb DMAs ordered first + 4-psum et-outer first vproj
   pass so PE starts at ~4us instead of 13us.
"""

import os
import sys

for _p in ("/opt/trn_rl_repo",):
    if _p not in sys.path:
        sys.path.insert(0, _p)

import numpy as np
import ml_dtypes

import concourse.bass as bass
from concourse import mybir
from concourse.tile import TileContext
from concourse.bass_utils import run_bass_kernel_spmd
import bass_rust

F32 = mybir.dt.float32
F32R = mybir.dt.float32r
F16 = mybir.dt.float16
FP8 = mybir.dt.float8e4
AF = mybir.ActivationFunctionType
ALU = mybir.AluOpType
DR = mybir.MatmulPerfMode.DoubleRow

# Problem constants (hardcoded per spec nn_EngramNew_2070174237244)
B, T, G, C, E = 2, 4096, 4, 1024, 1024
GC = G * C
KT, DIL = 4, 3          # conv taps / dilation
EPS = 1e-5
NORM_EPS = 1e-5
NCORES = 8
NTOK = (B * T) // NCORES    # 1024 tokens per core
HALO = (KT - 1) * DIL       # 9
NET = E // 128              # 8 e-tiles
NGCT = GC // 128            # 32 gc-tiles
NCT = C // 128              # 8 c-tiles
CHW = 512                   # token chunk width (1 PSUM bank of fp32)
NCH = NTOK // CHW           # 2 chunks



class PatchedTileContext(TileContext):
    """This walrus build allows only one sem wait per instruction (two on
    EventSemaphore). Tile attaches as many waits as an instruction needs,
    so after scheduling we hoist excess waits onto no-op instructions
    inserted just before the owner on the same engine (engines are strict
    FIFO, so observing the sems earlier is equivalent)."""

    def _split_excess_waits(self):
        nc = self.nc

        def make_nop(engine):
            bi = nc.engines[engine].nop()
            bb = nc.cur_bb.bb
            lst = list(bb.instructions)
            assert lst[-1] is bi.ins
            bb.instructions = lst[:-1]
            return bi.ins

        # Phase 1: snapshot every block BEFORE creating any nop, so nops
        # appended to cur_bb can never leak into the iteration or the rebuilt
        # lists (cur_bb may be one of the blocks being processed).
        snapshots = []
        for f in nc.m.functions:
            for blk in f.blocks:
                snapshots.append((blk, list(blk.instructions)))

        for blk, insts in snapshots:
            out = []
            changed = False
            for ins in insts:
                si = ins.sync_info
                waits = list(si.on_wait) if (si and si.on_wait) else []
                cap = 2 if isinstance(ins, mybir.InstEventSemaphore) else 1
                if len(waits) > cap:
                    changed = True
                    for w in waits[cap:]:
                        nop = make_nop(ins.engine)
                        nop.sync_info = bass_rust.SyncInfo(
                            on_wait=[w], on_update=[]
                        )
                        out.append(nop)
                    upd = list(si.on_update) if si.on_update else []
                    ins.sync_info = bass_rust.SyncInfo(
                        on_wait=waits[:cap], on_update=upd
                    )
                out.append(ins)
            if changed:
                blk.instructions = out

    def _drain_and_barrier(self, tick_clock, wait_clock):
        super()._drain_and_barrier(tick_clock, wait_clock)
        self._split_excess_waits()


def _r(ap):
    return ap.bitcast(F32R)


def build_program():
    nc = bass.Bass()

    # ---- DRAM parameters ----
    emb16 = nc.declare_dram_parameter("emb16", [E, NTOK], F16, isOutput=False)
    hidT = nc.declare_dram_parameter("hidT", [GC, NTOK], F16, isOutput=False)
    kwT = nc.declare_dram_parameter("kwT", [E, GC], F16, isOutput=False)
    vwT = nc.declare_dram_parameter("vwT", [E, C], F16, isOutput=False)
    keyb = nc.declare_dram_parameter("keyb", [128, NGCT], F32, isOutput=False)
    valb = nc.declare_dram_parameter("valb", [128, NCT], F32, isOutput=False)
    lk8 = nc.declare_dram_parameter("lk8", [128, 9 * 2 * 16], FP8,
                                    isOutput=False)
    lkq = nc.declare_dram_parameter("lkq", [NGCT, 128, 16], F32, isOutput=False)
    aux16 = nc.declare_dram_parameter("aux16", [128, 16 + 256], F16,
                                      isOutput=False)
    ceps = nc.declare_dram_parameter("ceps", [128, 24], F32, isOutput=False)
    dg16 = nc.declare_dram_parameter("dg16", [NGCT, 128, KT * 128], F16,
                                     isOutput=False)
    cwf = nc.declare_dram_parameter("cwf", [128, NGCT * KT], F32,
                                    isOutput=False)
    hc = nc.declare_dram_parameter("hc", [128, NGCT * HALO], F16,
                                   isOutput=False)
    id16 = nc.declare_dram_parameter("id16", [128, 128], F16, isOutput=False)
    out_d = nc.declare_dram_parameter("out", [GC, NTOK], F16, isOutput=True)

    with PatchedTileContext(nc) as tc:
        consts = tc.alloc_tile_pool(name="consts", bufs=1)
        kwpool = tc.alloc_tile_pool(name="kwpool", bufs=2)
        qpool = tc.alloc_tile_pool(name="qpool", bufs=3)
        mmp = tc.alloc_tile_pool(name="mmp", bufs=3, space=bass.MemorySpace.PSUM)
        sump = tc.alloc_tile_pool(name="sump", bufs=1, space=bass.MemorySpace.PSUM)
        epsum = tc.alloc_tile_pool(name="epsum", bufs=3,
                                   space=bass.MemorySpace.PSUM)
        scr = tc.alloc_tile_pool(name="scr", bufs=4)
        kqpool = tc.alloc_tile_pool(name="kqpool", bufs=4)
        rowm = tc.alloc_tile_pool(name="rowm", bufs=1)
        rowsc = tc.alloc_tile_pool(name="rowsc", bufs=9)
        npool = tc.alloc_tile_pool(name="npool", bufs=3)
        vpool = tc.alloc_tile_pool(name="vpool", bufs=4)
        opool = tc.alloc_tile_pool(name="opool", bufs=5)
        dgpool = tc.alloc_tile_pool(name="dgpool", bufs=3)
        cacc = tc.alloc_tile_pool(name="cacc", bufs=3)

        # ---- load order: vw(vv0) first, then emb per-et, then small consts
        vw_t0 = kwpool.tile([128, NET, 256], F16, name="vw_t0", tag="w")
        for eh in range(2):
            nc.sync.dma_start(
                out=vw_t0[:, eh * 4:(eh + 1) * 4, :],
                in_=vwT.rearrange("(et p) c -> p et c", p=128)[
                    :, eh * 4:(eh + 1) * 4, 0:256],
            )
        emb_all = consts.tile([128, NET, NTOK], F16)
        for et in range(NET):
            nc.sync.dma_start(out=emb_all[:, et, :],
                              in_=emb16[et * 128:(et + 1) * 128, :])
        vw_t1 = kwpool.tile([128, NET, 256], F16, name="vw_t1", tag="w")
        nc.sync.dma_start(
            out=vw_t1,
            in_=vwT.rearrange("(et p) c -> p et c", p=128)[:, :, 256:512],
        )
        valb_sb = consts.tile([128, NCT], F32)
        nc.sync.dma_start(out=valb_sb, in_=valb[:, :])
        aux_sb = consts.tile([128, 16 + 256], F16)
        nc.sync.dma_start(out=aux_sb, in_=aux16[:, :])
        ceps_sb = consts.tile([128, 24], F32)
        nc.sync.dma_start(out=ceps_sb, in_=ceps[:, :])
        cepr_sb = consts.tile([128, 24], F32R)
        nc.sync.dma_start(out=cepr_sb, in_=_r(ceps[:, :]))
        keyb_sb = consts.tile([128, NGCT], F32)
        nc.sync.dma_start(out=keyb_sb, in_=keyb[:, :])
        lk8_sb = consts.tile([128, 9, 2, 16], FP8)
        nc.sync.dma_start(out=lk8_sb,
                          in_=lk8.rearrange("p (q i c) -> p q i c", i=2, c=16))
        lkq_sb = consts.tile([128, NGCT, 16], F32R)
        nc.sync.dma_start(out=lkq_sb, in_=_r(lkq.rearrange("n p m -> p n m")))
        cwf_sb = consts.tile([128, NGCT * KT], F32)
        nc.sync.dma_start(out=cwf_sb, in_=cwf[:, :])
        hc_sb = consts.tile([128, NGCT, HALO], F16)
        nc.sync.dma_start(out=hc_sb,
                          in_=hc.rearrange("p (n h) -> p n h", h=HALO))
        id16_sb = consts.tile([128, 128], F16)
        nc.sync.dma_start(out=id16_sb, in_=id16[:, :])
        vproj16 = consts.tile([128, NCT, NTOK], F16)
        bc2_sb = aux_sb[0:1, 16:16 + 128]

        # ---- gate sums: ONE [16, NTOK] psum shared by all stages via
        # disjoint one-hot rows: ak_g = row g, aq_g = 4+g, dot_g = 8+g,
        # sv = 12. Reset once (B's first vsq sum); everything accumulates.
        sums_all = sump.tile([16, NTOK], F32, name="sums_all", tag="sums")
        first_sum = [True] * NCH

        def sum_mm(stage, lhsT, rhs, ch, last=False, perf_mode=None):
            st = first_sum[ch]
            first_sum[ch] = False
            nc.tensor.matmul(
                sums_all[:, ch * CHW:(ch + 1) * CHW],
                lhsT, rhs, start=st, stop=last,
                perf_mode=perf_mode, skip_group_check=True,
            )

        # ---------- stage B: vproj = value_w @ emb + value_b ----------
        # vsq in fp8 (feeds only alpha), DoubleRow-reduced, deferred one vv
        pend_vsq = None

        def flush_vsq(v8):
            for ch in range(NCH):
                cols = slice(ch * CHW, (ch + 1) * CHW)
                sum_mm(3, lk8_sb[:, 8, :, :], v8[:, :, cols], ch,
                       perf_mode=DR)

        for vv in range(NCT // 2):
            if vv == 0:
                vw_t = vw_t0
            elif vv == 1:
                vw_t = vw_t1
            else:
                vw_t = kwpool.tile([128, NET, 256], F16, name="vw_t", tag="w")
                nc.sync.dma_start(
                    out=vw_t,
                    in_=vwT.rearrange("(et p) c -> p et c", p=128)[
                        :, :, vv * 256:(vv + 1) * 256],
                )
            vsq = scr.tile([128, 2, NTOK], FP8, name="vsq8", tag="p8")
            if vv == 0:
                # et-outer across 4 psums so PE rate-matches the emb DMAs
                ps4 = [mmp.tile([128, CHW], F32, name=f"psB0_{i}", tag="mm")
                       for i in range(3)]
                ps4.append(epsum.tile([128, CHW], F32, name="psB0_3",
                                      tag="mm"))
                for et in range(NET):
                    for i in range(4):
                        s2, ch = i // 2, i % 2
                        nc.tensor.matmul(
                            ps4[i],
                            vw_t[:, et, s2 * 128:(s2 + 1) * 128],
                            emb_all[:, et, ch * CHW:(ch + 1) * CHW],
                            start=(et == 0), stop=(et == NET - 1),
                        )
                for i in range(4):
                    s2, ch = i // 2, i % 2
                    ct = vv * 2 + s2
                    cols = slice(ch * CHW, (ch + 1) * CHW)
                    nc.scalar.activation(
                        vproj16[:, ct, cols], ps4[i],
                        AF.Identity, bias=valb_sb[:, ct:ct + 1], scale=1.0,
                    )
                    nc.scalar.activation(
                        vsq[:, s2, cols], ps4[i], AF.Square,
                        bias=valb_sb[:, ct:ct + 1], scale=1.0,
                    )
            else:
                for s2 in range(2):
                    ct = vv * 2 + s2
                    for ch in range(NCH):
                        cols = slice(ch * CHW, (ch + 1) * CHW)
                        ps = mmp.tile([128, CHW], F32, name="psB", tag="mm")
                        for et in range(NET):
                            nc.tensor.matmul(
                                ps,
                                vw_t[:, et, s2 * 128:(s2 + 1) * 128],
                                emb_all[:, et, ch * CHW:(ch + 1) * CHW],
                                start=(et == 0), stop=(et == NET - 1),
                            )
                        nc.scalar.activation(
                            vproj16[:, ct, cols], ps,
                            AF.Identity, bias=valb_sb[:, ct:ct + 1], scale=1.0,
                        )
                        nc.scalar.activation(
                            vsq[:, s2, cols], ps, AF.Square,
                            bias=valb_sb[:, ct:ct + 1], scale=1.0,
                        )
                if pend_vsq is not None:
                    flush_vsq(pend_vsq)
                    pend_vsq = None
            pend_vsq = vsq
        flush_vsq(pend_vsq)

        # ---------- stage C for one group-pair ----------
        def emit_c_kq(stage, gg):
            """k path for double-gct gg (two gc tiles); DR sums deferred."""
            kw_t = kwpool.tile([128, NET, 256], F16, name="kw_t", tag="w")
            nc.sync.dma_start(
                out=kw_t,
                in_=kwT.rearrange("(et p) c -> p et c", p=128)[
                    :, :, gg * 256:(gg + 1) * 256],
            )
            ksqp = scr.tile([128, 2, NTOK], FP8, name="ksqp", tag="p8")
            qsqp = scr.tile([128, 2, NTOK], FP8, name="qsqp", tag="p8")
            kqs = []
            for s2 in range(2):
                gct = gg * 2 + s2
                q_sb = qpool.tile([128, NTOK], F16, name="q_sb", tag="q")
                nc.sync.dma_start(
                    out=q_sb, in_=hidT[gct * 128:(gct + 1) * 128, :]
                )
                kq = kqpool.tile([128, NTOK], F32R, name="kq", tag="kq")
                for ch in range(NCH):
                    ps = mmp.tile([128, CHW], F32, name="psC", tag="mm")
                    for et in range(NET):
                        nc.tensor.matmul(
                            ps,
                            kw_t[:, et, s2 * 128:(s2 + 1) * 128],
                            emb_all[:, et, ch * CHW:(ch + 1) * CHW],
                            start=(et == 0), stop=(et == NET - 1),
                        )
                    cols = slice(ch * CHW, (ch + 1) * CHW)
                    nc.scalar.activation(
                        ksqp[:, s2, cols], ps, AF.Square,
                        bias=keyb_sb[:, gct:gct + 1], scale=1.0,
                    )
                    nc.gpsimd.tensor_mul(qsqp[:, s2, cols], q_sb[:, cols],
                                         q_sb[:, cols])
                    nc.vector.scalar_tensor_tensor(
                        kq[:, cols], ps, keyb_sb[:, gct:gct + 1],
                        q_sb[:, cols], op0=ALU.add, op1=ALU.mult,
                    )
                kqs.append((gct, kq))
            return ksqp, qsqp, kqs

        def emit_dr(stage, ksqp, qsqp, kqs, last_gg):
            for gct, kq in kqs:
                for ch in range(NCH):
                    sum_mm(stage, lkq_sb[:, gct, :],
                           kq[:, ch * CHW:(ch + 1) * CHW], ch)
            for ch in range(NCH):
                cols = slice(ch * CHW, (ch + 1) * CHW)
                sum_mm(stage, lk8_sb[:, stage, :, :], ksqp[:, :, cols], ch,
                       perf_mode=DR)
                sum_mm(stage, lk8_sb[:, 4 + stage, :, :], qsqp[:, :, cols],
                       ch, last=last_gg, perf_mode=DR)

        def emit_c_kq1(stage, gg, ch):
            """Single-chunk variant (window-3 ch-split passes)."""
            cols = slice(ch * CHW, (ch + 1) * CHW)
            kw_t = kwpool.tile([128, NET, 256], F16, name="kw_t", tag="w")
            nc.sync.dma_start(
                out=kw_t,
                in_=kwT.rearrange("(et p) c -> p et c", p=128)[
                    :, :, gg * 256:(gg + 1) * 256],
            )
            ksqp = scr.tile([128, 2, CHW], FP8, name="ksqp1", tag="p8")
            qsqp = scr.tile([128, 2, CHW], FP8, name="qsqp1", tag="p8")
            kqs = []
            for s2 in range(2):
                gct = gg * 2 + s2
                q_sb = qpool.tile([128, CHW], F16, name="q_sb1", tag="q")
                nc.sync.dma_start(
                    out=q_sb, in_=hidT[gct * 128:(gct + 1) * 128, cols]
                )
                kq = kqpool.tile([128, CHW], F32R, name="kq1", tag="kq")
                ps = mmp.tile([128, CHW], F32, name="psC", tag="mm")
                for et in range(NET):
                    nc.tensor.matmul(
                        ps,
                        kw_t[:, et, s2 * 128:(s2 + 1) * 128],
                        emb_all[:, et, cols],
                        start=(et == 0), stop=(et == NET - 1),
                    )
                nc.scalar.activation(
                    ksqp[:, s2, :], ps, AF.Square,
                    bias=keyb_sb[:, gct:gct + 1], scale=1.0,
                )
                nc.gpsimd.tensor_mul(qsqp[:, s2, :], q_sb, q_sb)
                nc.vector.scalar_tensor_tensor(
                    kq, ps, keyb_sb[:, gct:gct + 1],
                    q_sb, op0=ALU.add, op1=ALU.mult,
                )
                kqs.append((gct, kq))
            return ksqp, qsqp, kqs

        def emit_dr1(stage, ksqp, qsqp, kqs, ch, last_gg):
            for gct, kq in kqs:
                sum_mm(stage, lkq_sb[:, gct, :], kq, ch)
            sum_mm(stage, lk8_sb[:, stage, :, :], ksqp, ch, perf_mode=DR)
            sum_mm(stage, lk8_sb[:, 4 + stage, :, :], qsqp, ch, last=last_gg,
                   perf_mode=DR)

        # ---------- stage D ----------
        # Shared rms_v normalizer: rms_v = sqrt(gate^2*mean(vproj^2)+eps)
        # ~= gate*sqrt(mean(vproj^2)+eps) since gate=sigmoid(..)>0, so the
        # conv input normed = vproj*alpha with ONE shared alpha row; only the
        # residual (value = vproj*gate) needs the per-group gate.
        def emit_alpha():
            # sv (= sum vproj^2) sits at psum row 12: bounce the block to
            # SBUF and matmul-extract the row to partition 0.
            s3a = rowm.tile([16, NTOK], F32R, name="s3a", tag="svz")
            aln = rowsc.tile([1, NTOK], F32, name="aln", tag="rs")
            alpha16 = rowm.tile([1, NTOK], F16, name="alpha16", tag="alpha16")
            nc.scalar.activation(s3a, sums_all[:, :], AF.Copy)
            for ch in range(NCH):
                cols = slice(ch * CHW, (ch + 1) * CHW)
                p = epsum.tile([1, CHW], F32, name="svx", tag="mm")
                nc.tensor.matmul(p, cepr_sb[0:16, 20:21], s3a[:, cols],
                                 start=True, stop=True)
                nc.scalar.activation(aln[:, cols], p, AF.Ln,
                                     bias=ceps_sb[0:1, 6:7],
                                     scale=1.0 / float(C))
            nc.scalar.activation(alpha16, aln, AF.Exp, scale=-0.5)
            return alpha16

        def make_d_tiles(stage):
            T = {}
            for nm in ("p4", "lnp", "lnd", "lng", "sqg", "sgn", "ss4", "ab4",
                       "akr"):
                T[nm] = rowsc.tile([1, NTOK], F32, name=f"{nm}{stage}",
                                   tag="rs")
            T["gate16"] = rowm.tile([1, NTOK], F16, name=f"gate16{stage}",
                                    tag="gate16")
            T["s3"] = rowm.tile([16, NTOK], F32R, name=f"s3_{stage}",
                                tag="ext")
            return T

        def emit_d_s3(stage, T, chs=(0, 1)):
            """Psum sums -> partition-0-based SBUF bounce (+ stage biases)."""
            for ch in chs:
                sl = slice(ch * CHW, (ch + 1) * CHW)
                nc.scalar.activation(T["s3"][:, sl], sums_all[:, sl],
                                     AF.Identity,
                                     bias=ceps_sb[0:16, stage:stage + 1],
                                     scale=1.0)

        def emit_d(stage, T, mul_eng=None, chs=(0, 1)):
            """Per-group gate chain: gate = sigmoid(sign(dot)*sqrt(|graw|)).

            Engines only address partitions at 32-boundaries, so the psum
            region is Act-copied (aligned base -> partition 0) to s3, and
            rows 1+ are pulled to partition-0 psum via one-hot matmuls.
            Row layout: stages 0-2: [ak, aq, dot]; stage 3: [sv, aq, dot, ak].
            """
            me = mul_eng if mul_eng is not None else nc.vector
            s3 = T["s3"]
            p4, lnp, lnd, lng, sqg, sgn, ss4, ab4, gate16 = (
                T["p4"], T["lnp"], T["lnd"], T["lng"], T["sqg"], T["sgn"],
                T["ss4"], T["ab4"], T["gate16"])
            akr = T["akr"]
            if chs == (0, 1):
                sls = [slice(0, NTOK)]
            else:
                sls = [slice(ch * CHW, (ch + 1) * CHW) for ch in chs]

            def extract(row, ch):
                sel = cepr_sb[0:16, 8 + row:9 + row]
                p = epsum.tile([1, CHW], F32, name=f"x{row}_{stage}",
                               tag="mm")
                nc.tensor.matmul(p, sel,
                                 s3[:, ch * CHW:(ch + 1) * CHW],
                                 start=True, stop=True)
                return p

            # first layer reads the [1, CHW] psums (partition 0), per chunk
            for ch in chs:
                cols = slice(ch * CHW, (ch + 1) * CHW)
                ak_ps = extract(stage, ch)
                aq_ps = extract(4 + stage, ch)
                dot_ps = extract(8 + stage, ch)
                nc.scalar.activation(akr[:, cols], ak_ps, AF.Copy)
                nc.scalar.activation(ab4[:, cols], dot_ps, AF.Square)
                nc.scalar.activation(sgn[:, cols], dot_ps, AF.Sign)
                nc.vector.tensor_mul(p4[:, cols], akr[:, cols], aq_ps)
            # 2ln|dot| and ln(p4/C); 2ln|graw| = 2ln|dot| - ln(p4/C)
            # (plain subtract so the mul engine can be Pool)
            for sl in sls:
                nc.scalar.activation(lnd[:, sl], ab4[:, sl], AF.Ln,
                                     bias=ceps_sb[0:1, 7:8])
            for sl in sls:
                nc.scalar.activation(lnp[:, sl], p4[:, sl], AF.Ln,
                                     scale=1.0 / float(C))
            for sl in sls:
                me.tensor_sub(lng[:, sl], lnd[:, sl], lnp[:, sl])
            for sl in sls:
                nc.scalar.activation(sqg[:, sl], lng[:, sl], AF.Exp,
                                     scale=0.25)
            for sl in sls:
                me.tensor_mul(ss4[:, sl], sqg[:, sl], sgn[:, sl])
            for sl in sls:
                nc.scalar.activation(gate16[:, sl], ss4[:, sl], AF.Sigmoid)
            return gate16

        # ---------- stage E ----------
        def bcast_ch(src, dst, ch):
            bp = epsum.tile([128, CHW], F32, name="bp", tag="mm")
            nc.tensor.matmul(
                bp, bc2_sb[0:1, 0:128],
                src[:, ch * CHW:(ch + 1) * CHW],
                start=True, stop=True,
            )
            nc.scalar.activation(
                dst[:, ch * CHW:(ch + 1) * CHW], bp, AF.Copy)

        def bcast_row(src, tag):
            """[1, NTOK] f32/f16 row -> [128, NTOK] f16 via PE broadcast."""
            dst = rowm.tile([128, NTOK], F16, name=f"b_{tag}", tag=tag)
            for ch in range(NCH):
                bcast_ch(src, dst, ch)
            return dst

        # nx16[ct]: f16 conv input, shared by all 4 groups' units:
        # [9 zeros | vproj*alpha]; the halo contribution to the first 9
        # outputs is a host-computed f16 correction (hc) accumulated via an
        # identity matmul.
        PADW = HALO + NTOK
        nx8s = {}

        def emit_nx8(ct):
            nx8 = npool.tile([128, PADW], F16, name=f"nx16_{ct}",
                             tag=f"nx16_{ct}", bufs=1)
            nc.gpsimd.memset(nx8[:, 0:HALO], 0.0)
            nc.vector.tensor_mul(nx8[:, HALO:HALO + NTOK],
                                 vproj16[:, ct, :], ab16)
            nx8s[ct] = nx8

        def emit_val(gct, gb16, on_pool=False):
            ct = gct % NCT
            val = vpool.tile([128, NTOK], F16, name="val", tag="val")
            if on_pool:
                nc.gpsimd.tensor_mul(val, vproj16[:, ct, :], gb16)
            else:
                nc.vector.tensor_mul(val, vproj16[:, ct, :], gb16)
            return val

        def emit_e_conv_pe(gct, pools=None):
            """f16 conv taps + halo-fix matmul."""
            ct = gct % NCT
            nx8 = nx8s[ct]
            dg_t = dgpool.tile([128, KT * 128], F16, name="dg_t", tag="dg")
            nc.sync.dma_start(out=dg_t, in_=dg16[gct])
            accs = []
            for ch in range(NCH):
                pool = (pools[ch % len(pools)] if pools else epsum)
                acc = pool.tile([128, CHW], F32, name="acc", tag="mm")
                for k in range(KT):
                    base = ch * CHW + k * DIL
                    nc.tensor.matmul(
                        acc,
                        dg_t[:, k * 128:(k + 1) * 128],
                        nx8[:, base:base + CHW],
                        start=(k == 0), stop=(k == KT - 1 and ch == 1),
                        skip_group_check=True,
                    )
                if ch == 0:
                    nc.tensor.matmul(
                        acc[:, 0:HALO], id16_sb, hc_sb[:, gct, :],
                        start=False, stop=True, skip_group_check=True,
                    )
                accs.append(acc)
            return accs

        def emit_e_conv_dve(gct):
            """f16 conv as DVE scalar-ptr MAC chains (+ in-place halo fix)."""
            ct = gct % NCT
            nx8 = nx8s[ct]
            outs = []
            for ch in range(NCH):
                prev = None
                for k in range(KT):
                    win = nx8[:, ch * CHW + k * DIL:ch * CHW + k * DIL + CHW]
                    a = cacc.tile([128, CHW], F16, name=f"ca{k}", tag=f"ca{k}")
                    wcol = cwf_sb[:, gct * KT + k:gct * KT + k + 1]
                    if k == 0:
                        nc.vector.tensor_scalar_mul(a, win, wcol)
                    else:
                        nc.vector.scalar_tensor_tensor(
                            a, win, wcol, prev, op0=ALU.mult, op1=ALU.add)
                    prev = a
                if ch == 0:
                    nc.vector.tensor_tensor(prev[:, 0:HALO], prev[:, 0:HALO],
                                            hc_sb[:, gct, :], op=ALU.add)
                outs.append(prev)
            return outs

        def emit_silu(accs):
            sacc = opool.tile([128, NTOK], F16, name="sacc", tag="sacc")
            for ch in range(NCH):
                nc.scalar.activation(sacc[:, ch * CHW:(ch + 1) * CHW],
                                     accs[ch], AF.Silu)
            return sacc

        def emit_resid_out(gct, val, sacc, engine="pool"):
            ot = opool.tile([128, NTOK], F16, name="ot", tag="ot")
            if engine == "dve":
                nc.vector.tensor_tensor(ot, val, sacc, op=ALU.add)
            else:
                nc.gpsimd.tensor_add(ot, val, sacc)
            nc.sync.dma_start(out=out_d[gct * 128:(gct + 1) * 128, :], in_=ot)

        # ---------- pipeline ----------
        # conv+silu only needs the shared ab16; val/resid needs gate(g).
        # Window g: C(g) + chain(g-1) + full units of group g-1 + a few
        # group-3 conv units pulled early; tail: 3 conv units cover chain(3),
        # then group-3 val/resid.
        sacc3 = {}      # gct -> long-lived sacc for group-3 units
        ab16 = None
        TAIL3 = [29, 30, 31]
        EARLY3 = {0: [24, 25, 26], 1: [27], 2: [], 3: []}

        def conv_unit(u, long_lived=False, pools=None, defer_silu=False,
                      dve=False):
            if dve:
                accs = emit_e_conv_dve(u)
            else:
                accs = emit_e_conv_pe(u, pools=pools)
            if defer_silu:
                return accs
            if long_lived:
                sacc = opool.tile([128, NTOK], F16, name=f"sacc{u}",
                                  tag=f"sacc3_{u}", bufs=1)
            else:
                sacc = opool.tile([128, NTOK], F16, name=f"sacc{u}",
                                  tag="sacc")
            for ch in range(NCH):
                nc.scalar.activation(sacc[:, ch * CHW:(ch + 1) * CHW],
                                     accs[ch], AF.Silu)
            if long_lived:
                sacc3[u] = sacc
            return sacc

        def full_unit(u, gb16, dve=False):
            sacc = conv_unit(u, dve=dve)
            val = emit_val(u, gb16)
            emit_resid_out(u, val, sacc, engine="pool" if dve else "dve")

        gate_prev = None
        d_tiles = {}
        for g in range(3):
            dr_prev = None
            gb16 = None
            units = list(range((g - 1) * 8, g * 8)) if g else []
            for i, gg in enumerate(range(g * 4, (g + 1) * 4)):
                if i == 0 and g:
                    d_tiles[g - 1] = make_d_tiles(g - 1)
                    emit_d_s3(g - 1, d_tiles[g - 1])
                cur = emit_c_kq(g, gg)
                if i == 0:
                    if g == 0:
                        alpha16 = emit_alpha()
                    else:
                        gate_prev = emit_d(g - 1, d_tiles[g - 1],
                                           mul_eng=nc.gpsimd)
                if dr_prev is not None:
                    emit_dr(g, *dr_prev, last_gg=False)
                dr_prev = cur
                if i == 1:
                    if g == 0:
                        ab16 = bcast_row(alpha16, "ab16")
                        for ct in (0, 1, 2):
                            emit_nx8(ct)
                    elif g == 1:
                        for ct in (6, 7):
                            emit_nx8(ct)
                    batch = []
                elif i == 2:
                    if g:
                        gb16 = bcast_row(gate_prev, f"gb{g - 1}")
                        batch = units[0:3]
                    else:
                        emit_nx8(3)
                        batch = EARLY3[0][0:2]
                elif i == 3:
                    if g == 0:
                        emit_nx8(4)
                        emit_nx8(5)
                    batch = units[3:6] if g else EARLY3[0][2:3]
                else:
                    batch = []
                for u in batch:
                    if g:
                        full_unit(u, gb16, dve=(u % 8 in (0, 2, 4)))
                    else:
                        conv_unit(u, long_lived=True, dve=True)
            emit_dr(g, *dr_prev, last_gg=True)
            if g:
                for u in units[6:8]:
                    full_unit(u, gb16, dve=(u % 8 == 6))
                for u in EARLY3[g]:
                    conv_unit(u, long_lived=True)

        # ---------- window 3: chunk-split passes ----------
        # pass p computes C(3) for token chunk p only, so the stage-3 gate
        # chain + group-3 epilogue for chunk 0 hide under pass 1.
        units = list(range(16, 24))
        d_tiles[2] = make_d_tiles(2)
        emit_d_s3(2, d_tiles[2])
        T3 = None
        gb3 = rowm.tile([128, NTOK], F16, name="b_gb3", tag="gb3")

        def epi3_ch(u, ch):
            ct = u % NCT
            cols = slice(ch * CHW, (ch + 1) * CHW)
            val = vpool.tile([128, CHW], F16, name="val3", tag="val")
            nc.vector.tensor_mul(val, vproj16[:, ct, cols], gb3[:, cols])
            ot = opool.tile([128, CHW], F16, name="ot3", tag="ot")
            nc.vector.tensor_tensor(ot, val, sacc3[u][:, cols], op=ALU.add)
            nc.sync.dma_start(out=out_d[u * 128:(u + 1) * 128, cols], in_=ot)

        for p in range(2):
            dr_prev = None
            for i, gg in enumerate(range(12, 16)):
                cur = emit_c_kq1(3, gg, p)
                if p == 0 and i == 0:
                    gate2 = emit_d(2, d_tiles[2], mul_eng=nc.gpsimd)
                if p == 1 and i == 0:
                    T3 = make_d_tiles(3)
                    emit_d_s3(3, T3, chs=(0,))
                    gate3 = emit_d(3, T3, mul_eng=nc.vector, chs=(0,))
                if dr_prev is not None:
                    emit_dr1(3, *dr_prev, p, last_gg=False)
                dr_prev = cur
                if p == 0:
                    if i == 2:
                        gb2 = bcast_row(gate2, "gb2")
                        batch = units[0:3]
                    elif i == 3:
                        batch = units[3:6]
                    else:
                        batch = []
                    for u in batch:
                        full_unit(u, gb2, dve=(u % 8 in (0, 2, 4)))
                else:
                    if i == 1:
                        bcast_ch(gate3, gb3, 0)
                        for u in units[6:8]:
                            full_unit(u, gb2, dve=(u % 8 == 6))
                    elif i == 2:
                        for u in range(24, 28):
                            epi3_ch(u, 0)
                    elif i == 3:
                        pass
            emit_dr1(3, *dr_prev, p, last_gg=True)

        # ---------- tail: chunk 1 of the group-3 gate + epilogue ----------
        # TAIL3 conv matmuls cover the chain; their silus follow its Act ops
        emit_d_s3(3, T3, chs=(1,))
        acc_pools = [epsum, mmp]
        emit_d(3, T3, mul_eng=nc.vector, chs=(1,))
        TAIL4 = [28] + TAIL3
        tail_accs = [conv_unit(u, pools=acc_pools, defer_silu=True)
                     for u in TAIL4]
        bcast_ch(gate3, gb3, 1)
        for j, u in enumerate(TAIL4):
            sacc = opool.tile([128, NTOK], F16, name=f"sacc{u}",
                              tag=f"sacc3_{u}", bufs=1)
            for ch in range(NCH):
                nc.scalar.activation(sacc[:, ch * CHW:(ch + 1) * CHW],
                                     tail_accs[j][ch], AF.Silu)
            sacc3[u] = sacc
        for u in TAIL4:
            epi3_ch(u, 0)
        for u in range(24, 32):
            epi3_ch(u, 1)

        for p in (cacc, dgpool, opool, vpool, npool, rowsc, rowm, kqpool, scr,
                  epsum, sump, mmp, qpool, kwpool, consts):
            p.release()
    return nc


def host_prep(embeddings, hidden_states, key_w, key_b, value_w, value_b,
              w_key_norm, w_query_norm, w_norm, conv_weight):
    """Build the per-core input maps."""
    f32, f16 = np.float32, np.float16
    e4 = ml_dtypes.float8_e4m3fn
    embeddings = np.asarray(embeddings, f32)
    hidden_states = np.asarray(hidden_states, f32)
    key_w = np.asarray(key_w, f32)
    key_b = np.asarray(key_b, f32)
    value_w = np.asarray(value_w, f32)
    value_b = np.asarray(value_b, f32)
    w_key_norm = np.asarray(w_key_norm, f32)
    w_query_norm = np.asarray(w_query_norm, f32)
    w_norm = np.asarray(w_norm, f32)
    conv_weight = np.asarray(conv_weight, f32)

    kwT = np.ascontiguousarray(key_w.T).astype(f16)        # [E, GC]
    vwT = np.ascontiguousarray(value_w.T).astype(f16)      # [E, C]
    keyb_r = np.ascontiguousarray(key_b.reshape(NGCT, 128).T)  # [128, NGCT]
    valb_r = np.ascontiguousarray(value_b.reshape(NCT, 128).T)
    wkq = (w_key_norm * w_query_norm).reshape(GC)

    # one-hot lhsT tables. ONE shared [16, NTOK] psum accumulator with
    # disjoint rows: ak_g = row g, aq_g = 4+g, dot_g = 8+g, sv = 12.
    # (engines can only address 32-aligned partition bases, so rows are
    #  matmul-extracted after an Act bounce of the block to partition 0)
    lk8 = np.zeros((128, 9, 2, 16), f32)
    for g in range(G):
        lk8[:, g, :, g] = 1.0          # ksq -> row g
        lk8[:, 4 + g, :, 4 + g] = 1.0  # qsq -> row 4+g
    lk8[:, 8, :, 12] = 1.0             # vsq -> row 12 (sv)
    lk8 = lk8.reshape(128, 288).astype(e4)

    lkq = np.zeros((NGCT, 128, 16), f32)
    for gct in range(NGCT):
        g = gct // NCT
        lkq[gct, :, 8 + g] = wkq[gct * 128:(gct + 1) * 128]

    aux16 = np.zeros((128, 16 + 256), f16)
    aux16[:, 12] = 1.0        # lv one-hot: vsq -> row 12 (sv)
    for j in range(2):
        aux16[j, 16 + j * 128:16 + (j + 1) * 128] = 1.0

    # ceps: cols 0-3 = per-stage bias vectors (+C*EPS on ak/aq rows);
    #        cols 8+r = f32 one-hot row selectors (identity)
    ceps_h = np.zeros((128, 24), f32)
    for g in range(G):
        ceps_h[g, g] = float(C) * EPS
        ceps_h[4 + g, g] = float(C) * EPS
    for r in range(16):
        ceps_h[r, 8 + r] = 1.0
    ceps_h[0, 6] = NORM_EPS
    ceps_h[0, 7] = 1e-60

    # f16 diagonal conv weights + identity for the halo-fix matmul.
    cwf = (conv_weight.reshape(G, C, KT) * w_norm[:, :, None]).astype(f32)
    dg = np.zeros((NGCT, 128, KT * 128), f16)
    idx = np.arange(128)
    for gct in range(NGCT):
        g, ct = gct // NCT, gct % NCT
        for k in range(KT):
            dg[gct, idx, k * 128 + idx] = cwf[g, ct * 128 + idx, k].astype(f16)
    id16_h = np.zeros((128, 128), f16)
    id16_h[idx, idx] = 1.0
    cwf_r = np.zeros((128, NGCT * KT), f32)
    for gct in range(NGCT):
        g, ct = gct // NCT, gct % NCT
        for k in range(KT):
            cwf_r[:, gct * KT + k] = cwf[g, ct * 128:(ct + 1) * 128, k]

    in_maps = []
    for core in range(NCORES):
        b = core // (NCORES // B)
        t0 = (core % (NCORES // B)) * NTOK
        emb_s = embeddings[b, t0:t0 + NTOK]                # [NTOK, E]
        hid_s = hidden_states[b, t0:t0 + NTOK].reshape(NTOK, GC)
        emb_c = np.ascontiguousarray(emb_s.T).astype(f16)  # [E, NTOK]
        hid_c = np.ascontiguousarray(hid_s.T).astype(f16)  # [GC, NTOK]

        # halo: nhat (= value / rms_v, w_norm NOT applied) for the 9
        # preceding tokens feeds a host-computed conv correction hc for the
        # first 9 output tokens; zeros at the sequence start.
        if t0 == 0:
            hc_c = np.zeros((128, NGCT * HALO), f16)
        else:
            th = slice(t0 - HALO, t0)
            e9 = embeddings[b, th]                          # [9, E]
            k9 = (e9 @ key_w.T + key_b).reshape(HALO, G, C)
            q9 = hidden_states[b, th]                       # [9, G, C]
            rk = np.sqrt((k9 * k9).mean(-1) + EPS)
            rq = np.sqrt((q9 * q9).mean(-1) + EPS)
            d9 = np.einsum("tgc,gc,tgc,gc->tg", k9, w_key_norm, q9,
                           w_query_norm)
            graw = d9 / (rk * rq) / np.sqrt(f32(C))
            g9 = 1.0 / (1.0 + np.exp(-(np.where(graw >= 0, 1.0, -1.0)
                                       * np.sqrt(np.maximum(np.abs(graw),
                                                            1e-6)))))
            vp9 = e9 @ value_w.T + value_b                  # [9, C]
            val9 = vp9[:, None, :] * g9[..., None].astype(f32)
            rv9 = np.sqrt((val9 * val9).mean(-1) + NORM_EPS)
            nhat9 = val9 / rv9[..., None]                   # [9, G, C]
            # hc[c, gct, t] = sum_{k: t+k*DIL<9} cwf[g,c,k]*nhat9[t+k*DIL,g,c]
            hcf = np.zeros((HALO, G, C), f32)
            for t in range(HALO):
                for k in range(KT):
                    ix = t + k * DIL
                    if ix < HALO:
                        hcf[t] += cwf[:, :, k] * nhat9[ix]
            hg = hcf.transpose(1, 2, 0).reshape(NGCT, 128, HALO)
            hc_c = np.ascontiguousarray(
                hg.transpose(1, 0, 2).reshape(128, NGCT * HALO)).astype(f16)

        in_maps.append({
            "emb16": emb_c, "hidT": hid_c, "kwT": kwT, "vwT": vwT,
            "keyb": keyb_r, "valb": valb_r,
            "lk8": lk8, "lkq": lkq, "aux16": aux16, "ceps": ceps_h,
            "dg16": dg, "cwf": cwf_r, "hc": hc_c, "id16": id16_h,
        })
    return in_maps


_NC_CACHE = [None]
LAST_RESULT = [None]


def kernel(**inputs) -> np.ndarray:
    in_maps = host_prep(**inputs)
    if _NC_CACHE[0] is None:
        _NC_CACHE[0] = build_program()
    nc = _NC_CACHE[0]
    res = run_bass_kernel_spmd(nc, in_maps, list(range(NCORES)))
    LAST_RESULT[0] = res
    out = np.empty((B, T, G, C), np.float32)
    for core in range(NCORES):
        b = core // (NCORES // B)
        t0 = (core % (NCORES // B)) * NTOK
        oc = np.asarray(res.results[core]["out"]).astype(np.float32)
        out[b, t0:t0 + NTOK] = oc.reshape(G, C, NTOK).transpose(2, 0, 1)
    return out


# revision 121
# speedup vs baseline: 1.0125x; 1.0049x over previous
"""Trainium2 Bass kernel for the EngramNew module (dense_cnn), v3.

Sharding: B*T = 8192 tokens split across 8 cores (1024 tokens each); the conv
halo of (K-1)*DIL = 9 tokens is precomputed host-side.  On-device layout is
channels-on-partitions / tokens-on-free: [G*C, T_core].

v10 design vs the v2 baseline (291.8us -> 225.1us):
 - shared rms_v normalizer: rms_v = sqrt(gate^2*mean(vproj^2)+eps)
   ~= gate*sqrt(mean(vproj^2)+eps) since gate = sigmoid(..) > 0, so the
   conv input (normed) = vproj*alpha with ONE shared alpha row; only the
   residual needs the per-group gate.  This decouples the whole conv
   pipeline from the gate chains (error <= ~1e-3, verified vs reference).
 - ONE [16,NTOK] PSUM accumulator shared by all four gate-sum stages via
   disjoint one-hot rows (ak_g=g, aq_g=4+g, dot_g=8+g, sv=12), reset once:
   no PSUM bank-rotation WAR stalls.  Rows are bounced to partition 0 by
   an Act copy + f32r one-hot extraction matmuls (engines can only address
   32-aligned partition bases).
 - per-ct conv input tiles (zero prefix + vproj*alpha) shared by all 4
   groups; the halo contribution to the first 9 outputs is a host-computed
   correction added via an identity matmul.  15 of 32 conv units run as
   DVE MAC chains, the rest as PE diag matmuls.
 - C(3) is split into two per-chunk passes so chunk 0 of the stage-3 gate
   chain + epilogue hides under the chunk-1 pass; 3 conv units are held
   back as PE cover for the chunk-1 chain.
 - kq / ksq+qsq(fp8 DR) / vsq(fp8 DR) reductions all deferred by one
   gg/vv so their producers never stall the PE sum matmuls.
 - startup: split vw/emb DMAs ordered first + 4-psum et-outer first vproj
   pass so PE starts at ~4us instead of 13us.
"""

import os
import sys

for _p in ("/opt/trn_rl_repo",):
    if _p not in sys.path:
        sys.path.insert(0, _p)

import numpy as np
import ml_dtypes

import concourse.bass as bass
from concourse import mybir
from concourse.tile import TileContext
from concourse.bass_utils import run_bass_kernel_spmd
import bass_rust

F32 = mybir.dt.float32
F32R = mybir.dt.float32r
F16 = mybir.dt.float16
FP8 = mybir.dt.float8e4
AF = mybir.ActivationFunctionType
ALU = mybir.AluOpType
DR = mybir.MatmulPerfMode.DoubleRow

# Problem constants (hardcoded per spec nn_EngramNew_2070174237244)
B, T, G, C, E = 2, 4096, 4, 1024, 1024
GC = G * C
KT, DIL = 4, 3          # conv taps / dilation
EPS = 1e-5
NORM_EPS = 1e-5
NCORES = 8
NTOK = (B * T) // NCORES    # 1024 tokens per core
HALO = (KT - 1) * DIL       # 9
NET = E // 128              # 8 e-tiles
NGCT = GC // 128            # 32 gc-tiles
NCT = C // 128              # 8 c-tiles
CHW = 512                   # token chunk width (1 PSUM bank of fp32)
NCH = NTOK // CHW           # 2 chunks



class PatchedTileContext(TileContext):
    """This walrus build allows only one sem wait per instruction (two on
    EventSemaphore). Tile attaches as many waits as an instruction needs,
    so after scheduling we hoist excess waits onto no-op instructions
    inserted just before the owner on the same engine (engines are strict
    FIFO, so observing the sems earlier is equivalent)."""

    def _split_excess_waits(self):
        nc = self.nc

        def make_nop(engine):
            bi = nc.engines[engine].nop()
            bb = nc.cur_bb.bb
            lst = list(bb.instructions)
            assert lst[-1] is bi.ins
            bb.instructions = lst[:-1]
            return bi.ins

        # Phase 1: snapshot every block BEFORE creating any nop, so nops
        # appended to cur_bb can never leak into the iteration or the rebuilt
        # lists (cur_bb may be one of the blocks being processed).
        snapshots = []
        for f in nc.m.functions:
            for blk in f.blocks:
                snapshots.append((blk, list(blk.instructions)))

        for blk, insts in snapshots:
            out = []
            changed = False
            for ins in insts:
                si = ins.sync_info
                waits = list(si.on_wait) if (si and si.on_wait) else []
                cap = 2 if isinstance(ins, mybir.InstEventSemaphore) else 1
                if len(waits) > cap:
                    changed = True
                    for w in waits[cap:]:
                        nop = make_nop(ins.engine)
                        nop.sync_info = bass_rust.SyncInfo(
                            on_wait=[w], on_update=[]
                        )
                        out.append(nop)
                    upd = list(si.on_update) if si.on_update else []
                    ins.sync_info = bass_rust.SyncInfo(
                        on_wait=waits[:cap], on_update=upd
                    )
                out.append(ins)
            if changed:
                blk.instructions = out

    def _drain_and_barrier(self, tick_clock, wait_clock):
        super()._drain_and_barrier(tick_clock, wait_clock)
        self._split_excess_waits()


def _r(ap):
    return ap.bitcast(F32R)


def build_program():
    nc = bass.Bass()

    # ---- DRAM parameters ----
    emb16 = nc.declare_dram_parameter("emb16", [E, NTOK], F16, isOutput=False)
    hidT = nc.declare_dram_parameter("hidT", [GC, NTOK], F16, isOutput=False)
    kwT = nc.declare_dram_parameter("kwT", [E, GC], F16, isOutput=False)
    vwT = nc.declare_dram_parameter("vwT", [E, C], F16, isOutput=False)
    keyb = nc.declare_dram_parameter("keyb", [128, NGCT], F32, isOutput=False)
    valb = nc.declare_dram_parameter("valb", [128, NCT], F32, isOutput=False)
    lk8 = nc.declare_dram_parameter("lk8", [128, 9 * 2 * 16], FP8,
                                    isOutput=False)
    lkq = nc.declare_dram_parameter("lkq", [NGCT, 128, 16], F32, isOutput=False)
    aux16 = nc.declare_dram_parameter("aux16", [128, 16 + 256], F16,
                                      isOutput=False)
    ceps = nc.declare_dram_parameter("ceps", [128, 24], F32, isOutput=False)
    dg16 = nc.declare_dram_parameter("dg16", [NGCT, 128, KT * 128], F16,
                                     isOutput=False)
    cwf = nc.declare_dram_parameter("cwf", [128, NGCT * KT], F32,
                                    isOutput=False)
    hc = nc.declare_dram_parameter("hc", [128, NGCT * HALO], F16,
                                   isOutput=False)
    id16 = nc.declare_dram_parameter("id16", [128, 128], F16, isOutput=False)
    out_d = nc.declare_dram_parameter("out", [GC, NTOK], F16, isOutput=True)

    with PatchedTileContext(nc) as tc:
        consts = tc.alloc_tile_pool(name="consts", bufs=1)
        kwpool = tc.alloc_tile_pool(name="kwpool", bufs=2)
        qpool = tc.alloc_tile_pool(name="qpool", bufs=3)
        mmp = tc.alloc_tile_pool(name="mmp", bufs=4, space=bass.MemorySpace.PSUM)
        sump = tc.alloc_tile_pool(name="sump", bufs=1, space=bass.MemorySpace.PSUM)
        epsum = tc.alloc_tile_pool(name="epsum", bufs=2,
                                   space=bass.MemorySpace.PSUM)
        scr = tc.alloc_tile_pool(name="scr", bufs=4)
        kqpool = tc.alloc_tile_pool(name="kqpool", bufs=4)
        rowm = tc.alloc_tile_pool(name="rowm", bufs=1)
        rowsc = tc.alloc_tile_pool(name="rowsc", bufs=9)
        npool = tc.alloc_tile_pool(name="npool", bufs=3)
        vpool = tc.alloc_tile_pool(name="vpool", bufs=4)
        opool = tc.alloc_tile_pool(name="opool", bufs=5)
        dgpool = tc.alloc_tile_pool(name="dgpool", bufs=3)
        cacc = tc.alloc_tile_pool(name="cacc", bufs=3)

        # ---- load order: vw(vv0) first, then emb per-et, then small consts
        vw_t0 = kwpool.tile([128, NET, 256], F16, name="vw_t0", tag="w")
        for eh in range(2):
            nc.sync.dma_start(
                out=vw_t0[:, eh * 4:(eh + 1) * 4, :],
                in_=vwT.rearrange("(et p) c -> p et c", p=128)[
                    :, eh * 4:(eh + 1) * 4, 0:256],
            )
        emb_all = consts.tile([128, NET, NTOK], F16)
        for et in range(NET):
            nc.sync.dma_start(out=emb_all[:, et, :],
                              in_=emb16[et * 128:(et + 1) * 128, :])
        vw_t1 = kwpool.tile([128, NET, 256], F16, name="vw_t1", tag="w")
        nc.sync.dma_start(
            out=vw_t1,
            in_=vwT.rearrange("(et p) c -> p et c", p=128)[:, :, 256:512],
        )
        valb_sb = consts.tile([128, NCT], F32)
        nc.sync.dma_start(out=valb_sb, in_=valb[:, :])
        aux_sb = consts.tile([128, 16 + 256], F16)
        nc.sync.dma_start(out=aux_sb, in_=aux16[:, :])
        ceps_sb = consts.tile([128, 24], F32)
        nc.sync.dma_start(out=ceps_sb, in_=ceps[:, :])
        cepr_sb = consts.tile([128, 24], F32R)
        nc.sync.dma_start(out=cepr_sb, in_=_r(ceps[:, :]))
        keyb_sb = consts.tile([128, NGCT], F32)
        nc.sync.dma_start(out=keyb_sb, in_=keyb[:, :])
        lk8_sb = consts.tile([128, 9, 2, 16], FP8)
        nc.sync.dma_start(out=lk8_sb,
                          in_=lk8.rearrange("p (q i c) -> p q i c", i=2, c=16))
        lkq_sb = consts.tile([128, NGCT, 16], F32R)
        nc.sync.dma_start(out=lkq_sb, in_=_r(lkq.rearrange("n p m -> p n m")))
        cwf_sb = consts.tile([128, NGCT * KT], F32)
        nc.sync.dma_start(out=cwf_sb, in_=cwf[:, :])
        hc_sb = consts.tile([128, NGCT, HALO], F16)
        nc.sync.dma_start(out=hc_sb,
                          in_=hc.rearrange("p (n h) -> p n h", h=HALO))
        id16_sb = consts.tile([128, 128], F16)
        nc.sync.dma_start(out=id16_sb, in_=id16[:, :])
        vproj16 = consts.tile([128, NCT, NTOK], F16)
        bc2_sb = aux_sb[0:1, 16:16 + 128]

        # ---- gate sums: ONE [16, NTOK] psum shared by all stages via
        # disjoint one-hot rows: ak_g = row g, aq_g = 4+g, dot_g = 8+g,
        # sv = 12. Reset once (B's first vsq sum); everything accumulates.
        sums_all = sump.tile([16, NTOK], F32, name="sums_all", tag="sums")
        first_sum = [True] * NCH

        def sum_mm(stage, lhsT, rhs, ch, last=False, perf_mode=None):
            st = first_sum[ch]
            first_sum[ch] = False
            nc.tensor.matmul(
                sums_all[:, ch * CHW:(ch + 1) * CHW],
                lhsT, rhs, start=st, stop=last,
                perf_mode=perf_mode, skip_group_check=True,
            )

        # ---------- stage B: vproj = value_w @ emb + value_b ----------
        # vsq in fp8 (feeds only alpha), DoubleRow-reduced, deferred one vv
        pend_vsq = None

        def flush_vsq(v8):
            for ch in range(NCH):
                cols = slice(ch * CHW, (ch + 1) * CHW)
                sum_mm(3, lk8_sb[:, 8, :, :], v8[:, :, cols], ch,
                       perf_mode=DR)

        for vv in range(NCT // 2):
            if vv == 0:
                vw_t = vw_t0
            elif vv == 1:
                vw_t = vw_t1
            else:
                vw_t = kwpool.tile([128, NET, 256], F16, name="vw_t", tag="w")
                nc.sync.dma_start(
                    out=vw_t,
                    in_=vwT.rearrange("(et p) c -> p et c", p=128)[
                        :, :, vv * 256:(vv + 1) * 256],
                )
            vsq = scr.tile([128, 2, NTOK], FP8, name="vsq8", tag="p8")
            if vv == 0:
                # et-outer across 4 psums so PE rate-matches the emb DMAs
                ps4 = [mmp.tile([128, CHW], F32, name=f"psB0_{i}", tag="mm")
                       for i in range(3)]
                ps4.append(epsum.tile([128, CHW], F32, name="psB0_3",
                                      tag="mm"))
                for et in range(NET):
                    for i in range(4):
                        s2, ch = i // 2, i % 2
                        nc.tensor.matmul(
                            ps4[i],
                            vw_t[:, et, s2 * 128:(s2 + 1) * 128],
                            emb_all[:, et, ch * CHW:(ch + 1) * CHW],
                            start=(et == 0), stop=(et == NET - 1),
                        )
                for i in range(4):
                    s2, ch = i // 2, i % 2
                    ct = vv * 2 + s2
                    cols = slice(ch * CHW, (ch + 1) * CHW)
                    nc.scalar.activation(
                        vproj16[:, ct, cols], ps4[i],
                        AF.Identity, bias=valb_sb[:, ct:ct + 1], scale=1.0,
                    )
                    nc.scalar.activation(
                        vsq[:, s2, cols], ps4[i], AF.Square,
                        bias=valb_sb[:, ct:ct + 1], scale=1.0,
                    )
            else:
                for s2 in range(2):
                    ct = vv * 2 + s2
                    for ch in range(NCH):
                        cols = slice(ch * CHW, (ch + 1) * CHW)
                        ps = mmp.tile([128, CHW], F32, name="psB", tag="mm")
                        for et in range(NET):
                            nc.tensor.matmul(
                                ps,
                                vw_t[:, et, s2 * 128:(s2 + 1) * 128],
                                emb_all[:, et, ch * CHW:(ch + 1) * CHW],
                                start=(et == 0), stop=(et == NET - 1),
                            )
                        nc.scalar.activation(
                            vproj16[:, ct, cols], ps,
                            AF.Identity, bias=valb_sb[:, ct:ct + 1], scale=1.0,
                        )
                        nc.scalar.activation(
                            vsq[:, s2, cols], ps, AF.Square,
                            bias=valb_sb[:, ct:ct + 1], scale=1.0,
                        )
                if pend_vsq is not None:
                    flush_vsq(pend_vsq)
                    pend_vsq = None
            pend_vsq = vsq
        flush_vsq(pend_vsq)

        # ---------- stage C for one group-pair ----------
        def emit_c_kq(stage, gg):
            """k path for double-gct gg (two gc tiles); DR sums deferred."""
            kw_t = kwpool.tile([128, NET, 256], F16, name="kw_t", tag="w")
            nc.sync.dma_start(
                out=kw_t,
                in_=kwT.rearrange("(et p) c -> p et c", p=128)[
                    :, :, gg * 256:(gg + 1) * 256],
            )
            ksqp = scr.tile([128, 2, NTOK], FP8, name="ksqp", tag="p8")
            qsqp = scr.tile([128, 2, NTOK], FP8, name="qsqp", tag="p8")
            kqs = []
            for s2 in range(2):
                gct = gg * 2 + s2
                q_sb = qpool.tile([128, NTOK], F16, name="q_sb", tag="q")
                nc.sync.dma_start(
                    out=q_sb, in_=hidT[gct * 128:(gct + 1) * 128, :]
                )
                kq = kqpool.tile([128, NTOK], F32R, name="kq", tag="kq")
                for ch in range(NCH):
                    ps = mmp.tile([128, CHW], F32, name="psC", tag="mm")
                    for et in range(NET):
                        nc.tensor.matmul(
                            ps,
                            kw_t[:, et, s2 * 128:(s2 + 1) * 128],
                            emb_all[:, et, ch * CHW:(ch + 1) * CHW],
                            start=(et == 0), stop=(et == NET - 1),
                        )
                    cols = slice(ch * CHW, (ch + 1) * CHW)
                    nc.scalar.activation(
                        ksqp[:, s2, cols], ps, AF.Square,
                        bias=keyb_sb[:, gct:gct + 1], scale=1.0,
                    )
                    nc.gpsimd.tensor_mul(qsqp[:, s2, cols], q_sb[:, cols],
                                         q_sb[:, cols])
                    nc.vector.scalar_tensor_tensor(
                        kq[:, cols], ps, keyb_sb[:, gct:gct + 1],
                        q_sb[:, cols], op0=ALU.add, op1=ALU.mult,
                    )
                kqs.append((gct, kq))
            return ksqp, qsqp, kqs

        def emit_dr(stage, ksqp, qsqp, kqs, last_gg):
            for gct, kq in kqs:
                for ch in range(NCH):
                    sum_mm(stage, lkq_sb[:, gct, :],
                           kq[:, ch * CHW:(ch + 1) * CHW], ch)
            for ch in range(NCH):
                cols = slice(ch * CHW, (ch + 1) * CHW)
                sum_mm(stage, lk8_sb[:, stage, :, :], ksqp[:, :, cols], ch,
                       perf_mode=DR)
                sum_mm(stage, lk8_sb[:, 4 + stage, :, :], qsqp[:, :, cols],
                       ch, last=last_gg, perf_mode=DR)

        def emit_c_kq1(stage, gg, ch):
            """Single-chunk variant (window-3 ch-split passes)."""
            cols = slice(ch * CHW, (ch + 1) * CHW)
            kw_t = kwpool.tile([128, NET, 256], F16, name="kw_t", tag="w")
            nc.sync.dma_start(
                out=kw_t,
                in_=kwT.rearrange("(et p) c -> p et c", p=128)[
                    :, :, gg * 256:(gg + 1) * 256],
            )
            ksqp = scr.tile([128, 2, CHW], FP8, name="ksqp1", tag="p8")
            qsqp = scr.tile([128, 2, CHW], FP8, name="qsqp1", tag="p8")
            kqs = []
            for s2 in range(2):
                gct = gg * 2 + s2
                q_sb = qpool.tile([128, CHW], F16, name="q_sb1", tag="q")
                nc.sync.dma_start(
                    out=q_sb, in_=hidT[gct * 128:(gct + 1) * 128, cols]
                )
                kq = kqpool.tile([128, CHW], F32R, name="kq1", tag="kq")
                ps = mmp.tile([128, CHW], F32, name="psC", tag="mm")
                for et in range(NET):
                    nc.tensor.matmul(
                        ps,
                        kw_t[:, et, s2 * 128:(s2 + 1) * 128],
                        emb_all[:, et, cols],
                        start=(et == 0), stop=(et == NET - 1),
                    )
                nc.scalar.activation(
                    ksqp[:, s2, :], ps, AF.Square,
                    bias=keyb_sb[:, gct:gct + 1], scale=1.0,
                )
                nc.gpsimd.tensor_mul(qsqp[:, s2, :], q_sb, q_sb)
                nc.vector.scalar_tensor_tensor(
                    kq, ps, keyb_sb[:, gct:gct + 1],
                    q_sb, op0=ALU.add, op1=ALU.mult,
                )
                kqs.append((gct, kq))
            return ksqp, qsqp, kqs

        def emit_dr1(stage, ksqp, qsqp, kqs, ch, last_gg):
            for gct, kq in kqs:
                sum_mm(stage, lkq_sb[:, gct, :], kq, ch)
            sum_mm(stage, lk8_sb[:, stage, :, :], ksqp, ch, perf_mode=DR)
            sum_mm(stage, lk8_sb[:, 4 + stage, :, :], qsqp, ch, last=last_gg,
                   perf_mode=DR)

        # ---------- stage D ----------
        # Shared rms_v normalizer: rms_v = sqrt(gate^2*mean(vproj^2)+eps)
        # ~= gate*sqrt(mean(vproj^2)+eps) since gate=sigmoid(..)>0, so the
        # conv input normed = vproj*alpha with ONE shared alpha row; only the
        # residual (value = vproj*gate) needs the per-group gate.
        def emit_alpha():
            # sv (= sum vproj^2) sits at psum row 12: bounce the block to
            # SBUF and matmul-extract the row to partition 0.
            s3a = rowm.tile([16, NTOK], F32R, name="s3a", tag="svz")
            aln = rowsc.tile([1, NTOK], F32, name="aln", tag="rs")
            alpha16 = rowm.tile([1, NTOK], F16, name="alpha16", tag="alpha16")
            nc.scalar.activation(s3a, sums_all[:, :], AF.Copy)
            for ch in range(NCH):
                cols = slice(ch * CHW, (ch + 1) * CHW)
                p = epsum.tile([1, CHW], F32, name="svx", tag="mm")
                nc.tensor.matmul(p, cepr_sb[0:16, 20:21], s3a[:, cols],
                                 start=True, stop=True)
                nc.scalar.activation(aln[:, cols], p, AF.Ln,
                                     bias=ceps_sb[0:1, 6:7],
                                     scale=1.0 / float(C))
            nc.scalar.activation(alpha16, aln, AF.Exp, scale=-0.5)
            return alpha16

        def make_d_tiles(stage):
            T = {}
            for nm in ("p4", "lnp", "lnd", "lng", "sqg", "sgn", "ss4", "ab4",
                       "akr"):
                T[nm] = rowsc.tile([1, NTOK], F32, name=f"{nm}{stage}",
                                   tag="rs")
            T["gate16"] = rowm.tile([1, NTOK], F16, name=f"gate16{stage}",
                                    tag="gate16")
            T["s3"] = rowm.tile([16, NTOK], F32R, name=f"s3_{stage}",
                                tag="ext")
            return T

        def emit_d_s3(stage, T, chs=(0, 1)):
            """Psum sums -> partition-0-based SBUF bounce (+ stage biases)."""
            for ch in chs:
                sl = slice(ch * CHW, (ch + 1) * CHW)
                nc.scalar.activation(T["s3"][:, sl], sums_all[:, sl],
                                     AF.Identity,
                                     bias=ceps_sb[0:16, stage:stage + 1],
                                     scale=1.0)

        def emit_d(stage, T, mul_eng=None, chs=(0, 1)):
            """Per-group gate chain: gate = sigmoid(sign(dot)*sqrt(|graw|)).

            Engines only address partitions at 32-boundaries, so the psum
            region is Act-copied (aligned base -> partition 0) to s3, and
            rows 1+ are pulled to partition-0 psum via one-hot matmuls.
            Row layout: stages 0-2: [ak, aq, dot]; stage 3: [sv, aq, dot, ak].
            """
            me = mul_eng if mul_eng is not None else nc.vector
            s3 = T["s3"]
            p4, lnp, lnd, lng, sqg, sgn, ss4, ab4, gate16 = (
                T["p4"], T["lnp"], T["lnd"], T["lng"], T["sqg"], T["sgn"],
                T["ss4"], T["ab4"], T["gate16"])
            akr = T["akr"]
            if chs == (0, 1):
                sls = [slice(0, NTOK)]
            else:
                sls = [slice(ch * CHW, (ch + 1) * CHW) for ch in chs]

            def extract(row, ch):
                sel = cepr_sb[0:16, 8 + row:9 + row]
                p = epsum.tile([1, CHW], F32, name=f"x{row}_{stage}",
                               tag="mm")
                nc.tensor.matmul(p, sel,
                                 s3[:, ch * CHW:(ch + 1) * CHW],
                                 start=True, stop=True)
                return p

            # first layer reads the [1, CHW] psums (partition 0), per chunk
            for ch in chs:
                cols = slice(ch * CHW, (ch + 1) * CHW)
                ak_ps = extract(stage, ch)
                aq_ps = extract(4 + stage, ch)
                dot_ps = extract(8 + stage, ch)
                nc.scalar.activation(akr[:, cols], ak_ps, AF.Copy)
                nc.scalar.activation(ab4[:, cols], dot_ps, AF.Square)
                nc.scalar.activation(sgn[:, cols], dot_ps, AF.Sign)
                nc.vector.tensor_mul(p4[:, cols], akr[:, cols], aq_ps)
            # 2ln|dot| and ln(p4/C); 2ln|graw| = 2ln|dot| - ln(p4/C)
            # (plain subtract so the mul engine can be Pool)
            for sl in sls:
                nc.scalar.activation(lnd[:, sl], ab4[:, sl], AF.Ln,
                                     bias=ceps_sb[0:1, 7:8])
            for sl in sls:
                nc.scalar.activation(lnp[:, sl], p4[:, sl], AF.Ln,
                                     scale=1.0 / float(C))
            for sl in sls:
                me.tensor_sub(lng[:, sl], lnd[:, sl], lnp[:, sl])
            for sl in sls:
                nc.scalar.activation(sqg[:, sl], lng[:, sl], AF.Exp,
                                     scale=0.25)
            for sl in sls:
                me.tensor_mul(ss4[:, sl], sqg[:, sl], sgn[:, sl])
            for sl in sls:
                nc.scalar.activation(gate16[:, sl], ss4[:, sl], AF.Sigmoid)
            return gate16

        # ---------- stage E ----------
        def bcast_ch(src, dst, ch):
            bp = epsum.tile([128, CHW], F32, name="bp", tag="mm")
            nc.tensor.matmul(
                bp, bc2_sb[0:1, 0:128],
                src[:, ch * CHW:(ch + 1) * CHW],
                start=True, stop=True,
            )
            nc.scalar.activation(
                dst[:, ch * CHW:(ch + 1) * CHW], bp, AF.Copy)

        def bcast_row(src, tag):
            """[1, NTOK] f32/f16 row -> [128, NTOK] f16 via PE broadcast."""
            dst = rowm.tile([128, NTOK], F16, name=f"b_{tag}", tag=tag)
            for ch in range(NCH):
                bcast_ch(src, dst, ch)
            return dst

        # nx16[ct]: f16 conv input, shared by all 4 groups' units:
        # [9 zeros | vproj*alpha]; the halo contribution to the first 9
        # outputs is a host-computed f16 correction (hc) accumulated via an
        # identity matmul.
        PADW = HALO + NTOK
        nx8s = {}

        def emit_nx8(ct):
            nx8 = npool.tile([128, PADW], F16, name=f"nx16_{ct}",
                             tag=f"nx16_{ct}", bufs=1)
            nc.gpsimd.memset(nx8[:, 0:HALO], 0.0)
            nc.vector.tensor_mul(nx8[:, HALO:HALO + NTOK],
                                 vproj16[:, ct, :], ab16)
            nx8s[ct] = nx8

        def emit_val(gct, gb16, on_pool=False):
            ct = gct % NCT
            val = vpool.tile([128, NTOK], F16, name="val", tag="val")
            if on_pool:
                nc.gpsimd.tensor_mul(val, vproj16[:, ct, :], gb16)
            else:
                nc.vector.tensor_mul(val, vproj16[:, ct, :], gb16)
            return val

        def emit_e_conv_pe(gct, pools=None):
            """f16 conv taps + halo-fix matmul."""
            ct = gct % NCT
            nx8 = nx8s[ct]
            dg_t = dgpool.tile([128, KT * 128], F16, name="dg_t", tag="dg")
            nc.sync.dma_start(out=dg_t, in_=dg16[gct])
            accs = []
            for ch in range(NCH):
                pool = (pools[ch % len(pools)] if pools else epsum)
                acc = pool.tile([128, CHW], F32, name="acc", tag="mm")
                for k in range(KT):
                    base = ch * CHW + k * DIL
                    nc.tensor.matmul(
                        acc,
                        dg_t[:, k * 128:(k + 1) * 128],
                        nx8[:, base:base + CHW],
                        start=(k == 0), stop=(k == KT - 1 and ch == 1),
                        skip_group_check=True,
                    )
                if ch == 0:
                    nc.tensor.matmul(
                        acc[:, 0:HALO], id16_sb, hc_sb[:, gct, :],
                        start=False, stop=True, skip_group_check=True,
                    )
                accs.append(acc)
            return accs

        def emit_e_conv_dve(gct):
            """f16 conv as DVE scalar-ptr MAC chains (+ in-place halo fix)."""
            ct = gct % NCT
            nx8 = nx8s[ct]
            outs = []
            for ch in range(NCH):
                prev = None
                for k in range(KT):
                    win = nx8[:, ch * CHW + k * DIL:ch * CHW + k * DIL + CHW]
                    a = cacc.tile([128, CHW], F16, name=f"ca{k}", tag=f"ca{k}")
                    wcol = cwf_sb[:, gct * KT + k:gct * KT + k + 1]
                    if k == 0:
                        nc.vector.tensor_scalar_mul(a, win, wcol)
                    else:
                        nc.vector.scalar_tensor_tensor(
                            a, win, wcol, prev, op0=ALU.mult, op1=ALU.add)
                    prev = a
                if ch == 0:
                    nc.vector.tensor_tensor(prev[:, 0:HALO], prev[:, 0:HALO],
                                            hc_sb[:, gct, :], op=ALU.add)
                outs.append(prev)
            return outs

        def emit_silu(accs):
            sacc = opool.tile([128, NTOK], F16, name="sacc", tag="sacc")
            for ch in range(NCH):
                nc.scalar.activation(sacc[:, ch * CHW:(ch + 1) * CHW],
                                     accs[ch], AF.Silu)
            return sacc

        def emit_resid_out(gct, val, sacc, engine="pool"):
            ot = opool.tile([128, NTOK], F16, name="ot", tag="ot")
            if engine == "dve":
                nc.vector.tensor_tensor(ot, val, sacc, op=ALU.add)
            else:
                nc.gpsimd.tensor_add(ot, val, sacc)
            nc.sync.dma_start(out=out_d[gct * 128:(gct + 1) * 128, :], in_=ot)

        # ---------- pipeline ----------
        # conv+silu only needs the shared ab16; val/resid needs gate(g).
        # Window g: C(g) + chain(g-1) + full units of group g-1 + a few
        # group-3 conv units pulled early; tail: 3 conv units cover chain(3),
        # then group-3 val/resid.
        sacc3 = {}      # gct -> long-lived sacc for group-3 units
        ab16 = None
        TAIL3 = [29, 30, 31]
        EARLY3 = {0: [24, 25, 26], 1: [27], 2: [], 3: []}

        def conv_unit(u, long_lived=False, pools=None, defer_silu=False,
                      dve=False):
            if dve:
                accs = emit_e_conv_dve(u)
            else:
                accs = emit_e_conv_pe(u, pools=pools)
            if defer_silu:
                return accs
            if long_lived:
                sacc = opool.tile([128, NTOK], F16, name=f"sacc{u}",
                                  tag=f"sacc3_{u}", bufs=1)
            else:
                sacc = opool.tile([128, NTOK], F16, name=f"sacc{u}",
                                  tag="sacc")
            for ch in range(NCH):
                nc.scalar.activation(sacc[:, ch * CHW:(ch + 1) * CHW],
                                     accs[ch], AF.Silu)
            if long_lived:
                sacc3[u] = sacc
            return sacc

        def full_unit(u, gb16, dve=False):
            sacc = conv_unit(u, dve=dve)
            val = emit_val(u, gb16)
            emit_resid_out(u, val, sacc, engine="pool" if dve else "dve")

        gate_prev = None
        d_tiles = {}
        for g in range(3):
            dr_prev = None
            gb16 = None
            units = list(range((g - 1) * 8, g * 8)) if g else []
            for i, gg in enumerate(range(g * 4, (g + 1) * 4)):
                if i == 0 and g:
                    d_tiles[g - 1] = make_d_tiles(g - 1)
                    emit_d_s3(g - 1, d_tiles[g - 1])
                cur = emit_c_kq(g, gg)
                if i == 0:
                    if g == 0:
                        alpha16 = emit_alpha()
                    else:
                        gate_prev = emit_d(g - 1, d_tiles[g - 1],
                                           mul_eng=nc.gpsimd)
                if dr_prev is not None:
                    emit_dr(g, *dr_prev, last_gg=False)
                dr_prev = cur
                if i == 1:
                    if g == 0:
                        ab16 = bcast_row(alpha16, "ab16")
                        for ct in (0, 1, 2):
                            emit_nx8(ct)
                    elif g == 1:
                        for ct in (6, 7):
                            emit_nx8(ct)
                    batch = []
                elif i == 2:
                    if g:
                        gb16 = bcast_row(gate_prev, f"gb{g - 1}")
                        batch = units[0:3]
                    else:
                        emit_nx8(3)
                        batch = EARLY3[0][0:2]
                elif i == 3:
                    if g == 0:
                        emit_nx8(4)
                        emit_nx8(5)
                    batch = units[3:6] if g else EARLY3[0][2:3]
                else:
                    batch = []
                for u in batch:
                    if g:
                        full_unit(u, gb16, dve=(u % 8 in (0, 2, 4)))
                    else:
                        conv_unit(u, long_lived=True, dve=True)
            emit_dr(g, *dr_prev, last_gg=True)
            if g:
                for u in units[6:8]:
                    full_unit(u, gb16, dve=(u % 8 == 6))
                for u in EARLY3[g]:
                    conv_unit(u, long_lived=True)

        # ---------- window 3: chunk-split passes ----------
        # pass p computes C(3) for token chunk p only, so the stage-3 gate
        # chain + group-3 epilogue for chunk 0 hide under pass 1.
        units = list(range(16, 24))
        d_tiles[2] = make_d_tiles(2)
        emit_d_s3(2, d_tiles[2])
        T3 = None
        gb3 = rowm.tile([128, NTOK], F16, name="b_gb3", tag="gb3")

        def epi3_ch(u, ch):
            ct = u % NCT
            cols = slice(ch * CHW, (ch + 1) * CHW)
            val = vpool.tile([128, CHW], F16, name="val3", tag="val")
            nc.vector.tensor_mul(val, vproj16[:, ct, cols], gb3[:, cols])
            ot = opool.tile([128, CHW], F16, name="ot3", tag="ot")
            nc.vector.tensor_tensor(ot, val, sacc3[u][:, cols], op=ALU.add)
            nc.sync.dma_start(out=out_d[u * 128:(u + 1) * 128, cols], in_=ot)

        for p in range(2):
            dr_prev = None
            for i, gg in enumerate(range(12, 16)):
                cur = emit_c_kq1(3, gg, p)
                if p == 0 and i == 0:
                    gate2 = emit_d(2, d_tiles[2], mul_eng=nc.gpsimd)
                if p == 1 and i == 0:
                    T3 = make_d_tiles(3)
                    emit_d_s3(3, T3, chs=(0,))
                    gate3 = emit_d(3, T3, mul_eng=nc.vector, chs=(0,))
                if dr_prev is not None:
                    emit_dr1(3, *dr_prev, p, last_gg=False)
                dr_prev = cur
                if p == 0:
                    if i == 2:
                        gb2 = bcast_row(gate2, "gb2")
                        batch = units[0:3]
                    elif i == 3:
                        batch = units[3:6]
                    else:
                        batch = []
                    for u in batch:
                        full_unit(u, gb2, dve=(u % 8 in (0, 2, 4)))
                else:
                    if i == 1:
                        bcast_ch(gate3, gb3, 0)
                        for u in units[6:8]:
                            full_unit(u, gb2, dve=(u % 8 == 6))
                    elif i == 2:
                        for u in range(24, 28):
                            epi3_ch(u, 0)
                    elif i == 3:
                        pass
            emit_dr1(3, *dr_prev, p, last_gg=True)

        # ---------- tail: chunk 1 of the group-3 gate + epilogue ----------
        # TAIL3 conv matmuls cover the chain; their silus follow its Act ops
        emit_d_s3(3, T3, chs=(1,))
        acc_pools = [epsum, mmp]
        emit_d(3, T3, mul_eng=nc.vector, chs=(1,))
        TAIL4 = [28] + TAIL3
        tail_accs = [conv_unit(u, pools=acc_pools, defer_silu=True)
                     for u in TAIL4]
        bcast_ch(gate3, gb3, 1)
        for j, u in enumerate(TAIL4):
            sacc = opool.tile([128, NTOK], F16, name=f"sacc{u}",
                              tag=f"sacc3_{u}", bufs=1)
            for ch in range(NCH):
                nc.scalar.activation(sacc[:, ch * CHW:(ch + 1) * CHW],
                                     tail_accs[j][ch], AF.Silu)
            sacc3[u] = sacc
        for u in TAIL4:
            epi3_ch(u, 0)
        for u in range(24, 32):
            epi3_ch(u, 1)

        for p in (cacc, dgpool, opool, vpool, npool, rowsc, rowm, kqpool, scr,
                  epsum, sump, mmp, qpool, kwpool, consts):
            p.release()
    return nc


def host_prep(embeddings, hidden_states, key_w, key_b, value_w, value_b,
              w_key_norm, w_query_norm, w_norm, conv_weight):
    """Build the per-core input maps."""
    f32, f16 = np.float32, np.float16
    e4 = ml_dtypes.float8_e4m3fn
    embeddings = np.asarray(embeddings, f32)
    hidden_states = np.asarray(hidden_states, f32)
    key_w = np.asarray(key_w, f32)
    key_b = np.asarray(key_b, f32)
    value_w = np.asarray(value_w, f32)
    value_b = np.asarray(value_b, f32)
    w_key_norm = np.asarray(w_key_norm, f32)
    w_query_norm = np.asarray(w_query_norm, f32)
    w_norm = np.asarray(w_norm, f32)
    conv_weight = np.asarray(conv_weight, f32)

    kwT = np.ascontiguousarray(key_w.T).astype(f16)        # [E, GC]
    vwT = np.ascontiguousarray(value_w.T).astype(f16)      # [E, C]
    keyb_r = np.ascontiguousarray(key_b.reshape(NGCT, 128).T)  # [128, NGCT]
    valb_r = np.ascontiguousarray(value_b.reshape(NCT, 128).T)
    wkq = (w_key_norm * w_query_norm).reshape(GC)

    # one-hot lhsT tables. ONE shared [16, NTOK] psum accumulator with
    # disjoint rows: ak_g = row g, aq_g = 4+g, dot_g = 8+g, sv = 12.
    # (engines can only address 32-aligned partition bases, so rows are
    #  matmul-extracted after an Act bounce of the block to partition 0)
    lk8 = np.zeros((128, 9, 2, 16), f32)
    for g in range(G):
        lk8[:, g, :, g] = 1.0          # ksq -> row g
        lk8[:, 4 + g, :, 4 + g] = 1.0  # qsq -> row 4+g
    lk8[:, 8, :, 12] = 1.0             # vsq -> row 12 (sv)
    lk8 = lk8.reshape(128, 288).astype(e4)

    lkq = np.zeros((NGCT, 128, 16), f32)
    for gct in range(NGCT):
        g = gct // NCT
        lkq[gct, :, 8 + g] = wkq[gct * 128:(gct + 1) * 128]

    aux16 = np.zeros((128, 16 + 256), f16)
    aux16[:, 12] = 1.0        # lv one-hot: vsq -> row 12 (sv)
    for j in range(2):
        aux16[j, 16 + j * 128:16 + (j + 1) * 128] = 1.0

    # ceps: cols 0-3 = per-stage bias vectors (+C*EPS on ak/aq rows);
    #        cols 8+r = f32 one-hot row selectors (identity)
    ceps_h = np.zeros((128, 24), f32)
    for g in range(G):
        ceps_h[g, g] = float(C) * EPS
        ceps_h[4 + g, g] = float(C) * EPS
    for r in range(16):
        ceps_h[r, 8 + r] = 1.0
    ceps_h[0, 6] = NORM_EPS
    ceps_h[0, 7] = 1e-60

    # f16 diagonal conv weights + identity for the halo-fix matmul.
    cwf = (conv_weight.reshape(G, C, KT) * w_norm[:, :, None]).astype(f32)
    dg = np.zeros((NGCT, 128, KT * 128), f16)
    idx = np.arange(128)
    for gct in range(NGCT):
        g, ct = gct // NCT, gct % NCT
        for k in range(KT):
            dg[gct, idx, k * 128 + idx] = cwf[g, ct * 128 + idx, k].astype(f16)
    id16_h = np.zeros((128, 128), f16)
    id16_h[idx, idx] = 1.0
    cwf_r = np.zeros((128, NGCT * KT), f32)
    for gct in range(NGCT):
        g, ct = gct // NCT, gct % NCT
        for k in range(KT):
            cwf_r[:, gct * KT + k] = cwf[g, ct * 128:(ct + 1) * 128, k]

    in_maps = []
    for core in range(NCORES):
        b = core // (NCORES // B)
        t0 = (core % (NCORES // B)) * NTOK
        emb_s = embeddings[b, t0:t0 + NTOK]                # [NTOK, E]
        hid_s = hidden_states[b, t0:t0 + NTOK].reshape(NTOK, GC)
        emb_c = np.ascontiguousarray(emb_s.T).astype(f16)  # [E, NTOK]
        hid_c = np.ascontiguousarray(hid_s.T).astype(f16)  # [GC, NTOK]

        # halo: nhat (= value / rms_v, w_norm NOT applied) for the 9
        # preceding tokens feeds a host-computed conv correction hc for the
        # first 9 output tokens; zeros at the sequence start.
        if t0 == 0:
            hc_c = np.zeros((128, NGCT * HALO), f16)
        else:
            th = slice(t0 - HALO, t0)
            e9 = embeddings[b, th]                          # [9, E]
            k9 = (e9 @ key_w.T + key_b).reshape(HALO, G, C)
            q9 = hidden_states[b, th]                       # [9, G, C]
            rk = np.sqrt((k9 * k9).mean(-1) + EPS)
            rq = np.sqrt((q9 * q9).mean(-1) + EPS)
            d9 = np.einsum("tgc,gc,tgc,gc->tg", k9, w_key_norm, q9,
                           w_query_norm)
            graw = d9 / (rk * rq) / np.sqrt(f32(C))
            g9 = 1.0 / (1.0 + np.exp(-(np.where(graw >= 0, 1.0, -1.0)
                                       * np.sqrt(np.maximum(np.abs(graw),
                                                            1e-6)))))
            vp9 = e9 @ value_w.T + value_b                  # [9, C]
            val9 = vp9[:, None, :] * g9[..., None].astype(f32)
            rv9 = np.sqrt((val9 * val9).mean(-1) + NORM_EPS)
            nhat9 = val9 / rv9[..., None]                   # [9, G, C]
            # hc[c, gct, t] = sum_{k: t+k*DIL<9} cwf[g,c,k]*nhat9[t+k*DIL,g,c]
            hcf = np.zeros((HALO, G, C), f32)
            for t in range(HALO):
                for k in range(KT):
                    ix = t + k * DIL
                    if ix < HALO:
                        hcf[t] += cwf[:, :, k] * nhat9[ix]
            hg = hcf.transpose(1, 2, 0).reshape(NGCT, 128, HALO)
            hc_c = np.ascontiguousarray(
                hg.transpose(1, 0, 2).reshape(128, NGCT * HALO)).astype(f16)

        in_maps.append({
            "emb16": emb_c, "hidT": hid_c, "kwT": kwT, "vwT": vwT,
            "keyb": keyb_r, "valb": valb_r,
            "lk8": lk8, "lkq": lkq, "aux16": aux16, "ceps": ceps_h,
            "dg16": dg, "cwf": cwf_r, "hc": hc_c, "id16": id16_h,
        })
    return in_maps


_NC_CACHE = [None]
LAST_RESULT = [None]


def kernel(**inputs) -> np.ndarray:
    in_maps = host_prep(**inputs)
    if _NC_CACHE[0] is None:
        _NC_CACHE[0] = build_program()
    nc = _NC_CACHE[0]
    res = run_bass_kernel_spmd(nc, in_maps, list(range(NCORES)))
    LAST_RESULT[0] = res
    out = np.empty((B, T, G, C), np.float32)
    for core in range(NCORES):
        b = core // (NCORES // B)
        t0 = (core % (NCORES // B)) * NTOK
        oc = np.asarray(res.results[core]["out"]).astype(np.float32)
        out[b, t0:t0 + NTOK] = oc.reshape(G, C, NTOK).transpose(2, 0, 1)
    return out


# revision 126
# speedup vs baseline: 1.0134x; 1.0009x over previous
"""Trainium2 Bass kernel for the EngramNew module (dense_cnn), v3.

Sharding: B*T = 8192 tokens split across 8 cores (1024 tokens each); the conv
halo of (K-1)*DIL = 9 tokens is precomputed host-side.  On-device layout is
channels-on-partitions / tokens-on-free: [G*C, T_core].

v10 design vs the v2 baseline (291.8us -> 225.1us):
 - shared rms_v normalizer: rms_v = sqrt(gate^2*mean(vproj^2)+eps)
   ~= gate*sqrt(mean(vproj^2)+eps) since gate = sigmoid(..) > 0, so the
   conv input (normed) = vproj*alpha with ONE shared alpha row; only the
   residual needs the per-group gate.  This decouples the whole conv
   pipeline from the gate chains (error <= ~1e-3, verified vs reference).
 - ONE [16,NTOK] PSUM accumulator shared by all four gate-sum stages via
   disjoint one-hot rows (ak_g=g, aq_g=4+g, dot_g=8+g, sv=12), reset once:
   no PSUM bank-rotation WAR stalls.  Rows are bounced to partition 0 by
   an Act copy + f32r one-hot extraction matmuls (engines can only address
   32-aligned partition bases).
 - per-ct conv input tiles (zero prefix + vproj*alpha) shared by all 4
   groups; the halo contribution to the first 9 outputs is a host-computed
   correction added via an identity matmul.  15 of 32 conv units run as
   DVE MAC chains, the rest as PE diag matmuls.
 - C(3) is split into two per-chunk passes so chunk 0 of the stage-3 gate
   chain + epilogue hides under the chunk-1 pass; 3 conv units are held
   back as PE cover for the chunk-1 chain.
 - kq / ksq+qsq(fp8 DR) / vsq(fp8 DR) reductions all deferred by one
   gg/vv so their producers never stall the PE sum matmuls.
 - startup: split vw/emb DMAs ordered first + 4-psum et-outer first vproj
   pass so PE starts at ~4us instead of 13us.
"""

import os
import sys

for _p in ("/opt/trn_rl_repo",):
    if _p not in sys.path:
        sys.path.insert(0, _p)

import numpy as np
import ml_dtypes

import concourse.bass as bass
from concourse import mybir
from concourse.tile import TileContext
from concourse.bass_utils import run_bass_kernel_spmd
import bass_rust

F32 = mybir.dt.float32
F32R = mybir.dt.float32r
F16 = mybir.dt.float16
FP8 = mybir.dt.float8e4
AF = mybir.ActivationFunctionType
ALU = mybir.AluOpType
DR = mybir.MatmulPerfMode.DoubleRow

# Problem constants (hardcoded per spec nn_EngramNew_2070174237244)
B, T, G, C, E = 2, 4096, 4, 1024, 1024
GC = G * C
KT, DIL = 4, 3          # conv taps / dilation
EPS = 1e-5
NORM_EPS = 1e-5
NCORES = 8
NTOK = (B * T) // NCORES    # 1024 tokens per core
HALO = (KT - 1) * DIL       # 9
NET = E // 128              # 8 e-tiles
NGCT = GC // 128            # 32 gc-tiles
NCT = C // 128              # 8 c-tiles
CHW = 512                   # token chunk width (1 PSUM bank of fp32)
NCH = NTOK // CHW           # 2 chunks



class PatchedTileContext(TileContext):
    """This walrus build allows only one sem wait per instruction (two on
    EventSemaphore). Tile attaches as many waits as an instruction needs,
    so after scheduling we hoist excess waits onto no-op instructions
    inserted just before the owner on the same engine (engines are strict
    FIFO, so observing the sems earlier is equivalent)."""

    def _split_excess_waits(self):
        nc = self.nc

        def make_nop(engine):
            bi = nc.engines[engine].nop()
            bb = nc.cur_bb.bb
            lst = list(bb.instructions)
            assert lst[-1] is bi.ins
            bb.instructions = lst[:-1]
            return bi.ins

        # Phase 1: snapshot every block BEFORE creating any nop, so nops
        # appended to cur_bb can never leak into the iteration or the rebuilt
        # lists (cur_bb may be one of the blocks being processed).
        snapshots = []
        for f in nc.m.functions:
            for blk in f.blocks:
                snapshots.append((blk, list(blk.instructions)))

        for blk, insts in snapshots:
            out = []
            changed = False
            for ins in insts:
                si = ins.sync_info
                waits = list(si.on_wait) if (si and si.on_wait) else []
                cap = 2 if isinstance(ins, mybir.InstEventSemaphore) else 1
                if len(waits) > cap:
                    changed = True
                    for w in waits[cap:]:
                        nop = make_nop(ins.engine)
                        nop.sync_info = bass_rust.SyncInfo(
                            on_wait=[w], on_update=[]
                        )
                        out.append(nop)
                    upd = list(si.on_update) if si.on_update else []
                    ins.sync_info = bass_rust.SyncInfo(
                        on_wait=waits[:cap], on_update=upd
                    )
                out.append(ins)
            if changed:
                blk.instructions = out

    def _drain_and_barrier(self, tick_clock, wait_clock):
        super()._drain_and_barrier(tick_clock, wait_clock)
        self._split_excess_waits()


def _r(ap):
    return ap.bitcast(F32R)


def build_program():
    nc = bass.Bass()

    # ---- DRAM parameters ----
    emb16 = nc.declare_dram_parameter("emb16", [E, NTOK], F16, isOutput=False)
    hidT = nc.declare_dram_parameter("hidT", [GC, NTOK], F16, isOutput=False)
    kwT = nc.declare_dram_parameter("kwT", [E, GC], F16, isOutput=False)
    vwT = nc.declare_dram_parameter("vwT", [E, C], F16, isOutput=False)
    keyb = nc.declare_dram_parameter("keyb", [128, NGCT], F32, isOutput=False)
    valb = nc.declare_dram_parameter("valb", [128, NCT], F32, isOutput=False)
    lk8 = nc.declare_dram_parameter("lk8", [128, 9 * 2 * 16], FP8,
                                    isOutput=False)
    lkq = nc.declare_dram_parameter("lkq", [NGCT, 128, 16], F32, isOutput=False)
    aux16 = nc.declare_dram_parameter("aux16", [128, 16 + 256], F16,
                                      isOutput=False)
    ceps = nc.declare_dram_parameter("ceps", [128, 24], F32, isOutput=False)
    dg16 = nc.declare_dram_parameter("dg16", [NGCT, 128, KT * 128], F16,
                                     isOutput=False)
    cwf = nc.declare_dram_parameter("cwf", [128, NGCT * KT], F32,
                                    isOutput=False)
    hc = nc.declare_dram_parameter("hc", [128, NGCT * HALO], F16,
                                   isOutput=False)
    id16 = nc.declare_dram_parameter("id16", [128, 128], F16, isOutput=False)
    out_d = nc.declare_dram_parameter("out", [GC, NTOK], F16, isOutput=True)

    with PatchedTileContext(nc) as tc:
        consts = tc.alloc_tile_pool(name="consts", bufs=1)
        kwpool = tc.alloc_tile_pool(name="kwpool", bufs=2)
        qpool = tc.alloc_tile_pool(name="qpool", bufs=3)
        mmp = tc.alloc_tile_pool(name="mmp", bufs=4, space=bass.MemorySpace.PSUM)
        sump = tc.alloc_tile_pool(name="sump", bufs=1, space=bass.MemorySpace.PSUM)
        epsum = tc.alloc_tile_pool(name="epsum", bufs=2,
                                   space=bass.MemorySpace.PSUM)
        scr = tc.alloc_tile_pool(name="scr", bufs=4)
        kqpool = tc.alloc_tile_pool(name="kqpool", bufs=4)
        rowm = tc.alloc_tile_pool(name="rowm", bufs=1)
        rowsc = tc.alloc_tile_pool(name="rowsc", bufs=9)
        npool = tc.alloc_tile_pool(name="npool", bufs=3)
        vpool = tc.alloc_tile_pool(name="vpool", bufs=4)
        opool = tc.alloc_tile_pool(name="opool", bufs=5)
        dgpool = tc.alloc_tile_pool(name="dgpool", bufs=3)
        cacc = tc.alloc_tile_pool(name="cacc", bufs=3)

        # ---- load order: vw(vv0) first, then emb per-et, then small consts
        vw_t0 = kwpool.tile([128, NET, 256], F16, name="vw_t0", tag="w")
        for eh in range(2):
            nc.sync.dma_start(
                out=vw_t0[:, eh * 4:(eh + 1) * 4, :],
                in_=vwT.rearrange("(et p) c -> p et c", p=128)[
                    :, eh * 4:(eh + 1) * 4, 0:256],
            )
        emb_all = consts.tile([128, NET, NTOK], F16)
        for et in range(NET):
            nc.sync.dma_start(out=emb_all[:, et, :],
                              in_=emb16[et * 128:(et + 1) * 128, :])
        vw_t1 = kwpool.tile([128, NET, 256], F16, name="vw_t1", tag="w")
        nc.sync.dma_start(
            out=vw_t1,
            in_=vwT.rearrange("(et p) c -> p et c", p=128)[:, :, 256:512],
        )
        valb_sb = consts.tile([128, NCT], F32)
        nc.sync.dma_start(out=valb_sb, in_=valb[:, :])
        aux_sb = consts.tile([128, 16 + 256], F16)
        nc.sync.dma_start(out=aux_sb, in_=aux16[:, :])
        ceps_sb = consts.tile([128, 24], F32)
        nc.sync.dma_start(out=ceps_sb, in_=ceps[:, :])
        cepr_sb = consts.tile([128, 24], F32R)
        nc.sync.dma_start(out=cepr_sb, in_=_r(ceps[:, :]))
        keyb_sb = consts.tile([128, NGCT], F32)
        nc.sync.dma_start(out=keyb_sb, in_=keyb[:, :])
        lk8_sb = consts.tile([128, 9, 2, 16], FP8)
        nc.sync.dma_start(out=lk8_sb,
                          in_=lk8.rearrange("p (q i c) -> p q i c", i=2, c=16))
        lkq_sb = consts.tile([128, NGCT, 16], F32R)
        nc.sync.dma_start(out=lkq_sb, in_=_r(lkq.rearrange("n p m -> p n m")))
        cwf_sb = consts.tile([128, NGCT * KT], F32)
        nc.sync.dma_start(out=cwf_sb, in_=cwf[:, :])
        hc_sb = consts.tile([128, NGCT, HALO], F16)
        nc.sync.dma_start(out=hc_sb,
                          in_=hc.rearrange("p (n h) -> p n h", h=HALO))
        id16_sb = consts.tile([128, 128], F16)
        nc.sync.dma_start(out=id16_sb, in_=id16[:, :])
        vproj16 = consts.tile([128, NCT, NTOK], F16)
        bc2_sb = aux_sb[0:1, 16:16 + 128]

        # ---- gate sums: ONE [16, NTOK] psum shared by all stages via
        # disjoint one-hot rows: ak_g = row g, aq_g = 4+g, dot_g = 8+g,
        # sv = 12. Reset once (B's first vsq sum); everything accumulates.
        sums_all = sump.tile([16, NTOK], F32, name="sums_all", tag="sums")
        first_sum = [True] * NCH

        def sum_mm(stage, lhsT, rhs, ch, last=False, perf_mode=None):
            st = first_sum[ch]
            first_sum[ch] = False
            nc.tensor.matmul(
                sums_all[:, ch * CHW:(ch + 1) * CHW],
                lhsT, rhs, start=st, stop=last,
                perf_mode=perf_mode, skip_group_check=True,
            )

        # ---------- stage B: vproj = value_w @ emb + value_b ----------
        # vsq in fp8 (feeds only alpha), DoubleRow-reduced, deferred one vv
        pend_vsq = None

        def flush_vsq(v8):
            for ch in range(NCH):
                cols = slice(ch * CHW, (ch + 1) * CHW)
                sum_mm(3, lk8_sb[:, 8, :, :], v8[:, :, cols], ch,
                       perf_mode=DR)

        for vv in range(NCT // 2):
            if vv == 0:
                vw_t = vw_t0
            elif vv == 1:
                vw_t = vw_t1
            else:
                vw_t = kwpool.tile([128, NET, 256], F16, name="vw_t", tag="w")
                nc.sync.dma_start(
                    out=vw_t,
                    in_=vwT.rearrange("(et p) c -> p et c", p=128)[
                        :, :, vv * 256:(vv + 1) * 256],
                )
            vsq = scr.tile([128, 2, NTOK], FP8, name="vsq8", tag="p8")
            if vv == 0:
                # et-outer across 4 psums so PE rate-matches the emb DMAs
                ps4 = [mmp.tile([128, CHW], F32, name=f"psB0_{i}", tag="mm")
                       for i in range(3)]
                ps4.append(epsum.tile([128, CHW], F32, name="psB0_3",
                                      tag="mm"))
                for et in range(NET):
                    for i in range(4):
                        s2, ch = i // 2, i % 2
                        nc.tensor.matmul(
                            ps4[i],
                            vw_t[:, et, s2 * 128:(s2 + 1) * 128],
                            emb_all[:, et, ch * CHW:(ch + 1) * CHW],
                            start=(et == 0), stop=(et == NET - 1),
                        )
                for i in range(4):
                    s2, ch = i // 2, i % 2
                    ct = vv * 2 + s2
                    cols = slice(ch * CHW, (ch + 1) * CHW)
                    nc.scalar.activation(
                        vproj16[:, ct, cols], ps4[i],
                        AF.Identity, bias=valb_sb[:, ct:ct + 1], scale=1.0,
                    )
                    nc.scalar.activation(
                        vsq[:, s2, cols], ps4[i], AF.Square,
                        bias=valb_sb[:, ct:ct + 1], scale=1.0,
                    )
            else:
                for s2 in range(2):
                    ct = vv * 2 + s2
                    for ch in range(NCH):
                        cols = slice(ch * CHW, (ch + 1) * CHW)
                        ps = mmp.tile([128, CHW], F32, name="psB", tag="mm")
                        for et in range(NET):
                            nc.tensor.matmul(
                                ps,
                                vw_t[:, et, s2 * 128:(s2 + 1) * 128],
                                emb_all[:, et, ch * CHW:(ch + 1) * CHW],
                                start=(et == 0), stop=(et == NET - 1),
                            )
                        nc.scalar.activation(
                            vproj16[:, ct, cols], ps,
                            AF.Identity, bias=valb_sb[:, ct:ct + 1], scale=1.0,
                        )
                        nc.scalar.activation(
                            vsq[:, s2, cols], ps, AF.Square,
                            bias=valb_sb[:, ct:ct + 1], scale=1.0,
                        )
                if pend_vsq is not None:
                    flush_vsq(pend_vsq)
                    pend_vsq = None
            pend_vsq = vsq
        flush_vsq(pend_vsq)

        # ---------- stage C for one group-pair ----------
        def emit_c_kq(stage, gg):
            """k path for double-gct gg (two gc tiles); DR sums deferred."""
            kw_t = kwpool.tile([128, NET, 256], F16, name="kw_t", tag="w")
            for eh in range(2):
                nc.sync.dma_start(
                    out=kw_t[:, eh * 4:(eh + 1) * 4, :],
                    in_=kwT.rearrange("(et p) c -> p et c", p=128)[
                        :, eh * 4:(eh + 1) * 4, gg * 256:(gg + 1) * 256],
                )
            ksqp = scr.tile([128, 2, NTOK], FP8, name="ksqp", tag="p8")
            qsqp = scr.tile([128, 2, NTOK], FP8, name="qsqp", tag="p8")
            kqs = []
            for s2 in range(2):
                gct = gg * 2 + s2
                q_sb = qpool.tile([128, NTOK], F16, name="q_sb", tag="q")
                nc.sync.dma_start(
                    out=q_sb, in_=hidT[gct * 128:(gct + 1) * 128, :]
                )
                kq = kqpool.tile([128, NTOK], F32R, name="kq", tag="kq")
                for ch in range(NCH):
                    ps = mmp.tile([128, CHW], F32, name="psC", tag="mm")
                    for et in range(NET):
                        nc.tensor.matmul(
                            ps,
                            kw_t[:, et, s2 * 128:(s2 + 1) * 128],
                            emb_all[:, et, ch * CHW:(ch + 1) * CHW],
                            start=(et == 0), stop=(et == NET - 1),
                        )
                    cols = slice(ch * CHW, (ch + 1) * CHW)
                    nc.scalar.activation(
                        ksqp[:, s2, cols], ps, AF.Square,
                        bias=keyb_sb[:, gct:gct + 1], scale=1.0,
                    )
                    nc.gpsimd.tensor_mul(qsqp[:, s2, cols], q_sb[:, cols],
                                         q_sb[:, cols])
                    nc.vector.scalar_tensor_tensor(
                        kq[:, cols], ps, keyb_sb[:, gct:gct + 1],
                        q_sb[:, cols], op0=ALU.add, op1=ALU.mult,
                    )
                kqs.append((gct, kq))
            return ksqp, qsqp, kqs

        def emit_dr(stage, ksqp, qsqp, kqs, last_gg):
            for gct, kq in kqs:
                for ch in range(NCH):
                    sum_mm(stage, lkq_sb[:, gct, :],
                           kq[:, ch * CHW:(ch + 1) * CHW], ch)
            for ch in range(NCH):
                cols = slice(ch * CHW, (ch + 1) * CHW)
                sum_mm(stage, lk8_sb[:, stage, :, :], ksqp[:, :, cols], ch,
                       perf_mode=DR)
                sum_mm(stage, lk8_sb[:, 4 + stage, :, :], qsqp[:, :, cols],
                       ch, last=last_gg, perf_mode=DR)

        def emit_c_kq1(stage, gg, ch):
            """Single-chunk variant (window-3 ch-split passes)."""
            cols = slice(ch * CHW, (ch + 1) * CHW)
            kw_t = kwpool.tile([128, NET, 256], F16, name="kw_t", tag="w")
            for eh in range(2):
                nc.sync.dma_start(
                    out=kw_t[:, eh * 4:(eh + 1) * 4, :],
                    in_=kwT.rearrange("(et p) c -> p et c", p=128)[
                        :, eh * 4:(eh + 1) * 4, gg * 256:(gg + 1) * 256],
                )
            ksqp = scr.tile([128, 2, CHW], FP8, name="ksqp1", tag="p8")
            qsqp = scr.tile([128, 2, CHW], FP8, name="qsqp1", tag="p8")
            kqs = []
            for s2 in range(2):
                gct = gg * 2 + s2
                q_sb = qpool.tile([128, CHW], F16, name="q_sb1", tag="q")
                nc.sync.dma_start(
                    out=q_sb, in_=hidT[gct * 128:(gct + 1) * 128, cols]
                )
                kq = kqpool.tile([128, CHW], F32R, name="kq1", tag="kq")
                ps = mmp.tile([128, CHW], F32, name="psC", tag="mm")
                for et in range(NET):
                    nc.tensor.matmul(
                        ps,
                        kw_t[:, et, s2 * 128:(s2 + 1) * 128],
                        emb_all[:, et, cols],
                        start=(et == 0), stop=(et == NET - 1),
                    )
                nc.scalar.activation(
                    ksqp[:, s2, :], ps, AF.Square,
                    bias=keyb_sb[:, gct:gct + 1], scale=1.0,
                )
                nc.gpsimd.tensor_mul(qsqp[:, s2, :], q_sb, q_sb)
                nc.vector.scalar_tensor_tensor(
                    kq, ps, keyb_sb[:, gct:gct + 1],
                    q_sb, op0=ALU.add, op1=ALU.mult,
                )
                kqs.append((gct, kq))
            return ksqp, qsqp, kqs

        def emit_dr1(stage, ksqp, qsqp, kqs, ch, last_gg):
            for gct, kq in kqs:
                sum_mm(stage, lkq_sb[:, gct, :], kq, ch)
            sum_mm(stage, lk8_sb[:, stage, :, :], ksqp, ch, perf_mode=DR)
            sum_mm(stage, lk8_sb[:, 4 + stage, :, :], qsqp, ch, last=last_gg,
                   perf_mode=DR)

        # ---------- stage D ----------
        # Shared rms_v normalizer: rms_v = sqrt(gate^2*mean(vproj^2)+eps)
        # ~= gate*sqrt(mean(vproj^2)+eps) since gate=sigmoid(..)>0, so the
        # conv input normed = vproj*alpha with ONE shared alpha row; only the
        # residual (value = vproj*gate) needs the per-group gate.
        def emit_alpha():
            # sv (= sum vproj^2) sits at psum row 12: bounce the block to
            # SBUF and matmul-extract the row to partition 0.
            s3a = rowm.tile([16, NTOK], F32R, name="s3a", tag="svz")
            aln = rowsc.tile([1, NTOK], F32, name="aln", tag="rs")
            alpha16 = rowm.tile([1, NTOK], F16, name="alpha16", tag="alpha16")
            nc.scalar.activation(s3a, sums_all[:, :], AF.Copy)
            for ch in range(NCH):
                cols = slice(ch * CHW, (ch + 1) * CHW)
                p = epsum.tile([1, CHW], F32, name="svx", tag="mm")
                nc.tensor.matmul(p, cepr_sb[0:16, 20:21], s3a[:, cols],
                                 start=True, stop=True)
                nc.scalar.activation(aln[:, cols], p, AF.Ln,
                                     bias=ceps_sb[0:1, 6:7],
                                     scale=1.0 / float(C))
            nc.scalar.activation(alpha16, aln, AF.Exp, scale=-0.5)
            return alpha16

        def make_d_tiles(stage):
            T = {}
            for nm in ("p4", "lnp", "lnd", "lng", "sqg", "sgn", "ss4", "ab4",
                       "akr"):
                T[nm] = rowsc.tile([1, NTOK], F32, name=f"{nm}{stage}",
                                   tag="rs")
            T["gate16"] = rowm.tile([1, NTOK], F16, name=f"gate16{stage}",
                                    tag="gate16")
            T["s3"] = rowm.tile([16, NTOK], F32R, name=f"s3_{stage}",
                                tag="ext")
            return T

        def emit_d_s3(stage, T, chs=(0, 1)):
            """Psum sums -> partition-0-based SBUF bounce (+ stage biases)."""
            for ch in chs:
                sl = slice(ch * CHW, (ch + 1) * CHW)
                nc.scalar.activation(T["s3"][:, sl], sums_all[:, sl],
                                     AF.Identity,
                                     bias=ceps_sb[0:16, stage:stage + 1],
                                     scale=1.0)

        def emit_d(stage, T, mul_eng=None, chs=(0, 1)):
            """Per-group gate chain: gate = sigmoid(sign(dot)*sqrt(|graw|)).

            Engines only address partitions at 32-boundaries, so the psum
            region is Act-copied (aligned base -> partition 0) to s3, and
            rows 1+ are pulled to partition-0 psum via one-hot matmuls.
            Row layout: stages 0-2: [ak, aq, dot]; stage 3: [sv, aq, dot, ak].
            """
            me = mul_eng if mul_eng is not None else nc.vector
            s3 = T["s3"]
            p4, lnp, lnd, lng, sqg, sgn, ss4, ab4, gate16 = (
                T["p4"], T["lnp"], T["lnd"], T["lng"], T["sqg"], T["sgn"],
                T["ss4"], T["ab4"], T["gate16"])
            akr = T["akr"]
            if chs == (0, 1):
                sls = [slice(0, NTOK)]
            else:
                sls = [slice(ch * CHW, (ch + 1) * CHW) for ch in chs]

            def extract(row, ch):
                sel = cepr_sb[0:16, 8 + row:9 + row]
                p = epsum.tile([1, CHW], F32, name=f"x{row}_{stage}",
                               tag="mm")
                nc.tensor.matmul(p, sel,
                                 s3[:, ch * CHW:(ch + 1) * CHW],
                                 start=True, stop=True)
                return p

            # first layer reads the [1, CHW] psums (partition 0), per chunk
            for ch in chs:
                cols = slice(ch * CHW, (ch + 1) * CHW)
                ak_ps = extract(stage, ch)
                aq_ps = extract(4 + stage, ch)
                dot_ps = extract(8 + stage, ch)
                nc.scalar.activation(akr[:, cols], ak_ps, AF.Copy)
                nc.scalar.activation(ab4[:, cols], dot_ps, AF.Square)
                nc.scalar.activation(sgn[:, cols], dot_ps, AF.Sign)
                nc.vector.tensor_mul(p4[:, cols], akr[:, cols], aq_ps)
            # 2ln|dot| and ln(p4/C); 2ln|graw| = 2ln|dot| - ln(p4/C)
            # (plain subtract so the mul engine can be Pool)
            for sl in sls:
                nc.scalar.activation(lnd[:, sl], ab4[:, sl], AF.Ln,
                                     bias=ceps_sb[0:1, 7:8])
            for sl in sls:
                nc.scalar.activation(lnp[:, sl], p4[:, sl], AF.Ln,
                                     scale=1.0 / float(C))
            for sl in sls:
                me.tensor_sub(lng[:, sl], lnd[:, sl], lnp[:, sl])
            for sl in sls:
                nc.scalar.activation(sqg[:, sl], lng[:, sl], AF.Exp,
                                     scale=0.25)
            for sl in sls:
                me.tensor_mul(ss4[:, sl], sqg[:, sl], sgn[:, sl])
            for sl in sls:
                nc.scalar.activation(gate16[:, sl], ss4[:, sl], AF.Sigmoid)
            return gate16

        # ---------- stage E ----------
        def bcast_ch(src, dst, ch):
            bp = epsum.tile([128, CHW], F32, name="bp", tag="mm")
            nc.tensor.matmul(
                bp, bc2_sb[0:1, 0:128],
                src[:, ch * CHW:(ch + 1) * CHW],
                start=True, stop=True,
            )
            nc.scalar.activation(
                dst[:, ch * CHW:(ch + 1) * CHW], bp, AF.Copy)

        def bcast_row(src, tag):
            """[1, NTOK] f32/f16 row -> [128, NTOK] f16 via PE broadcast."""
            dst = rowm.tile([128, NTOK], F16, name=f"b_{tag}", tag=tag)
            for ch in range(NCH):
                bcast_ch(src, dst, ch)
            return dst

        # nx16[ct]: f16 conv input, shared by all 4 groups' units:
        # [9 zeros | vproj*alpha]; the halo contribution to the first 9
        # outputs is a host-computed f16 correction (hc) accumulated via an
        # identity matmul.
        PADW = HALO + NTOK
        nx8s = {}

        def emit_nx8(ct):
            nx8 = npool.tile([128, PADW], F16, name=f"nx16_{ct}",
                             tag=f"nx16_{ct}", bufs=1)
            nc.gpsimd.memset(nx8[:, 0:HALO], 0.0)
            nc.vector.tensor_mul(nx8[:, HALO:HALO + NTOK],
                                 vproj16[:, ct, :], ab16)
            nx8s[ct] = nx8

        def emit_val(gct, gb16, on_pool=False):
            ct = gct % NCT
            val = vpool.tile([128, NTOK], F16, name="val", tag="val")
            if on_pool:
                nc.gpsimd.tensor_mul(val, vproj16[:, ct, :], gb16)
            else:
                nc.vector.tensor_mul(val, vproj16[:, ct, :], gb16)
            return val

        def emit_e_conv_pe(gct, pools=None):
            """f16 conv taps + halo-fix matmul."""
            ct = gct % NCT
            nx8 = nx8s[ct]
            dg_t = dgpool.tile([128, KT * 128], F16, name="dg_t", tag="dg")
            nc.sync.dma_start(out=dg_t, in_=dg16[gct])
            accs = []
            for ch in range(NCH):
                pool = (pools[ch % len(pools)] if pools else epsum)
                acc = pool.tile([128, CHW], F32, name="acc", tag="mm")
                for k in range(KT):
                    base = ch * CHW + k * DIL
                    nc.tensor.matmul(
                        acc,
                        dg_t[:, k * 128:(k + 1) * 128],
                        nx8[:, base:base + CHW],
                        start=(k == 0), stop=(k == KT - 1 and ch == 1),
                        skip_group_check=True,
                    )
                if ch == 0:
                    nc.tensor.matmul(
                        acc[:, 0:HALO], id16_sb, hc_sb[:, gct, :],
                        start=False, stop=True, skip_group_check=True,
                    )
                accs.append(acc)
            return accs

        def emit_e_conv_dve(gct):
            """f16 conv as DVE scalar-ptr MAC chains (+ in-place halo fix)."""
            ct = gct % NCT
            nx8 = nx8s[ct]
            outs = []
            for ch in range(NCH):
                prev = None
                for k in range(KT):
                    win = nx8[:, ch * CHW + k * DIL:ch * CHW + k * DIL + CHW]
                    a = cacc.tile([128, CHW], F16, name=f"ca{k}", tag=f"ca{k}")
                    wcol = cwf_sb[:, gct * KT + k:gct * KT + k + 1]
                    if k == 0:
                        nc.vector.tensor_scalar_mul(a, win, wcol)
                    else:
                        nc.vector.scalar_tensor_tensor(
                            a, win, wcol, prev, op0=ALU.mult, op1=ALU.add)
                    prev = a
                if ch == 0:
                    nc.vector.tensor_tensor(prev[:, 0:HALO], prev[:, 0:HALO],
                                            hc_sb[:, gct, :], op=ALU.add)
                outs.append(prev)
            return outs

        def emit_silu(accs):
            sacc = opool.tile([128, NTOK], F16, name="sacc", tag="sacc")
            for ch in range(NCH):
                nc.scalar.activation(sacc[:, ch * CHW:(ch + 1) * CHW],
                                     accs[ch], AF.Silu)
            return sacc

        def emit_resid_out(gct, val, sacc, engine="pool"):
            ot = opool.tile([128, NTOK], F16, name="ot", tag="ot")
            if engine == "dve":
                nc.vector.tensor_tensor(ot, val, sacc, op=ALU.add)
            else:
                nc.gpsimd.tensor_add(ot, val, sacc)
            nc.sync.dma_start(out=out_d[gct * 128:(gct + 1) * 128, :], in_=ot)

        # ---------- pipeline ----------
        # conv+silu only needs the shared ab16; val/resid needs gate(g).
        # Window g: C(g) + chain(g-1) + full units of group g-1 + a few
        # group-3 conv units pulled early; tail: 3 conv units cover chain(3),
        # then group-3 val/resid.
        sacc3 = {}      # gct -> long-lived sacc for group-3 units
        ab16 = None
        TAIL3 = [29, 30, 31]
        EARLY3 = {0: [24, 25, 26], 1: [27], 2: [], 3: []}

        def conv_unit(u, long_lived=False, pools=None, defer_silu=False,
                      dve=False):
            if dve:
                accs = emit_e_conv_dve(u)
            else:
                accs = emit_e_conv_pe(u, pools=pools)
            if defer_silu:
                return accs
            if long_lived:
                sacc = opool.tile([128, NTOK], F16, name=f"sacc{u}",
                                  tag=f"sacc3_{u}", bufs=1)
            else:
                sacc = opool.tile([128, NTOK], F16, name=f"sacc{u}",
                                  tag="sacc")
            for ch in range(NCH):
                nc.scalar.activation(sacc[:, ch * CHW:(ch + 1) * CHW],
                                     accs[ch], AF.Silu)
            if long_lived:
                sacc3[u] = sacc
            return sacc

        def full_unit(u, gb16, dve=False):
            sacc = conv_unit(u, dve=dve)
            val = emit_val(u, gb16)
            emit_resid_out(u, val, sacc, engine="pool" if dve else "dve")

        gate_prev = None
        d_tiles = {}
        for g in range(3):
            dr_prev = None
            gb16 = None
            units = list(range((g - 1) * 8, g * 8)) if g else []
            for i, gg in enumerate(range(g * 4, (g + 1) * 4)):
                if i == 0 and g:
                    d_tiles[g - 1] = make_d_tiles(g - 1)
                    emit_d_s3(g - 1, d_tiles[g - 1])
                cur = emit_c_kq(g, gg)
                if i == 0:
                    if g == 0:
                        alpha16 = emit_alpha()
                    else:
                        gate_prev = emit_d(g - 1, d_tiles[g - 1],
                                           mul_eng=nc.gpsimd)
                if dr_prev is not None:
                    emit_dr(g, *dr_prev, last_gg=False)
                dr_prev = cur
                if i == 1:
                    if g == 0:
                        ab16 = bcast_row(alpha16, "ab16")
                        for ct in (0, 1, 2):
                            emit_nx8(ct)
                    elif g == 1:
                        for ct in (6, 7):
                            emit_nx8(ct)
                    batch = []
                elif i == 2:
                    if g:
                        gb16 = bcast_row(gate_prev, f"gb{g - 1}")
                        batch = units[0:3]
                    else:
                        emit_nx8(3)
                        batch = EARLY3[0][0:2]
                elif i == 3:
                    if g == 0:
                        emit_nx8(4)
                        emit_nx8(5)
                    batch = units[3:6] if g else EARLY3[0][2:3]
                else:
                    batch = []
                for u in batch:
                    if g:
                        full_unit(u, gb16, dve=(u % 8 in (0, 2, 4)))
                    else:
                        conv_unit(u, long_lived=True, dve=True)
            emit_dr(g, *dr_prev, last_gg=True)
            if g:
                for u in units[6:8]:
                    full_unit(u, gb16, dve=(u % 8 == 6))
                for u in EARLY3[g]:
                    conv_unit(u, long_lived=True)

        # ---------- window 3: chunk-split passes ----------
        # pass p computes C(3) for token chunk p only, so the stage-3 gate
        # chain + group-3 epilogue for chunk 0 hide under pass 1.
        units = list(range(16, 24))
        d_tiles[2] = make_d_tiles(2)
        emit_d_s3(2, d_tiles[2])
        T3 = None
        gb3 = rowm.tile([128, NTOK], F16, name="b_gb3", tag="gb3")

        def epi3_ch(u, ch):
            ct = u % NCT
            cols = slice(ch * CHW, (ch + 1) * CHW)
            val = vpool.tile([128, CHW], F16, name="val3", tag="val")
            nc.vector.tensor_mul(val, vproj16[:, ct, cols], gb3[:, cols])
            ot = opool.tile([128, CHW], F16, name="ot3", tag="ot")
            nc.vector.tensor_tensor(ot, val, sacc3[u][:, cols], op=ALU.add)
            nc.sync.dma_start(out=out_d[u * 128:(u + 1) * 128, cols], in_=ot)

        for p in range(2):
            dr_prev = None
            for i, gg in enumerate(range(12, 16)):
                cur = emit_c_kq1(3, gg, p)
                if p == 0 and i == 0:
                    gate2 = emit_d(2, d_tiles[2], mul_eng=nc.gpsimd)
                if p == 1 and i == 0:
                    T3 = make_d_tiles(3)
                    emit_d_s3(3, T3, chs=(0,))
                    gate3 = emit_d(3, T3, mul_eng=nc.vector, chs=(0,))
                if dr_prev is not None:
                    emit_dr1(3, *dr_prev, p, last_gg=False)
                dr_prev = cur
                if p == 0:
                    if i == 2:
                        gb2 = bcast_row(gate2, "gb2")
                        batch = units[0:3]
                    elif i == 3:
                        batch = units[3:6]
                    else:
                        batch = []
                    for u in batch:
                        full_unit(u, gb2, dve=(u % 8 in (0, 2, 4)))
                else:
                    if i == 1:
                        bcast_ch(gate3, gb3, 0)
                        for u in units[6:8]:
                            full_unit(u, gb2, dve=(u % 8 == 6))
                    elif i == 2:
                        for u in range(24, 28):
                            epi3_ch(u, 0)
                    elif i == 3:
                        pass
            emit_dr1(3, *dr_prev, p, last_gg=True)

        # ---------- tail: chunk 1 of the group-3 gate + epilogue ----------
        # TAIL3 conv matmuls cover the chain; their silus follow its Act ops
        emit_d_s3(3, T3, chs=(1,))
        acc_pools = [epsum, mmp]
        emit_d(3, T3, mul_eng=nc.vector, chs=(1,))
        TAIL4 = [28] + TAIL3
        tail_accs = [conv_unit(u, pools=acc_pools, defer_silu=True)
                     for u in TAIL4]
        bcast_ch(gate3, gb3, 1)
        for j, u in enumerate(TAIL4):
            sacc = opool.tile([128, NTOK], F16, name=f"sacc{u}",
                              tag=f"sacc3_{u}", bufs=1)
            for ch in range(NCH):
                nc.scalar.activation(sacc[:, ch * CHW:(ch + 1) * CHW],
                                     tail_accs[j][ch], AF.Silu)
            sacc3[u] = sacc
        for u in TAIL4:
            epi3_ch(u, 0)
        for u in range(24, 32):
            epi3_ch(u, 1)

        for p in (cacc, dgpool, opool, vpool, npool, rowsc, rowm, kqpool, scr,
                  epsum, sump, mmp, qpool, kwpool, consts):
            p.release()
    return nc


def host_prep(embeddings, hidden_states, key_w, key_b, value_w, value_b,
              w_key_norm, w_query_norm, w_norm, conv_weight):
    """Build the per-core input maps."""
    f32, f16 = np.float32, np.float16
    e4 = ml_dtypes.float8_e4m3fn
    embeddings = np.asarray(embeddings, f32)
    hidden_states = np.asarray(hidden_states, f32)
    key_w = np.asarray(key_w, f32)
    key_b = np.asarray(key_b, f32)
    value_w = np.asarray(value_w, f32)
    value_b = np.asarray(value_b, f32)
    w_key_norm = np.asarray(w_key_norm, f32)
    w_query_norm = np.asarray(w_query_norm, f32)
    w_norm = np.asarray(w_norm, f32)
    conv_weight = np.asarray(conv_weight, f32)

    kwT = np.ascontiguousarray(key_w.T).astype(f16)        # [E, GC]
    vwT = np.ascontiguousarray(value_w.T).astype(f16)      # [E, C]
    keyb_r = np.ascontiguousarray(key_b.reshape(NGCT, 128).T)  # [128, NGCT]
    valb_r = np.ascontiguousarray(value_b.reshape(NCT, 128).T)
    wkq = (w_key_norm * w_query_norm).reshape(GC)

    # one-hot lhsT tables. ONE shared [16, NTOK] psum accumulator with
    # disjoint rows: ak_g = row g, aq_g = 4+g, dot_g = 8+g, sv = 12.
    # (engines can only address 32-aligned partition bases, so rows are
    #  matmul-extracted after an Act bounce of the block to partition 0)
    lk8 = np.zeros((128, 9, 2, 16), f32)
    for g in range(G):
        lk8[:, g, :, g] = 1.0          # ksq -> row g
        lk8[:, 4 + g, :, 4 + g] = 1.0  # qsq -> row 4+g
    lk8[:, 8, :, 12] = 1.0             # vsq -> row 12 (sv)
    lk8 = lk8.reshape(128, 288).astype(e4)

    lkq = np.zeros((NGCT, 128, 16), f32)
    for gct in range(NGCT):
        g = gct // NCT
        lkq[gct, :, 8 + g] = wkq[gct * 128:(gct + 1) * 128]

    aux16 = np.zeros((128, 16 + 256), f16)
    aux16[:, 12] = 1.0        # lv one-hot: vsq -> row 12 (sv)
    for j in range(2):
        aux16[j, 16 + j * 128:16 + (j + 1) * 128] = 1.0

    # ceps: cols 0-3 = per-stage bias vectors (+C*EPS on ak/aq rows);
    #        cols 8+r = f32 one-hot row selectors (identity)
    ceps_h = np.zeros((128, 24), f32)
    for g in range(G):
        ceps_h[g, g] = float(C) * EPS
        ceps_h[4 + g, g] = float(C) * EPS
    for r in range(16):
        ceps_h[r, 8 + r] = 1.0
    ceps_h[0, 6] = NORM_EPS
    ceps_h[0, 7] = 1e-60

    # f16 diagonal conv weights + identity for the halo-fix matmul.
    cwf = (conv_weight.reshape(G, C, KT) * w_norm[:, :, None]).astype(f32)
    dg = np.zeros((NGCT, 128, KT * 128), f16)
    idx = np.arange(128)
    for gct in range(NGCT):
        g, ct = gct // NCT, gct % NCT
        for k in range(KT):
            dg[gct, idx, k * 128 + idx] = cwf[g, ct * 128 + idx, k].astype(f16)
    id16_h = np.zeros((128, 128), f16)
    id16_h[idx, idx] = 1.0
    cwf_r = np.zeros((128, NGCT * KT), f32)
    for gct in range(NGCT):
        g, ct = gct // NCT, gct % NCT
        for k in range(KT):
            cwf_r[:, gct * KT + k] = cwf[g, ct * 128:(ct + 1) * 128, k]

    in_maps = []
    for core in range(NCORES):
        b = core // (NCORES // B)
        t0 = (core % (NCORES // B)) * NTOK
        emb_s = embeddings[b, t0:t0 + NTOK]                # [NTOK, E]
        hid_s = hidden_states[b, t0:t0 + NTOK].reshape(NTOK, GC)
        emb_c = np.ascontiguousarray(emb_s.T).astype(f16)  # [E, NTOK]
        hid_c = np.ascontiguousarray(hid_s.T).astype(f16)  # [GC, NTOK]

        # halo: nhat (= value / rms_v, w_norm NOT applied) for the 9
        # preceding tokens feeds a host-computed conv correction hc for the
        # first 9 output tokens; zeros at the sequence start.
        if t0 == 0:
            hc_c = np.zeros((128, NGCT * HALO), f16)
        else:
            th = slice(t0 - HALO, t0)
            e9 = embeddings[b, th]                          # [9, E]
            k9 = (e9 @ key_w.T + key_b).reshape(HALO, G, C)
            q9 = hidden_states[b, th]                       # [9, G, C]
            rk = np.sqrt((k9 * k9).mean(-1) + EPS)
            rq = np.sqrt((q9 * q9).mean(-1) + EPS)
            d9 = np.einsum("tgc,gc,tgc,gc->tg", k9, w_key_norm, q9,
                           w_query_norm)
            graw = d9 / (rk * rq) / np.sqrt(f32(C))
            g9 = 1.0 / (1.0 + np.exp(-(np.where(graw >= 0, 1.0, -1.0)
                                       * np.sqrt(np.maximum(np.abs(graw),
                                                            1e-6)))))
            vp9 = e9 @ value_w.T + value_b                  # [9, C]
            val9 = vp9[:, None, :] * g9[..., None].astype(f32)
            rv9 = np.sqrt((val9 * val9).mean(-1) + NORM_EPS)
            nhat9 = val9 / rv9[..., None]                   # [9, G, C]
            # hc[c, gct, t] = sum_{k: t+k*DIL<9} cwf[g,c,k]*nhat9[t+k*DIL,g,c]
            hcf = np.zeros((HALO, G, C), f32)
            for t in range(HALO):
                for k in range(KT):
                    ix = t + k * DIL
                    if ix < HALO:
                        hcf[t] += cwf[:, :, k] * nhat9[ix]
            hg = hcf.transpose(1, 2, 0).reshape(NGCT, 128, HALO)
            hc_c = np.ascontiguousarray(
                hg.transpose(1, 0, 2).reshape(128, NGCT * HALO)).astype(f16)

        in_maps.append({
            "emb16": emb_c, "hidT": hid_c, "kwT": kwT, "vwT": vwT,
            "keyb": keyb_r, "valb": valb_r,
            "lk8": lk8, "lkq": lkq, "aux16": aux16, "ceps": ceps_h,
            "dg16": dg, "cwf": cwf_r, "hc": hc_c, "id16": id16_h,
        })
    return in_maps


_NC_CACHE = [None]
LAST_RESULT = [None]


def kernel(**inputs) -> np.ndarray:
    in_maps = host_prep(**inputs)
    if _NC_CACHE[0] is None:
        _NC_CACHE[0] = build_program()
    nc = _NC_CACHE[0]
    res = run_bass_kernel_spmd(nc, in_maps, list(range(NCORES)))
    LAST_RESULT[0] = res
    out = np.empty((B, T, G, C), np.float32)
    for core in range(NCORES):
        b = core // (NCORES // B)
        t0 = (core % (NCORES // B)) * NTOK
        oc = np.asarray(res.results[core]["out"]).astype(np.float32)
        out[b, t0:t0 + NTOK] = oc.reshape(G, C, NTOK).transpose(2, 0, 1)
    return out
